# revision 1
# baseline (speedup 1.0000x reference)
"""Trainium2 Bass kernel for nn_Loss_dict_50646254354805 (NeRF-style loss).

Self-contained: accepts FULL inputs, shards across 8 NeuronCores (rays for
the per-ray losses, samples for the hash loss), runs one SPMD Bass module,
host-sums the 8 partial scalars.

Inter-loss algorithm (per ray, per prop level): the reference's
blur_step_function + sorted_interp_quad is reproduced exactly in a "merged
domain": tag query/event values in 2 mantissa LSBs, bitonic-merge the events
(sdist-+pw) with the prop_sdist queries, rebuild the blurred-density CDF with
prefix scans (matching the reference's cumsum structure), and compact the CDF
at query slots with per-partition local_scatter. No per-ray gather needed.
"""
import numpy as np

import concourse.bass as bass
import concourse.mybir as mybir
import concourse.tile as tile
from concourse import bacc
from concourse.bass_utils import run_bass_kernel_spmd

dt = mybir.dt
Alu = mybir.AluOpType
AX = mybir.AxisListType
P = 128

# problem constants
PULSE = (0.01, 0.005)
W_RGB, W_INTER, W_DIST, W_HASH = 1.0, 1.0, 0.01, 0.1
NUM_SEGMENTS = 65536
R, N = 4096, 48
M = R * N
N_CORES = 8
RPC = R // N_CORES            # rays per core (512)
NBLK = RPC // P               # ray tiles per core (4)
MPC = M // N_CORES            # hash samples per core (24576)
HALO = 64                     # hash run halo
HROW = MPC // P               # hash samples per partition (192)
HCOLS = HROW + HALO + 1       # loaded cols per partition (257)
HSLICE = HALO + MPC + HALO    # per-core hash slice length (24704)

# per-level geometry
LVL = {0: dict(X=257, n2=512), 1: dict(X=97, n2=256)}
for _L in LVL.values():
    _L["EW"] = ((_L["X"] + 98 + 1 + 7) // 8) * 8        # 360 / 200
    _L["QW"] = _L["n2"] - 96                            # 416 / 160
BIGF = 3.0                    # merge pad value (> max real value ~2.02)


def _ts_int(eng, out, in0, imm1, op0, imm2=None, op1=None):
    """tensor_scalar with int32 immediates (for bitwise ops)."""
    ins_ = [eng.lower_ap(in0), mybir.ImmediateValue(dtype=dt.int32, value=int(imm1))]
    kw = dict(op0=op0)
    if imm2 is not None:
        ins_.append(mybir.ImmediateValue(dtype=dt.int32, value=int(imm2)))
        kw["op1"] = op1
    return eng.add_instruction(mybir.InstTensorScalarPtr(
        name=eng.bass.get_next_instruction_name(),
        ins=ins_, outs=[eng.lower_ap(out)], **kw))


def _bcast_row(nc, dst_ap, src_ap, n, eng=None):
    """DMA a replicated HBM const [P, n] into dst [P, n]."""
    eng = eng or nc.scalar
    eng.dma_start(dst_ap, src_ap[:, 0:n])


def _blk(ap, n2):
    """[P, NBLK*n2] AP -> [P, NBLK, n2] view."""
    return ap.rearrange("p (b n) -> p b n", b=NBLK)


def _bitonic_merge(eng, bufa, bufb, width, descending):
    """Ping-pong bitonic merge over [P, NBLK*width] f32-viewed int tiles.

    Returns (result_buf, scratch_buf)."""
    cur, nxt = bufa, bufb
    d = width // 2
    while d >= 1:
        c3 = cur[:].bitcast(dt.float32).rearrange("p (c td) -> p c td", td=2 * d)
        n3 = nxt[:].bitcast(dt.float32).rearrange("p (c td) -> p c td", td=2 * d)
        lo_in, hi_in = c3[:, :, 0:d], c3[:, :, d:2 * d]
        if descending:
            eng.tensor_tensor(n3[:, :, 0:d], lo_in, hi_in, Alu.max)
            eng.tensor_tensor(n3[:, :, d:2 * d], lo_in, hi_in, Alu.min)
        else:
            eng.tensor_tensor(n3[:, :, 0:d], lo_in, hi_in, Alu.min)
            eng.tensor_tensor(n3[:, :, d:2 * d], lo_in, hi_in, Alu.max)
        cur, nxt = nxt, cur
        d //= 2
    return cur, nxt


def _split_u16(nc, ap_f32_src, lo_dst, hi_dst):
    vu = ap_f32_src.bitcast(dt.uint16).rearrange("p (n two) -> p n two", two=2)
    nc.gpsimd.tensor_copy(lo_dst, vu[:, :, 0])
    nc.gpsimd.tensor_copy(hi_dst, vu[:, :, 1])


def _emit_level(nc, tc, pool, lvl, s_sh, radio, x_ap, pwt_ap, inter_acc, aps,
                VM=None, VS=None, VE=None):
    """Inter-loss pipeline for one prop level. Careful manual buffer reuse.

    Post-merge work runs in a compact [NBLK, LW] layout (LW = EW + 24) to cut
    scan/elementwise volume; the merge itself needs pow2 [NBLK, n2] blocks."""
    VM = VM or nc.vector
    VS = VS or nc.vector
    VE = VE or nc.vector
    L = LVL[lvl]
    X, n2, EW, QW = L["X"], L["n2"], L["EW"], L["QW"]
    LW = EW + 24
    SL = NBLK * n2
    NL = NBLK * LW
    NEV = NBLK * EW
    pw = PULSE[lvl]

    def blkL(ap):
        return ap.rearrange("p (b n) -> p b n", b=NBLK)

    # ---------- big slots: B0/B1 merge-sized, B2..B7 compact ----------
    B0 = pool.tile([P, SL], dt.float32, tag="big0", name="big0")
    B1 = pool.tile([P, SL], dt.float32, tag="big1", name="big1")
    bigs = []
    for i in range(2, 8):
        b = pool.tile([P, NL], dt.float32, tag=f"big{i}", name=f"big{i}")
        bigs.append(b)
    B2, B3, B4, B5, B6, B7 = bigs

    # ---------- per-level constants ----------
    iota_loc16 = pool.tile([P, SL], dt.int16, tag="iota_loc16")
    _bcast_row(nc, iota_loc16[:], aps[f"c_iota16_l{lvl}"], SL)
    iotaP1f = pool.tile([P, NL], dt.float32, tag="iotaP1f")
    _bcast_row(nc, iotaP1f[:], aps[f"c_iotap1_l{lvl}"], NL)
    mask_scan = pool.tile([P, NL], dt.float32, tag="mask_scan")
    _bcast_row(nc, mask_scan[:], aps[f"c_mask_l{lvl}"], NL)

    # ---------- inputs ----------
    xt = pool.tile([P, NBLK * X], dt.float32, tag="xt")
    nc.sync.dma_start(_blk(xt[:], X), x_ap.rearrange("(b p) x -> p b x", p=P))
    pwt = pool.tile([P, NBLK * (X - 1)], dt.float32, tag="pwt")
    nc.sync.dma_start(_blk(pwt[:], X - 1), pwt_ap.rearrange("(b p) x -> p b x", p=P))

    # ---------- exact shifted event values ----------
    emsh = pool.tile([P, NBLK * 49], dt.float32, tag="emsh")
    nc.scalar.activation(emsh[:], s_sh[:],
                         mybir.ActivationFunctionType.Copy, bias=1.0 - pw)
    epsh = pool.tile([P, NBLK * 49], dt.float32, tag="epsh")
    nc.scalar.activation(epsh[:], s_sh[:],
                         mybir.ActivationFunctionType.Copy, bias=1.0 + pw)

    # ---------- B1 merge: tagged events, descending ----------
    b1a = pool.tile([P, NBLK * 128], dt.int32, tag="b1a")
    b1b = pool.tile([P, NBLK * 128], dt.int32, tag="b1b")
    _bcast_row(nc, b1a[:].bitcast(dt.float32), aps["c_bigf"], NBLK * 128)
    b1a3 = _blk(b1a[:], 128)
    _ts_int(nc.vector, b1a3[:, :, 0:49], _blk(emsh[:], 49).bitcast(dt.int32),
            ~3, Alu.bitwise_and, 1, Alu.bitwise_or)
    ept = pool.tile([P, NBLK * 49], dt.int32, tag="ept")
    _ts_int(nc.vector, ept[:], epsh[:].bitcast(dt.int32), ~3,
            Alu.bitwise_and, 3, Alu.bitwise_or)
    nc.vector.tensor_copy(b1a3[:, :, 79:128].bitcast(dt.float32),
                          _blk(ept[:], 49).bitcast(dt.float32)[:, :, ::-1])
    b1, _ = _bitonic_merge(VM, b1a, b1b, 128, descending=True)

    # ---------- B2 merge: queries + events, ascending ----------
    b2a = B0[:].bitcast(dt.int32)
    _bcast_row(nc, B0[:], aps["c_bigf"], SL)
    xsh = pool.tile([P, NBLK * X], dt.float32, tag="xsh")
    nc.scalar.activation(xsh[:], xt[:],
                         mybir.ActivationFunctionType.Copy, bias=1.0)
    b2a3 = _blk(b2a, n2)
    _ts_int(nc.vector, b2a3[:, :, 0:X], _blk(xsh[:], X).bitcast(dt.int32), ~3,
            Alu.bitwise_and)
    nc.gpsimd.tensor_copy(b2a3[:, :, n2 - 128:n2].bitcast(dt.float32),
                          _blk(b1[:], 128).bitcast(dt.float32))
    SMt, SAt = _bitonic_merge(VM, B0, B1, n2, descending=False)
    m = SMt[:].bitcast(dt.int32)       # merged tagged values, [NBLK, n2] layout
    mS = _blk(m, n2)[:, :, 0:LW]       # strided view of the real+pad prefix
    SA = SAt                           # free merge-sized big

    # ---------- tags (into compact layout) ----------
    tag = B2[:].bitcast(dt.int32)
    _ts_int(nc.vector, tag, mS, 3, Alu.bitwise_and)
    em_f = B3
    _ts_int(nc.vector, em_f[:], tag, 1, Alu.is_equal)
    ep_f = B4
    _ts_int(nc.vector, ep_f[:], tag, 3, Alu.is_equal)
    ev_f = B5
    nc.vector.tensor_tensor(ev_f[:], em_f[:], ep_f[:], Alu.add)

    # ---------- counts ----------
    C = B2                             # overwrites tag (dead)
    VS.tensor_tensor_scan(C[:], mask_scan[:], ev_f[:], 0.0, Alu.mult, Alu.add)
    Cm = B6
    VS.tensor_tensor_scan(Cm[:], mask_scan[:], em_f[:], 0.0, Alu.mult, Alu.add)
    tmpf = B7

    # ---------- event position compaction (block-local slots) ----------
    idx16 = pool.tile([P, NL], dt.int16, tag="idx16")
    pos_m = pool.tile([P, NBLK * 64], dt.int16, tag="pos_m")
    pos_p = pool.tile([P, NBLK * 64], dt.int16, tag="pos_p")
    tmpf3 = blkL(tmpf[:])
    idx163 = blkL(idx16[:])
    C3 = blkL(C[:])
    Cm3 = blkL(Cm[:])
    em3 = blkL(em_f[:])
    ep3 = blkL(ep_f[:])
    for which, pos in ((0, pos_m), (1, pos_p)):
        if which == 0:
            VE.tensor_tensor(tmpf3[:, :, 0:EW], Cm3[:, :, 0:EW],
                             em3[:, :, 0:EW], Alu.mult)
        else:
            VE.tensor_tensor(tmpf3[:, :, 0:EW], C3[:, :, 0:EW],
                             Cm3[:, :, 0:EW], Alu.subtract)
            VE.tensor_tensor(tmpf3[:, :, 0:EW], tmpf3[:, :, 0:EW],
                             ep3[:, :, 0:EW], Alu.mult)
        nc.scalar.activation(idx163[:, :, 0:EW], tmpf3[:, :, 0:EW],
                             mybir.ActivationFunctionType.Copy, bias=-1.0)
        for b in range(NBLK):
            nc.gpsimd.local_scatter(pos[:, b * 64:(b + 1) * 64],
                                    iota_loc16[:, b * n2:b * n2 + EW],
                                    idx16[:, b * LW:b * LW + EW], channels=P,
                                    num_elems=64, num_idxs=EW)

    # ---------- radio + exact event value scatters (targets in LW coords) ----
    tgt16 = pool.tile([P, NBLK * 128], dt.int16, tag="tgt16")
    t3 = _blk(tgt16[:], 128)
    for b in range(NBLK):
        _ts_int(nc.vector, t3[:, b, 0:49], pos_m[:, b * 64:b * 64 + 49],
                b * LW, Alu.add)
        _ts_int(nc.vector, t3[:, b, 49:98], pos_p[:, b * 64:b * 64 + 49],
                b * LW, Alu.add)
    nc.gpsimd.memset(t3[:, :, 98:128], -1)

    radcat = pool.tile([P, NBLK * 128], dt.float32, tag="radcat")
    nc.gpsimd.memset(_blk(radcat[:], 128)[:, :, 98:128], 0.0)
    r3 = _blk(radcat[:], 128)
    nc.vector.tensor_copy(r3[:, :, 0:49], _blk(radio[:], 49))
    nc.vector.tensor_scalar(r3[:, :, 49:98], _blk(radio[:], 49), -1.0, None,
                            Alu.mult)
    evcat = pool.tile([P, NBLK * 128], dt.float32, tag="evcat")
    nc.gpsimd.memset(_blk(evcat[:], 128)[:, :, 98:128], 0.0)
    e3 = _blk(evcat[:], 128)
    nc.vector.tensor_copy(e3[:, :, 0:49], _blk(emsh[:], 49))
    nc.vector.tensor_copy(e3[:, :, 49:98], _blk(epsh[:], 49))

    lo_s = pool.tile([P, NBLK * 128], dt.uint16, tag="lo_s")
    hi_s = pool.tile([P, NBLK * 128], dt.uint16, tag="hi_s")
    b7u = B7[:].bitcast(dt.uint16)
    rad_lo = b7u[:, 0:NL]
    rad_hi = b7u[:, NL:2 * NL]
    vev_lo_t = pool.tile([P, NL], dt.uint16, tag="vev_lo")
    vev_hi_t = pool.tile([P, NL], dt.uint16, tag="vev_hi")
    _split_u16(nc, radcat[:], lo_s[:], hi_s[:])
    nc.gpsimd.local_scatter(rad_lo, lo_s[:], tgt16[:], channels=P,
                            num_elems=NL, num_idxs=NBLK * 128)
    nc.gpsimd.local_scatter(rad_hi, hi_s[:], tgt16[:], channels=P,
                            num_elems=NL, num_idxs=NBLK * 128)
    _split_u16(nc, evcat[:], lo_s[:], hi_s[:])
    nc.gpsimd.local_scatter(vev_lo_t[:], lo_s[:], tgt16[:], channels=P,
                            num_elems=NL, num_idxs=NBLK * 128)
    nc.gpsimd.local_scatter(vev_hi_t[:], hi_s[:], tgt16[:], channels=P,
                            num_elems=NL, num_idxs=NBLK * 128)

    # ---------- recombine radio into compact layout (ls pre-zeroed dests) ----
    radio_m = SA[:][:, 0:NL]
    rm_u = radio_m.bitcast(dt.uint16).rearrange("p (n two) -> p n two", two=2)
    nc.gpsimd.tensor_copy(rm_u[:, :, 0], rad_lo)
    nc.gpsimd.tensor_copy(rm_u[:, :, 1], rad_hi)

    # ---------- slope scan ----------
    g = B4                             # ep_f dead
    VS.tensor_tensor_scan(g[:], mask_scan[:], radio_m, 0.0, Alu.mult, Alu.add)

    # ---------- v: cleaned values, event slots replaced by exact values ------
    v = B3                             # em_f dead
    _ts_int(nc.vector, v[:].bitcast(dt.int32), mS, ~3, Alu.bitwise_and)
    vev32 = SA[:][:, 0:NL]             # radio_m dead (after g scan)
    vv_u = vev32.bitcast(dt.uint16).rearrange("p (n two) -> p n two", two=2)
    nc.gpsimd.tensor_copy(vv_u[:, :, 0], vev_lo_t[:])
    nc.gpsimd.tensor_copy(vv_u[:, :, 1], vev_hi_t[:])
    one_m_ev = B7                      # rad halves consumed by recombine above
    nc.scalar.activation(one_m_ev[:], ev_f[:],
                         mybir.ActivationFunctionType.Copy, bias=1.0, scale=-1.0)
    vf3 = blkL(v[:])
    om3 = blkL(one_m_ev[:])
    VE.tensor_tensor(vf3[:, :, 0:EW], vf3[:, :, 0:EW], om3[:, :, 0:EW], Alu.mult)
    VE.tensor_tensor(vf3[:, :, 0:EW], vf3[:, :, 0:EW],
                     blkL(vev32)[:, :, 0:EW], Alu.add)

    # ---------- density reconstruction ----------
    dv = B6                            # Cm dead
    dv3 = blkL(dv[:])
    nc.gpsimd.memset(dv3[:, :, 0:1], 0.0)
    VE.tensor_tensor(dv3[:, :, 1:EW], vf3[:, :, 1:EW], vf3[:, :, 0:EW - 1],
                     Alu.subtract)
    wg = SA[:][:, 0:NL]                # vev32 dead (after v combine)
    wg3 = blkL(wg)
    nc.gpsimd.memset(wg3[:, :, 0:1], 0.0)
    nc.gpsimd.memset(wg3[:, :, EW:LW], 0.0)
    VE.tensor_tensor(wg3[:, :, 1:EW], dv3[:, :, 1:EW],
                     blkL(g[:])[:, :, 0:EW - 1], Alu.mult)
    w = SMt                            # m dead (after v extraction)
    wv = w[:][:, 0:NL]
    VS.tensor_tensor_scan(wv, mask_scan[:], wg, 0.0, Alu.mult, Alu.add)
    wc = B3                            # v dead (after dv)
    nc.scalar.activation(wc[:], wv, mybir.ActivationFunctionType.Relu)
    scr = SA[:][:, 0:NL]               # wg dead (after w scan)
    scr3 = blkL(scr)
    wc3 = blkL(wc[:])
    VE.tensor_tensor(scr3[:, :, 1:EW], wc3[:, :, 1:EW], wc3[:, :, 0:EW - 1],
                     Alu.add)
    area = B4                          # g dead (after wg)
    a3 = blkL(area[:])
    nc.gpsimd.memset(a3[:, :, 0:1], 0.0)
    nc.gpsimd.memset(a3[:, :, EW:LW], 0.0)
    VE.scalar_tensor_tensor(a3[:, :, 1:EW], scr3[:, :, 1:EW], 0.5,
                            dv3[:, :, 1:EW], Alu.mult, Alu.mult)
    cdf = B6                           # dv dead (after area)
    VS.tensor_tensor_scan(cdf[:], mask_scan[:], area[:], 0.0, Alu.mult, Alu.add)

    # ---------- compact cdf at query slots ----------
    qf = SA[:][:, 0:NL]                # scr dead (after area)
    nc.scalar.activation(qf, ev_f[:], mybir.ActivationFunctionType.Copy,
                         bias=1.0, scale=-1.0)
    tmpf = B7                          # one_m_ev value no longer needed
    tmpf3 = blkL(tmpf[:])
    iq3 = blkL(iotaP1f[:])
    qf3 = blkL(qf)
    VE.tensor_tensor(tmpf3[:, :, 0:EW], iq3[:, :, 0:EW], C3[:, :, 0:EW],
                     Alu.subtract)
    VE.tensor_tensor(tmpf3[:, :, 0:EW], tmpf3[:, :, 0:EW], qf3[:, :, 0:EW],
                     Alu.mult)
    nc.scalar.activation(idx163[:, :, 0:EW], tmpf3[:, :, 0:EW],
                         mybir.ActivationFunctionType.Copy, bias=-1.0)
    b5u = B5[:].bitcast(dt.uint16)     # ev_f dead (after qf)
    cdf_lo16 = b5u[:, 0:NL]
    cdf_hi16 = b5u[:, NL:2 * NL]
    cdf_u = cdf[:].bitcast(dt.uint16).rearrange("p (n two) -> p n two", two=2)
    nc.gpsimd.tensor_copy(cdf_lo16, cdf_u[:, :, 0])
    nc.gpsimd.tensor_copy(cdf_hi16, cdf_u[:, :, 1])
    QWS = EW - 98                      # compact dest width (covers pad slots)
    smu = SMt[:].bitcast(dt.uint16)    # w dead (after wc)
    cq_lo = smu[:, 0:NBLK * QWS]
    cq_hi = smu[:, SL:SL + NBLK * QWS]
    for b in range(NBLK):
        nc.gpsimd.local_scatter(cq_lo[:, b * QWS:(b + 1) * QWS],
                                cdf_lo16[:, b * LW:b * LW + EW],
                                idx16[:, b * LW:b * LW + EW], channels=P,
                                num_elems=QWS, num_idxs=EW)
        nc.gpsimd.local_scatter(cq_hi[:, b * QWS:(b + 1) * QWS],
                                cdf_hi16[:, b * LW:b * LW + EW],
                                idx16[:, b * LW:b * LW + EW], channels=P,
                                num_elems=QWS, num_idxs=EW)
    cdfq = B3[:].bitcast(dt.int32)     # wc dead (after scr)
    cq_u = cdfq.bitcast(dt.uint16).rearrange("p (b n two) -> p b n two",
                                             b=NBLK, two=2)
    nc.gpsimd.tensor_copy(cq_u[:, :, 0:X, 0], _blk(cq_lo, QWS)[:, :, 0:X])
    nc.gpsimd.tensor_copy(cq_u[:, :, 0:X, 1], _blk(cq_hi, QWS)[:, :, 0:X])

    # ---------- loss tail ----------
    b4f = B4                           # area dead (after cdf scan)
    NW = NBLK * (X - 1)
    ws = b4f[:][:, 0:NW]
    cqf = cdfq.bitcast(dt.float32).rearrange("p (b n) -> p b n", b=NBLK)
    ws3 = ws.rearrange("p (b n) -> p b n", b=NBLK)
    VE.tensor_tensor(ws3, cqf[:, :, 1:X], cqf[:, :, 0:X - 1], Alu.subtract)
    VE.tensor_tensor(ws, ws, pwt[:], Alu.subtract)
    den = pool.tile([P, NW], dt.float32, tag="dent")
    nc.scalar.activation(den[:], pwt[:], mybir.ActivationFunctionType.Copy,
                         bias=1e-5)
    nc.vector.reciprocal(den[:], den[:])
    rsl = pool.tile([P, NW], dt.float32, tag="rsl")
    nc.scalar.activation(rsl[:], ws, mybir.ActivationFunctionType.Relu)
    VE.tensor_tensor(ws, ws, rsl[:], Alu.mult)
    VE.tensor_tensor(ws, ws, den[:], Alu.mult)
    part = pool.tile([P, 1], dt.float32, tag="part")
    nc.vector.tensor_reduce(part[:], ws3, AX.XY, Alu.add)
    nc.vector.tensor_scalar(inter_acc[:], part[:], 1.0 / (R * (X - 1)), None,
                            Alu.mult)


def build_module(parts=("rgb", "dist", "hash", "l0", "l1")):
    nc = bacc.Bacc("TRN2", target_bir_lowering=False, debug=False,
                   enable_asserts=False, num_devices=N_CORES)
    aps = {}

    def din(name, shape, dtype=dt.float32):
        aps[name] = nc.dram_tensor(name, shape, dtype, kind="ExternalInput").ap()
    din("pd", [RPC, 3]); din("gt", [RPC, 3])
    din("sd", [RPC, 49]); din("rw", [RPC, 48])
    din("ps0", [RPC, 257]); din("pw0", [RPC, 256])
    din("ps1", [RPC, 97]); din("pw1", [RPC, 96])
    din("hi0", [HSLICE], dt.int32); din("he0", [HSLICE * 2])
    din("hi1", [HSLICE], dt.int32); din("he1", [HSLICE * 2])
    din("c_iota16_l0", [P, NBLK * 512], dt.int16)
    din("c_iota16_l1", [P, NBLK * 256], dt.int16)
    din("c_iotap1_l0", [P, NBLK * 384]); din("c_iotap1_l1", [P, NBLK * 224])
    din("c_mask_l0", [P, NBLK * 384]); din("c_mask_l1", [P, NBLK * 224])
    din("c_mask48", [P, NBLK * 48]); din("c_ones", [P, HCOLS])
    din("c_zeros", [P, NBLK * 512]); din("c_bigf", [P, NBLK * 512])
    out_ap = nc.dram_tensor("out", [1, 1], dt.float32, kind="ExternalOutput").ap()

    with tile.TileContext(nc) as tc:
        _emit(nc, tc, aps, out_ap, parts)
    nc.compile()
    return nc


def _emit(nc, tc, aps, out_ap, parts=("rgb", "dist", "hash", "l0", "l1")):
    import contextlib
    with contextlib.ExitStack() as ctx:
        cpool = ctx.enter_context(tc.tile_pool(name="consts", bufs=1))
        mask48 = cpool.tile([P, NBLK * 48], dt.float32, tag="mask48")
        _bcast_row(nc, mask48[:], aps["c_mask48"], NBLK * 48)
        ones_h = cpool.tile([P, HCOLS], dt.float32, tag="ones_h")
        _bcast_row(nc, ones_h[:], aps["c_ones"], HCOLS)

        accs = {}
        for name in ("rgb", "inter", "inter1", "p1", "p2", "hash"):
            a = cpool.tile([P, 1], dt.float32, tag=f"acc_{name}")
            accs[name] = a

        for a in accs.values():
            nc.vector.memset(a[:], 0.0)

        # ---------- shared render tables + radio + dist ----------
        spool = ctx.enter_context(tc.tile_pool(name="shared", bufs=1))
        s_sh = spool.tile([P, NBLK * 49], dt.float32, tag="s_sh")
        nc.sync.dma_start(_blk(s_sh[:], 49),
                          aps["sd"].rearrange("(b p) x -> p b x", p=P))
        radios = {0: spool.tile([P, NBLK * 49], dt.float32, tag="radio0",
                                name="radio0"),
                  1: spool.tile([P, NBLK * 49], dt.float32, tag="radio1",
                                name="radio1")}

        with tc.tile_pool(name="setup", bufs=1) as pool:
            rw_sh = pool.tile([P, NBLK * 48], dt.float32, tag="rw_sh")
            nc.sync.dma_start(_blk(rw_sh[:], 48),
                              aps["rw"].rearrange("(b p) x -> p b x", p=P))
            s3 = _blk(s_sh[:], 49)
            ds = pool.tile([P, NBLK * 48], dt.float32, tag="ds")
            nc.vector.tensor_tensor(_blk(ds[:], 48), s3[:, :, 1:49],
                                    s3[:, :, 0:48], Alu.subtract)
            dse = pool.tile([P, NBLK * 48], dt.float32, tag="dse")
            nc.vector.tensor_scalar(dse[:], ds[:], 1e-8, None, Alu.add)
            wnorm = pool.tile([P, NBLK * 48], dt.float32, tag="wnorm")
            nc.vector.reciprocal(dse[:], dse[:])
            nc.vector.tensor_tensor(wnorm[:], rw_sh[:], dse[:], Alu.mult)
            wnp = pool.tile([P, NBLK * 50], dt.float32, tag="wnp")
            nc.vector.memset(wnp[:], 0.0)
            nc.vector.tensor_copy(_blk(wnp[:], 50)[:, :, 1:49], _blk(wnorm[:], 48))
            diff = pool.tile([P, NBLK * 49], dt.float32, tag="diff")
            wnp3 = _blk(wnp[:], 50)
            nc.vector.tensor_tensor(_blk(diff[:], 49), wnp3[:, :, 1:50],
                                    wnp3[:, :, 0:49], Alu.subtract)
            for lvl in (0, 1):
                nc.vector.tensor_scalar(radios[lvl][:], diff[:],
                                        1.0 / (2 * PULSE[lvl]), None, Alu.mult)

            # distortion
            mid = pool.tile([P, NBLK * 48], dt.float32, tag="mid")
            nc.vector.tensor_tensor(_blk(mid[:], 48), s3[:, :, 1:49],
                                    s3[:, :, 0:48], Alu.add)
            nc.vector.tensor_scalar(mid[:], mid[:], 0.5, None, Alu.mult)
            wm = pool.tile([P, NBLK * 48], dt.float32, tag="wm")
            nc.vector.tensor_tensor(wm[:], rw_sh[:], mid[:], Alu.mult)
            Cin = pool.tile([P, NBLK * 48], dt.float32, tag="Cin")
            nc.vector.tensor_tensor_scan(Cin[:], mask48[:], rw_sh[:], 0.0,
                                         Alu.mult, Alu.add)
            Sin = pool.tile([P, NBLK * 48], dt.float32, tag="Sin")
            nc.vector.tensor_tensor_scan(Sin[:], mask48[:], wm[:], 0.0,
                                         Alu.mult, Alu.add)
            A = pool.tile([P, NBLK * 47], dt.float32, tag="A47")
            m3 = _blk(mid[:], 48)
            c3 = _blk(Cin[:], 48)
            sw3 = _blk(Sin[:], 48)
            rw3 = _blk(rw_sh[:], 48)
            A3 = _blk(A[:], 47)
            nc.vector.tensor_tensor(A3, m3[:, :, 1:48], c3[:, :, 0:47], Alu.mult)
            nc.vector.tensor_tensor(A3, A3, sw3[:, :, 0:47], Alu.subtract)
            nc.vector.tensor_tensor(A3, A3, rw3[:, :, 1:48], Alu.mult)
            nc.vector.tensor_reduce(accs["p1"][:], A3, AX.XY, Alu.add)
            t2 = pool.tile([P, NBLK * 48], dt.float32, tag="t2d")
            nc.vector.tensor_tensor(t2[:], rw_sh[:], rw_sh[:], Alu.mult)
            nc.vector.tensor_tensor(t2[:], t2[:], ds[:], Alu.mult)
            nc.vector.tensor_reduce(accs["p2"][:], _blk(t2[:], 48), AX.XY, Alu.add)

        # ---------- inter loss (levels emitted concurrently) ----------
        inter_lvls = [l for l in (0, 1) if f"l{l}" in parts]
        if not inter_lvls:
            nc.vector.memset(accs["inter"][:], 0.0)
        lvl_pools = {l: ctx.enter_context(tc.tile_pool(name=f"lvl{l}", bufs=1))
                     for l in inter_lvls}
        for lvl in inter_lvls:
            eng = {}
            _emit_level(nc, tc, lvl_pools[lvl], lvl, s_sh, radios[lvl],
                        aps[f"ps{lvl}"], aps[f"pw{lvl}"],
                        accs["inter" if lvl == 0 else "inter1"], aps, **eng)

        # ---------- rgb ----------
        with tc.tile_pool(name="rgb", bufs=1) as pool:
            pdt = pool.tile([P, NBLK * 3], dt.float32, tag="pdt")
            gtt = pool.tile([P, NBLK * 3], dt.float32, tag="gtt")
            nc.sync.dma_start(_blk(pdt[:], 3),
                              aps["pd"].rearrange("(b p) c -> p b c", p=P))
            nc.sync.dma_start(_blk(gtt[:], 3),
                              aps["gt"].rearrange("(b p) c -> p b c", p=P))
            d = pool.tile([P, NBLK * 3], dt.float32, tag="rgbd")
            nc.vector.tensor_tensor(d[:], pdt[:], gtt[:], Alu.subtract)
            nc.vector.tensor_tensor(d[:], d[:], d[:], Alu.mult)
            nc.vector.tensor_reduce(accs["rgb"][:], d[:], AX.X, Alu.add)

        # ---------- hash ----------
        for lvl in ((0, 1) if "hash" in parts else ()):
            with tc.tile_pool(name=f"hash{lvl}", bufs=1) as pool:
                idx = pool.tile([P, HCOLS], dt.int32, tag="hidx")
                src = aps[f"hi{lvl}"]
                nc.sync.dma_start(idx[:], bass.AP(tensor=src.tensor,
                                                  offset=src.offset,
                                                  ap=[[HROW, P], [1, HCOLS]]))
                emb = pool.tile([P, HCOLS * 2], dt.float32, tag="hemb")
                esrc = aps[f"he{lvl}"]
                nc.sync.dma_start(emb[:], bass.AP(tensor=esrc.tensor,
                                                  offset=esrc.offset,
                                                  ap=[[HROW * 2, P], [1, HCOLS * 2]]))
                sq = pool.tile([P, HCOLS * 2], dt.float32, tag="hsq")
                nc.vector.tensor_tensor(sq[:], emb[:], emb[:], Alu.mult)
                wv = pool.tile([P, HCOLS], dt.float32, tag="hw")
                sq3 = sq[:].rearrange("p (n two) -> p n two", two=2)
                nc.vector.tensor_tensor(wv[:], sq3[:, :, 0], sq3[:, :, 1], Alu.add)
                eq = pool.tile([P, HCOLS], dt.float32, tag="heq")
                nc.vector.memset(eq[:, 0:1], 0.0)
                nc.vector.tensor_tensor(eq[:, 1:HCOLS], idx[:, 1:HCOLS],
                                        idx[:, 0:HCOLS - 1], Alu.is_equal)
                S = pool.tile([P, HCOLS], dt.float32, tag="hS")
                nc.vector.tensor_tensor_scan(S[:], eq[:], wv[:], 0.0,
                                             Alu.mult, Alu.add)
                cc = pool.tile([P, HCOLS], dt.float32, tag="hcc")
                nc.vector.tensor_tensor_scan(cc[:], eq[:], ones_h[:], 0.0,
                                             Alu.mult, Alu.add)
                ratio = pool.tile([P, HCOLS], dt.float32, tag="hr")
                nc.vector.reciprocal(cc[:], cc[:])
                nc.vector.tensor_tensor(ratio[:], S[:], cc[:], Alu.mult)
                me = pool.tile([P, HCOLS], dt.float32, tag="hme")
                nc.vector.tensor_scalar(me[:, 0:HCOLS - 1], eq[:, 1:HCOLS], -1.0,
                                        1.0, Alu.mult, Alu.add)
                nc.vector.tensor_tensor(ratio[:, HALO:HALO + HROW],
                                        ratio[:, HALO:HALO + HROW],
                                        me[:, HALO:HALO + HROW], Alu.mult)
                part = pool.tile([P, 1], dt.float32, tag="hpart")
                nc.vector.tensor_reduce(part[:], ratio[:, HALO:HALO + HROW],
                                        AX.X, Alu.add)
                if lvl == 0:
                    nc.vector.tensor_copy(accs["hash"][:], part[:])
                else:
                    nc.vector.tensor_tensor(accs["hash"][:], accs["hash"][:],
                                            part[:], Alu.add)

        # ---------- combine + output ----------
        with tc.tile_pool(name="fin", bufs=1) as pool:
            tot = pool.tile([P, 1], dt.float32, tag="tot")
            nc.vector.tensor_scalar(tot[:], accs["rgb"][:], W_RGB / (R * 3), None,
                                    Alu.mult)
            nc.vector.scalar_tensor_tensor(tot[:], accs["inter"][:], W_INTER,
                                           tot[:], Alu.mult, Alu.add)
            nc.vector.scalar_tensor_tensor(tot[:], accs["inter1"][:], W_INTER,
                                           tot[:], Alu.mult, Alu.add)
            nc.vector.scalar_tensor_tensor(tot[:], accs["p1"][:], W_DIST * 2.0 / R,
                                           tot[:], Alu.mult, Alu.add)
            nc.vector.scalar_tensor_tensor(tot[:], accs["p2"][:],
                                           W_DIST / (3.0 * R), tot[:],
                                           Alu.mult, Alu.add)
            nc.vector.scalar_tensor_tensor(tot[:], accs["hash"][:],
                                           W_HASH / (NUM_SEGMENTS * 2.0), tot[:],
                                           Alu.mult, Alu.add)
            res = pool.tile([1, 1], dt.float32, tag="res")
            nc.gpsimd.tensor_reduce(res[:], tot[:], AX.C, Alu.add)
            nc.sync.dma_start(out_ap, res[:])


# ---------------- host side ----------------
_module_cache = {}


def _get_module():
    if "nc" not in _module_cache:
        _module_cache["nc"] = build_module()
    return _module_cache["nc"]


def shard_inputs(inputs):
    """Full inputs -> list of 8 per-core in_maps."""
    f32 = np.float32
    pd = np.ascontiguousarray(inputs["pd_rgbs"], f32)
    gt = np.ascontiguousarray(inputs["gt_rgbs"], f32)
    sd = np.ascontiguousarray(inputs["render_sdist"], f32)
    rw = np.ascontiguousarray(inputs["render_weights"], f32)
    ps0 = np.ascontiguousarray(inputs["prop_sdist_0"], f32)
    pw0 = np.ascontiguousarray(inputs["prop_weights_0"], f32)
    ps1 = np.ascontiguousarray(inputs["prop_sdist_1"], f32)
    pw1 = np.ascontiguousarray(inputs["prop_weights_1"], f32)
    hashes = {}
    for lvl in (0, 1):
        idx = np.asarray(inputs[f"enc_idx_{lvl}"]).astype(np.int32)
        emb = np.ascontiguousarray(inputs[f"enc_embds_{lvl}"], f32)
        idx_pad = np.full(M + 2 * HALO, -1, np.int32)
        idx_pad[HALO:HALO + M] = idx
        emb_pad = np.zeros((M + 2 * HALO, 2), f32)
        emb_pad[HALO:HALO + M] = emb
        hashes[lvl] = (idx_pad, emb_pad)

    consts = {}
    rep = lambda row: np.ascontiguousarray(np.tile(row, (P, 1)))
    for lvl, L in LVL.items():
        n2 = L["n2"]
        consts[f"c_iota16_l{lvl}"] = rep(np.tile(np.arange(n2, dtype=np.int16),
                                                 NBLK))
        LWc = L["EW"] + 24
        consts[f"c_iotap1_l{lvl}"] = rep(np.tile(
            np.arange(1, LWc + 1, dtype=np.float32), NBLK))
        msk = np.ones(NBLK * LWc, np.float32)
        msk[::LWc] = 0.0
        consts[f"c_mask_l{lvl}"] = rep(msk)
    m48 = np.ones(NBLK * 48, np.float32)
    m48[::48] = 0.0
    consts["c_mask48"] = rep(m48)
    consts["c_ones"] = rep(np.ones(HCOLS, np.float32))
    consts["c_zeros"] = rep(np.zeros(NBLK * 512, np.float32))
    consts["c_bigf"] = rep(np.full(NBLK * 512, BIGF, np.float32))

    in_maps = []
    for c in range(N_CORES):
        r0 = c * RPC
        lo = c * MPC
        im = {
            "pd": pd[r0:r0 + RPC], "gt": gt[r0:r0 + RPC],
            "sd": sd[r0:r0 + RPC], "rw": rw[r0:r0 + RPC],
            "ps0": ps0[r0:r0 + RPC], "pw0": pw0[r0:r0 + RPC],
            "ps1": ps1[r0:r0 + RPC], "pw1": pw1[r0:r0 + RPC],
        }
        for lvl in (0, 1):
            idx_pad, emb_pad = hashes[lvl]
            im[f"hi{lvl}"] = np.ascontiguousarray(idx_pad[lo:lo + HSLICE])
            im[f"he{lvl}"] = np.ascontiguousarray(
                emb_pad[lo:lo + HSLICE].reshape(-1))
        im.update(consts)
        in_maps.append(im)
    return in_maps


def kernel(**inputs) -> np.ndarray:
    nc = _get_module()
    in_maps = shard_inputs(inputs)
    res = run_bass_kernel_spmd(nc, in_maps, core_ids=list(range(N_CORES)))
    total = np.float64(0.0)
    for r in res.results:
        total += np.float64(r["out"][0, 0])
    return np.float32(total)



# revision 3
# speedup vs baseline: 1.5066x; 1.5066x over previous
"""Trainium2 Bass kernel v2 for nn_Loss_dict_50646254354805 (NeRF-style loss).

v2 vs baseline:
- bitonic merges on uint16 quantized keys (value*15000 + 2 tag bits) -> DVE
  2x perf mode; keys determine ORDER only.
- exact f32 values (queries/em/ep) and radio reach the merged domain via
  batched u16-half local_scatters through one shared index table (idxcat):
  merged positions come from the C/Cm count scans.
- one merge per level; post-merge work split into two 2-block half-streams
  with per-stream engine maps; all generators emitted stage-interleaved so
  DVE / Pool / Act overlap.
"""
import numpy as np

import concourse.bass as bass
import concourse.mybir as mybir
import concourse.tile as tile
from concourse import bacc
from concourse.bass_utils import run_bass_kernel_spmd

dt = mybir.dt
Alu = mybir.AluOpType
AX = mybir.AxisListType
Act = mybir.ActivationFunctionType
P = 128

PULSE = (0.01, 0.005)
W_RGB, W_INTER, W_DIST, W_HASH = 1.0, 1.0, 0.01, 0.1
NUM_SEGMENTS = 65536
R, N = 4096, 48
M = R * N
N_CORES = 8
RPC = R // N_CORES
NBLK = RPC // P               # 4 ray blocks per core
MPC = M // N_CORES
HALO = 64
HROW = MPC // P
HCOLS = HROW + HALO + 1
HSLICE = HALO + MPC + HALO

VOFF = 0.97
QS = 15000.0                  # key quantization scale
PADK = 0xFFFC

LVL = {0: dict(X=257, n2=512), 1: dict(X=97, n2=256)}
for _L in LVL.values():
    _L["EW"] = ((_L["X"] + 98 + 1 + 7) // 8) * 8        # 360 / 200
    _L["QWS"] = _L["EW"] - 98                           # 262 / 102

NB = 2                        # blocks per half-stream
HALVES = [("l0a", 0, 0), ("l0b", 0, 2), ("l1a", 1, 0), ("l1b", 1, 2)]


def _blk(ap, n):
    return ap.rearrange("p (b n) -> p b n", n=n)


def _merge(eng, bufa, bufb, width, ew=None):
    """Ascending bitonic merge over [P, NBLK*width] u16 ping-pong tiles.

    If ew is given, only outputs [0, ew+2d-1] of each block are needed
    downstream, so late stages skip whole 2d-chunks beyond that window."""
    cur, nxt = bufa, bufb
    d = width // 2
    while d >= 1:
        nch = width // (2 * d)
        keep = nch
        if ew is not None:
            keep = min(nch, -(-(ew + 2 * d - 1) // (2 * d)))
        if keep == nch:
            c3 = cur[:].rearrange("p (c td) -> p c td", td=2 * d)
            n3 = nxt[:].rearrange("p (c td) -> p c td", td=2 * d)
        else:
            c3 = cur[:].rearrange("p (b c td) -> p (b c) td",
                                  td=2 * d, c=nch)[: , 0:0]  # placeholder
        if keep == nch:
            lo_in, hi_in = c3[:, :, 0:d], c3[:, :, d:2 * d]
            eng.tensor_tensor(n3[:, :, 0:d], lo_in, hi_in, Alu.min)
            eng.tensor_tensor(n3[:, :, d:2 * d], lo_in, hi_in, Alu.max)
        else:
            c4 = cur[:].rearrange("p (b c td) -> p b c td", td=2 * d, c=nch)
            n4 = nxt[:].rearrange("p (b c td) -> p b c td", td=2 * d, c=nch)
            lo_in = c4[:, :, 0:keep, 0:d]
            hi_in = c4[:, :, 0:keep, d:2 * d]
            eng.tensor_tensor(n4[:, :, 0:keep, 0:d], lo_in, hi_in, Alu.min)
            eng.tensor_tensor(n4[:, :, 0:keep, d:2 * d], lo_in, hi_in, Alu.max)
        cur, nxt = nxt, cur
        d //= 2
    return cur


def _emit_level_merge(nc, tc, pool, lvl, s_sh, x_ap, pwt_ap, out, eng):
    """Generator: quantize + b1/b2 merges for all 4 blocks of one level."""
    ME, ME2, EE = eng["ME"], eng["ME2"], eng["EE"]
    AE = nc.scalar
    L = LVL[lvl]
    X, n2 = L["X"], L["n2"]
    pw = PULSE[lvl]

    xt = pool.tile([P, NBLK * X], dt.float32, tag="xt")
    nc.sync.dma_start(_blk(xt[:], X), x_ap.rearrange("(b p) x -> p b x", p=P))
    pwt = pool.tile([P, NBLK * (X - 1)], dt.float32, tag="pwt")
    nc.sync.dma_start(_blk(pwt[:], X - 1),
                      pwt_ap.rearrange("(b p) x -> p b x", p=P))
    out["xt"] = xt
    out["pwt"] = pwt

    b2a = pool.tile([P, NBLK * n2], dt.uint16, tag="b2a")
    b2b = pool.tile([P, NBLK * n2], dt.uint16, tag="b2b")
    b2a3 = _blk(b2a[:], n2)
    b1a = pool.tile([P, NBLK * 128], dt.uint16, tag="b1a")
    b1b = pool.tile([P, NBLK * 128], dt.uint16, tag="b1b")
    nc.gpsimd.memset(b1a[:], PADK)
    b1a3 = _blk(b1a[:], 128)
    emq = pool.tile([P, NBLK * 49], dt.uint16, tag="emq")
    EE.tensor_scalar(emq[:], s_sh[:], QS, (1.0 - pw - VOFF) * QS + 0.5,
                     Alu.mult, Alu.add)
    epq = pool.tile([P, NBLK * 49], dt.uint16, tag="epq")
    EE.tensor_scalar(epq[:], s_sh[:], QS, (1.0 + pw - VOFF) * QS + 0.5,
                     Alu.mult, Alu.add)
    EE.tensor_scalar(b1a3[:, :, 0:49], _blk(emq[:], 49), 4, 1,
                     Alu.mult, Alu.add)
    EE.tensor_scalar(b1a3[:, :, 79:128][:, :, ::-1], _blk(epq[:], 49), 4, 3,
                     Alu.mult, Alu.add)
    yield
    b1 = _merge(ME, b1a, b1b, 128, ew=98)
    yield
    nc.gpsimd.memset(b2a3[:, :, X:n2 - 128], PADK)
    xq = pool.tile([P, NBLK * X], dt.uint16, tag="xq")
    EE.tensor_scalar(xq[:], xt[:], QS, (1.0 - VOFF) * QS + 0.5,
                     Alu.mult, Alu.add)
    EE.tensor_scalar(b2a3[:, :, 0:X], _blk(xq[:], X), 4, None, Alu.mult)
    EE.tensor_copy(b2a3[:, :, n2 - 128:n2][:, :, ::-1], _blk(b1[:], 128))
    yield
    out["m"] = _merge(ME2, b2a, b2b, n2, ew=L["EW"])
    yield


def _emit_half(nc, pool, lvl, b0, s_sh, radio_full, mout, consts, acc, eng):
    """Generator: post-merge pipeline for blocks [b0, b0+NB) of one level."""
    SE, XE, EE, FE = (eng[k] for k in ("SE", "XE", "EE", "FE"))
    TE = eng.get("TE", EE)
    SE2 = eng.get("SE2", SE)
    AE = nc.scalar
    L = LVL[lvl]
    X, n2, EW, QWS = L["X"], L["n2"], L["EW"], L["QWS"]
    NL = NB * EW
    NQ = NB * QWS
    NE = NB * 49
    VW = NQ + 2 * NE          # vcat width: [x | em | ep]
    pw = PULSE[lvl]
    maskf, mask_cnt, io49p, ioG, ioQ2 = consts

    def blkE(ap):
        return ap.rearrange("p (b n) -> p b n", b=NB)

    ss = s_sh[:][:, b0 * 49:(b0 + NB) * 49]

    # ---------- sources: exact values + radio (independent of merge) ----------
    vcat = pool.tile([P, VW], dt.float32, tag="vcat")
    nc.gpsimd.memset(_blk(vcat[:, 0:NQ], QWS)[:, :, X:QWS], 0.0)
    radcat = pool.tile([P, 2 * NE], dt.float32, tag="radcat")
    rsl_ = radio_full[:][:, b0 * 49:(b0 + NB) * 49]
    FE.tensor_copy(radcat[:, 0:NE], rsl_)
    FE.tensor_scalar(radcat[:, NE:2 * NE], radcat[:, 0:NE], -1.0, None, Alu.mult)
    yield
    # wait for merge result
    while "m" not in mout:
        yield
    m = mout["m"]
    xt, pwt_full = mout["xt"], mout["pwt"]
    mSh = _blk(m[:], n2)[:, b0:b0 + NB, 0:EW]       # [P, NB, EW] strided
    xts = _blk(xt[:], X)[:, b0:b0 + NB]             # [P, NB, X]
    pwt = _blk(pwt_full[:], X - 1)[:, b0:b0 + NB]
    AE.activation(_blk(vcat[:, 0:NQ], QWS)[:, :, 0:X], xts, Act.Copy)
    AE.activation(_blk(vcat[:, NQ:NQ + NE], 49), _blk(ss, 49), Act.Copy, bias=-pw)
    AE.activation(_blk(vcat[:, NQ + NE:VW], 49), _blk(ss, 49), Act.Copy, bias=pw)
    yield

    # ---------- tags + counts ----------
    tagb = pool.tile([P, NL], dt.uint16, tag="tagb")
    XE.tensor_scalar(blkE(tagb[:]), mSh, 3, None, Alu.bitwise_and)
    ev_f = pool.tile([P, NL], dt.uint16, tag="ev_f")
    TE.tensor_scalar(ev_f[:], tagb[:], 1, None, Alu.bitwise_and)
    em_f = pool.tile([P, NL], dt.uint16, tag="em_f")
    TE.tensor_scalar(em_f[:], tagb[:], 1, None, Alu.is_equal)
    ep_f = pool.tile([P, NL], dt.uint16, tag="ep_f")
    TE.tensor_scalar(ep_f[:], tagb[:], 3, None, Alu.is_equal)
    yield
    C = pool.tile([P, NL], dt.uint16, tag="C")
    SE.tensor_tensor_scan(C[:], mask_cnt, ev_f[:], 0.0, Alu.mult, Alu.add)
    Cm = pool.tile([P, NL], dt.uint16, tag="Cm")
    SE.tensor_tensor_scan(Cm[:], mask_cnt, em_f[:], 0.0, Alu.mult, Alu.add)
    yield

    # ---------- idxcat: merged position of every source element ----------
    t1 = tagb                                       # tagb dead after masks
    t2 = pool.tile([P, NL], dt.uint16, tag="t2")
    t3 = pool.tile([P, NL], dt.uint16, tag="t3")
    # block offsets (49b/98b) ride in from the mask_cnt scan carry; section
    # offsets NQ / NQ+NE are flat immediates. One combined scatter:
    # t1 = (Cm'+NQ)*em + (C'-Cm'+NQ+NE)*ep + (ioQ2-C')*qf - 1
    EE.tensor_tensor(t2[:], C[:], Cm[:], Alu.subtract)
    EE.tensor_scalar(t2[:], t2[:], NQ + NE, None, Alu.add)
    EE.tensor_tensor(t2[:], t2[:], ep_f[:], Alu.mult)
    EE.tensor_scalar(t1[:], Cm[:], NQ, None, Alu.add)
    EE.tensor_tensor(t1[:], t1[:], em_f[:], Alu.mult)
    EE.tensor_tensor(t1[:], t1[:], t2[:], Alu.add)
    qf = em_f                                       # em_f dead after t1
    TE.tensor_scalar(qf[:], ev_f[:], 0, None, Alu.is_equal)
    EE.tensor_tensor(t3[:], ioQ2, C[:], Alu.subtract)
    EE.tensor_tensor(t3[:], t3[:], qf[:], Alu.mult)
    EE.tensor_tensor(t1[:], t1[:], t3[:], Alu.add)
    EE.tensor_scalar(t1[:], t1[:], 1, None, Alu.subtract)       # idx all
    EE.tensor_scalar(t2[:], t3[:], 1, None, Alu.subtract)       # idxq
    yield
    idxcat = pool.tile([P, VW], dt.uint16, tag="idxcat")
    nc.gpsimd.local_scatter(idxcat[:].bitcast(dt.int16), ioG,
                            t1[:].bitcast(dt.int16), channels=P,
                            num_elems=VW, num_idxs=NL)
    idx2 = pool.tile([P, 2 * VW], dt.uint16, tag="idx2")
    i2v = idx2[:].rearrange("p (n two) -> p n two", two=2)
    AE.activation(i2v[:, :, 0], idxcat[:], Act.Copy)
    AE.activation(i2v[:, :, 1], idxcat[:], Act.Copy, bias=1.0)
    yield

    # ---------- pair-scatter exact values + radio into merged domain ----------
    v = pool.tile([P, NL], dt.float32, tag="v")
    nc.gpsimd.local_scatter(v[:].bitcast(dt.int16),
                            vcat[:].bitcast(dt.int16),
                            idx2[:].bitcast(dt.int16), channels=P,
                            num_elems=2 * NL, num_idxs=2 * VW)
    F1 = pool.tile([P, NL], dt.float32, tag="F1")   # radio_m
    nc.gpsimd.local_scatter(F1[:].bitcast(dt.int16),
                            radcat[:].bitcast(dt.int16),
                            idx2[:, 2 * NQ:2 * VW].bitcast(dt.int16), channels=P,
                            num_elems=2 * NL, num_idxs=4 * NE)
    yield

    # ---------- density reconstruction ----------
    F2 = pool.tile([P, NL], dt.float32, tag="F2")
    SE2.tensor_tensor_scan(F2[:], maskf, F1[:], 0.0, Alu.mult, Alu.add)  # g
    dv = pool.tile([P, NL], dt.float32, tag="dv")
    dv3 = blkE(dv[:])
    v3 = blkE(v[:])
    nc.gpsimd.memset(dv3[:, :, 0:1], 0.0)
    FE.tensor_tensor(dv3[:, :, 1:EW], v3[:, :, 1:EW], v3[:, :, 0:EW - 1],
                     Alu.subtract)
    yield
    wg = v                                          # v dead after dv
    wg3 = blkE(wg[:])
    nc.gpsimd.memset(wg3[:, :, 0:1], 0.0)
    FE.tensor_tensor(wg3[:, :, 1:EW], dv3[:, :, 1:EW],
                     blkE(F2[:])[:, :, 0:EW - 1], Alu.mult)
    w_t = F1                                        # radio dead after g
    SE2.tensor_tensor_scan(w_t[:], maskf, wg[:], 0.0, Alu.mult, Alu.add)
    yield
    wc = wg                                         # wg dead
    AE.activation(wc[:], w_t[:], Act.Relu, scale=0.5)
    scr = pool.tile([P, NL], dt.float32, tag="scr")
    wc3 = blkE(wc[:])
    s3_ = blkE(scr[:])
    nc.gpsimd.memset(s3_[:, :, 0:1], 0.0)
    FE.tensor_tensor(s3_[:, :, 1:EW], wc3[:, :, 1:EW], wc3[:, :, 0:EW - 1],
                     Alu.add)
    area = w_t                                      # w dead after wc
    a3 = blkE(area[:])
    nc.gpsimd.memset(a3[:, :, 0:1], 0.0)
    FE.tensor_tensor(a3[:, :, 1:EW], s3_[:, :, 1:EW],
                     dv3[:, :, 1:EW], Alu.mult)
    cdf = F2                                        # g dead after wg
    SE2.tensor_tensor_scan(cdf[:], maskf, area[:], 0.0, Alu.mult, Alu.add)
    yield

    # ---------- compact cdf at query slots (pair-scatter) ----------
    idx2q = idx2                                    # idx2 dead after scatters
    i2qv = idx2q[:][:, 0:2 * NL].rearrange("p (n two) -> p n two", two=2)
    tq = tagb                                       # dead
    AE.activation(tq[:], t2[:], Act.Copy, scale=2.0)
    AE.activation(i2qv[:, :, 0], tq[:], Act.Copy)
    AE.activation(i2qv[:, :, 1], tq[:], Act.Copy, bias=1.0)
    cdfq = vcat                                     # vcat dead after v scatter
    cqn = cdfq[:][:, 0:NQ]
    nc.gpsimd.local_scatter(cqn.bitcast(dt.int16),
                            cdf[:].bitcast(dt.int16),
                            idx2q[:][:, 0:2 * NL].bitcast(dt.int16), channels=P,
                            num_elems=2 * NQ, num_idxs=2 * NL)
    yield

    # ---------- loss tail ----------
    NW = NB * (X - 1)
    ws = scr                                        # dead after area
    ws2 = ws[:][:, 0:NW]
    cqf = _blk(cdfq[:][:, 0:NQ], QWS)
    FE.tensor_tensor(_blk(ws2, X - 1), cqf[:, :, 1:X], cqf[:, :, 0:X - 1],
                     Alu.subtract)
    FE.tensor_tensor(_blk(ws2, X - 1), _blk(ws2, X - 1), pwt, Alu.subtract)
    den = area                                      # dead after cdf
    den2 = den[:][:, 0:NW]
    AE.activation(_blk(den2, X - 1), pwt, Act.Copy, bias=1e-5)
    nc.vector.reciprocal(den2, den2)
    rsl = dv                                        # dead after area
    AE.activation(rsl[:][:, 0:NW], ws2, Act.Relu)
    AE.activation(ws2, rsl[:][:, 0:NW], Act.Square)
    FE.tensor_tensor(ws2, ws2, den2, Alu.mult)
    nc.vector.tensor_reduce(acc[:], _blk(ws2, X - 1), AX.XY, Alu.add)
    yield


def _emit_setup(nc, pool, s_sh, radios, accs, mask48, aps):
    V, G = nc.vector, nc.gpsimd
    rw_sh = pool.tile([P, NBLK * 48], dt.float32, tag="rw_sh")
    nc.sync.dma_start(_blk(rw_sh[:], 48),
                      aps["rw"].rearrange("(b p) x -> p b x", p=P))
    s3 = _blk(s_sh[:], 49)
    ds = pool.tile([P, NBLK * 48], dt.float32, tag="ds")
    V.tensor_tensor(_blk(ds[:], 48), s3[:, :, 1:49], s3[:, :, 0:48], Alu.subtract)
    dse = pool.tile([P, NBLK * 48], dt.float32, tag="dse")
    nc.scalar.activation(dse[:], ds[:], Act.Copy, bias=1e-8)
    V.reciprocal(dse[:], dse[:])
    wnorm = pool.tile([P, NBLK * 48], dt.float32, tag="wnorm")
    V.tensor_tensor(wnorm[:], rw_sh[:], dse[:], Alu.mult)
    wnp = pool.tile([P, NBLK * 50], dt.float32, tag="wnp")
    G.memset(wnp[:], 0.0)
    V.tensor_copy(_blk(wnp[:], 50)[:, :, 1:49], _blk(wnorm[:], 48))
    diff = pool.tile([P, NBLK * 49], dt.float32, tag="diff")
    wnp3 = _blk(wnp[:], 50)
    V.tensor_tensor(_blk(diff[:], 49), wnp3[:, :, 1:50], wnp3[:, :, 0:49],
                    Alu.subtract)
    for lvl in (0, 1):
        V.tensor_scalar(radios[lvl][:], diff[:], 1.0 / (2 * PULSE[lvl]), None,
                        Alu.mult)
    yield

    mid = pool.tile([P, NBLK * 48], dt.float32, tag="mid")
    V.tensor_tensor(_blk(mid[:], 48), s3[:, :, 1:49], s3[:, :, 0:48], Alu.add)
    wm = pool.tile([P, NBLK * 48], dt.float32, tag="wm")
    V.scalar_tensor_tensor(wm[:], mid[:], 0.5, rw_sh[:], Alu.mult, Alu.mult)
    Cin = pool.tile([P, NBLK * 48], dt.float32, tag="Cin")
    V.tensor_tensor_scan(Cin[:], mask48, rw_sh[:], 0.0, Alu.mult, Alu.add)
    Sin = pool.tile([P, NBLK * 48], dt.float32, tag="Sin")
    V.tensor_tensor_scan(Sin[:], mask48, wm[:], 0.0, Alu.mult, Alu.add)
    yield
    A = pool.tile([P, NBLK * 47], dt.float32, tag="A47")
    m3 = _blk(mid[:], 48)
    c3 = _blk(Cin[:], 48)
    sw3 = _blk(Sin[:], 48)
    rw3 = _blk(rw_sh[:], 48)
    A3 = _blk(A[:], 47)
    V.scalar_tensor_tensor(A3, m3[:, :, 1:48], 0.5, c3[:, :, 0:47],
                           Alu.mult, Alu.mult)
    V.tensor_tensor(A3, A3, sw3[:, :, 0:47], Alu.subtract)
    V.tensor_tensor(A3, A3, rw3[:, :, 1:48], Alu.mult)
    V.tensor_reduce(accs["p1"][:], A3, AX.XY, Alu.add)
    t2 = pool.tile([P, NBLK * 48], dt.float32, tag="t2d")
    G.tensor_tensor(t2[:], rw_sh[:], rw_sh[:], Alu.mult)
    G.tensor_tensor(t2[:], t2[:], ds[:], Alu.mult)
    V.tensor_reduce(accs["p2"][:], _blk(t2[:], 48), AX.XY, Alu.add)
    yield

    pdt = pool.tile([P, NBLK * 3], dt.float32, tag="pdt")
    gtt = pool.tile([P, NBLK * 3], dt.float32, tag="gtt")
    nc.sync.dma_start(_blk(pdt[:], 3), aps["pd"].rearrange("(b p) c -> p b c", p=P))
    nc.sync.dma_start(_blk(gtt[:], 3), aps["gt"].rearrange("(b p) c -> p b c", p=P))
    d = pool.tile([P, NBLK * 3], dt.float32, tag="rgbd")
    V.tensor_tensor(d[:], pdt[:], gtt[:], Alu.subtract)
    V.tensor_tensor(d[:], d[:], d[:], Alu.mult)
    V.tensor_reduce(accs["rgb"][:], d[:], AX.X, Alu.add)
    yield


def _emit_hash(nc, pool, lvl, ones_h, acc, aps, first):
    E = nc.gpsimd if lvl == 0 else nc.vector
    idx = pool.tile([P, HCOLS], dt.int32, tag="hidx")
    src = aps[f"hi{lvl}"]
    nc.sync.dma_start(idx[:], bass.AP(tensor=src.tensor, offset=src.offset,
                                      ap=[[HROW, P], [1, HCOLS]]))
    emb = pool.tile([P, HCOLS * 2], dt.float32, tag="hemb")
    esrc = aps[f"he{lvl}"]
    nc.sync.dma_start(emb[:], bass.AP(tensor=esrc.tensor, offset=esrc.offset,
                                      ap=[[HROW * 2, P], [1, HCOLS * 2]]))
    sq = pool.tile([P, HCOLS * 2], dt.float32, tag="hsq")
    E.tensor_tensor(sq[:], emb[:], emb[:], Alu.mult)
    wv = pool.tile([P, HCOLS], dt.float32, tag="hw")
    sq3 = sq[:].rearrange("p (n two) -> p n two", two=2)
    E.tensor_tensor(wv[:], sq3[:, :, 0], sq3[:, :, 1], Alu.add)
    eq = pool.tile([P, HCOLS], dt.float32, tag="heq")
    nc.gpsimd.memset(eq[:, 0:1], 0.0)
    nc.vector.tensor_tensor(eq[:, 1:HCOLS], idx[:, 1:HCOLS], idx[:, 0:HCOLS - 1],
                             Alu.is_equal)
    yield
    S = pool.tile([P, HCOLS], dt.float32, tag="hS")
    nc.vector.tensor_tensor_scan(S[:], eq[:], wv[:], 0.0, Alu.mult, Alu.add)
    cc = pool.tile([P, HCOLS], dt.float32, tag="hcc")
    nc.vector.tensor_tensor_scan(cc[:], eq[:], ones_h, 0.0, Alu.mult, Alu.add)
    yield
    ratio = pool.tile([P, HCOLS], dt.float32, tag="hr")
    nc.vector.reciprocal(cc[:], cc[:])
    E.tensor_tensor(ratio[:], S[:], cc[:], Alu.mult)
    me = pool.tile([P, HCOLS], dt.float32, tag="hme")
    nc.scalar.activation(me[:, 0:HCOLS - 1], eq[:, 1:HCOLS], Act.Copy,
                         bias=1.0, scale=-1.0)
    E.tensor_tensor(ratio[:, HALO:HALO + HROW], ratio[:, HALO:HALO + HROW],
                    me[:, HALO:HALO + HROW], Alu.mult)
    if first:
        nc.vector.tensor_reduce(acc[:], ratio[:, HALO:HALO + HROW], AX.X, Alu.add)
    else:
        part = pool.tile([P, 1], dt.float32, tag="hpart")
        nc.vector.tensor_reduce(part[:], ratio[:, HALO:HALO + HROW], AX.X,
                                Alu.add)
        E.tensor_tensor(acc[:], acc[:], part[:], Alu.add)
    yield


def build_module(parts=("rgb", "dist", "hash", "l0", "l1")):
    nc = bacc.Bacc("TRN2", target_bir_lowering=False, debug=False,
                   enable_asserts=False, num_devices=N_CORES)
    aps = {}

    def din(name, shape, dtype=dt.float32):
        aps[name] = nc.dram_tensor(name, shape, dtype, kind="ExternalInput").ap()
    din("pd", [RPC, 3]); din("gt", [RPC, 3])
    din("sd", [RPC, 49]); din("rw", [RPC, 48])
    din("ps0", [RPC, 257]); din("pw0", [RPC, 256])
    din("ps1", [RPC, 97]); din("pw1", [RPC, 96])
    din("hi0", [HSLICE], dt.int32); din("he0", [HSLICE * 2])
    din("hi1", [HSLICE], dt.int32); din("he1", [HSLICE * 2])
    for lvl, L in LVL.items():
        NL = NB * L["EW"]
        din(f"c_u16_l{lvl}", [P, 4 * NL], dt.int16)
        din(f"c_maskf_l{lvl}", [P, 2 * NL])
    din("c_mask48", [P, NBLK * 48]); din("c_ones", [P, HCOLS])
    out_ap = nc.dram_tensor("out", [1, 1], dt.float32, kind="ExternalOutput").ap()

    with tile.TileContext(nc) as tc:
        _emit(nc, tc, aps, out_ap, parts)
    nc.compile()
    return nc


def _emit(nc, tc, aps, out_ap, parts=("rgb", "dist", "hash", "l0", "l1")):
    import contextlib
    V, G = nc.vector, nc.gpsimd
    with contextlib.ExitStack() as ctx:
        spool = ctx.enter_context(tc.tile_pool(name="shared", bufs=1))
        s_sh = spool.tile([P, NBLK * 49], dt.float32, tag="s_sh")
        nc.sync.dma_start(_blk(s_sh[:], 49),
                          aps["sd"].rearrange("(b p) x -> p b x", p=P))
        radios = {l: spool.tile([P, NBLK * 49], dt.float32, tag=f"radio{l}",
                                name=f"radio{l}")
                  for l in (0, 1)}

        cpool = ctx.enter_context(tc.tile_pool(name="consts", bufs=1))
        mask48 = cpool.tile([P, NBLK * 48], dt.float32, tag="mask48")
        ones_h = cpool.tile([P, HCOLS], dt.float32, tag="ones_h")
        lvl_consts = {}
        cdma = []
        cdma.append((mask48[:], aps["c_mask48"]))
        cdma.append((ones_h[:], aps["c_ones"]))
        for lvl, L in LVL.items():
            NL = NB * L["EW"]
            cu = cpool.tile([P, 4 * NL], dt.int16, tag=f"cu16_{lvl}",
                            name=f"cu16_{lvl}")
            mf = cpool.tile([P, 2 * NL], dt.float32, tag=f"maskf_{lvl}",
                            name=f"maskf_{lvl}")
            cdma.append((cu[:], aps[f"c_u16_l{lvl}"]))
            cdma.append((mf[:], aps[f"c_maskf_l{lvl}"]))
            cuv = cu[:].bitcast(dt.uint16)
            lvl_consts[lvl] = (mf[:][:, 0:NL], mf[:][:, NL:2 * NL],
                               cuv[:, NL:2 * NL],
                               cu[:][:, 2 * NL:3 * NL], cuv[:, 3 * NL:4 * NL])
            # (maskf, mask_cnt(f32), io49p, ioG(i16), ioQ2)

        def _emit_consts():
            for dst, src_ap in cdma:
                nc.sync.dma_start(dst, src_ap)
            yield

        accs = {}
        for name in ("rgb", "p1", "p2", "hash", "l0a", "l0b", "l1a", "l1b"):
            accs[name] = cpool.tile([P, 1], dt.float32, tag=f"acc_{name}",
                                    name=f"acc_{name}")
            V.memset(accs[name][:], 0.0)

        spool = ctx.enter_context(tc.tile_pool(name="shared", bufs=1))
        s_sh = spool.tile([P, NBLK * 49], dt.float32, tag="s_sh")
        nc.sync.dma_start(_blk(s_sh[:], 49),
                          aps["sd"].rearrange("(b p) x -> p b x", p=P))
        radios = {l: spool.tile([P, NBLK * 49], dt.float32, tag=f"radio{l}",
                                name=f"radio{l}")
                  for l in (0, 1)}

        MRG = {0: dict(ME=V, ME2=V, EE=V),
               1: dict(ME=V, ME2=V, EE=V)}
        HEM = {
            "l0a": dict(SE=V, SE2=V, XE=V, EE=V, TE=V, FE=G),
            "l0b": dict(SE=V, SE2=V, XE=V, EE=V, TE=V, FE=G),
            "l1a": dict(SE=V, SE2=V, XE=V, EE=V, TE=V, FE=G),
            "l1b": dict(SE=V, SE2=V, XE=V, EE=V, TE=V, FE=G),
        }

        gens = []
        setup_pool = ctx.enter_context(tc.tile_pool(name="setup", bufs=1))
        gens.append(_emit_setup(nc, setup_pool, s_sh, radios, accs, mask48[:],
                                aps))
        mouts = {}
        for lvl in (0, 1):
            if f"l{lvl}" not in parts:
                continue
            mouts[lvl] = {}
            mp = ctx.enter_context(tc.tile_pool(name=f"mrg{lvl}", bufs=1))
            gens.append(_emit_level_merge(nc, tc, mp, lvl, s_sh,
                                          aps[f"ps{lvl}"], aps[f"pw{lvl}"],
                                          mouts[lvl], MRG[lvl]))
        gens.append(_emit_consts())
        if "hash" in parts:
            for lvl in (0, 1):
                hp2 = ctx.enter_context(tc.tile_pool(name=f"hash{lvl}", bufs=1))
                gens.append(_emit_hash(nc, hp2, lvl, ones_h[:], accs["hash"],
                                       aps, first=(lvl == 0)))
        for name, lvl, b0 in HALVES:
            if f"l{lvl}" not in parts:
                continue
            hp = ctx.enter_context(tc.tile_pool(name=name, bufs=1))
            gens.append(_emit_half(nc, hp, lvl, b0, s_sh, radios[lvl],
                                   mouts[lvl], lvl_consts[lvl], accs[name],
                                   HEM[name]))

        while gens:
            nxt = []
            for g in gens:
                try:
                    next(g)
                    nxt.append(g)
                except StopIteration:
                    pass
            gens = nxt

        with tc.tile_pool(name="fin", bufs=1) as pool:
            tot = pool.tile([P, 1], dt.float32, tag="tot")
            V.tensor_scalar(tot[:], accs["rgb"][:], W_RGB / (R * 3), None,
                            Alu.mult)
            for snm, lvl, _ in HALVES:
                V.scalar_tensor_tensor(tot[:], accs[snm][:],
                                       W_INTER / (R * (LVL[lvl]["X"] - 1)),
                                       tot[:], Alu.mult, Alu.add)
            V.scalar_tensor_tensor(tot[:], accs["p1"][:], W_DIST * 2.0 / R,
                                   tot[:], Alu.mult, Alu.add)
            V.scalar_tensor_tensor(tot[:], accs["p2"][:], W_DIST / (3.0 * R),
                                   tot[:], Alu.mult, Alu.add)
            V.scalar_tensor_tensor(tot[:], accs["hash"][:],
                                   W_HASH / (NUM_SEGMENTS * 2.0), tot[:],
                                   Alu.mult, Alu.add)
            res = pool.tile([1, 1], dt.float32, tag="res")
            G.tensor_reduce(res[:], tot[:], AX.C, Alu.add)
            nc.sync.dma_start(out_ap, res[:])


# ---------------- host side ----------------
_module_cache = {}


def _get_module():
    if "nc" not in _module_cache:
        _module_cache["nc"] = build_module()
    return _module_cache["nc"]


def shard_inputs(inputs):
    f32 = np.float32
    pd = np.ascontiguousarray(inputs["pd_rgbs"], f32)
    gt = np.ascontiguousarray(inputs["gt_rgbs"], f32)
    sd = np.ascontiguousarray(inputs["render_sdist"], f32)
    rw = np.ascontiguousarray(inputs["render_weights"], f32)
    ps0 = np.ascontiguousarray(inputs["prop_sdist_0"], f32)
    pw0 = np.ascontiguousarray(inputs["prop_weights_0"], f32)
    ps1 = np.ascontiguousarray(inputs["prop_sdist_1"], f32)
    pw1 = np.ascontiguousarray(inputs["prop_weights_1"], f32)
    hashes = {}
    for lvl in (0, 1):
        idx = np.asarray(inputs[f"enc_idx_{lvl}"]).astype(np.int32)
        emb = np.ascontiguousarray(inputs[f"enc_embds_{lvl}"], f32)
        idx_pad = np.full(M + 2 * HALO, -1, np.int32)
        idx_pad[HALO:HALO + M] = idx
        emb_pad = np.zeros((M + 2 * HALO, 2), f32)
        emb_pad[HALO:HALO + M] = emb
        hashes[lvl] = (idx_pad, emb_pad)

    consts = {}
    rep = lambda row: np.ascontiguousarray(np.tile(row, (P, 1)))
    for lvl, L in LVL.items():
        EW, QWS, X = L["EW"], L["QWS"], L["X"]
        NL = NB * EW
        NQ = NB * QWS
        io49m = np.zeros(NB * EW, np.uint16)            # unused slot
        io49p = np.full(NB * EW, NB * 49, np.uint16)    # ep dest offset
        ioG = np.concatenate([2 * np.arange(b * EW, (b + 1) * EW,
                                            dtype=np.uint16)
                              for b in range(NB)])
        # query dest: rank-1 + b*QWS ... C' = C + 98b so fold +98b here; the
        # combined-scatter also needs the em/ep region offset handled via
        # t1's own terms, and query dests must land in [0, NQ): ioQ2 value
        # = i+1 + b*QWS + 98b  (so (ioQ2 - C')*qf - 1 = rank-1 + b*QWS)
        ioQ2 = np.concatenate([np.arange(1, EW + 1, dtype=np.uint16)
                               + b * QWS + 98 * b for b in range(NB)])
        packed = np.concatenate([io49m, io49p, ioG, ioQ2]).astype(np.uint16)
        consts[f"c_u16_l{lvl}"] = rep(packed.view(np.int16))
        msk = np.ones(NL, f32)
        msk[::EW] = 0.0
        mcnt = np.ones(NL, f32)
        for b in range(NB):
            mcnt[b * EW] = b
        consts[f"c_maskf_l{lvl}"] = rep(np.concatenate([msk, mcnt]))
    m48 = np.ones(NBLK * 48, f32)
    m48[::48] = 0.0
    consts["c_mask48"] = rep(m48)
    consts["c_ones"] = rep(np.ones(HCOLS, f32))

    in_maps = []
    for c in range(N_CORES):
        r0 = c * RPC
        lo = c * MPC
        im = {
            "pd": pd[r0:r0 + RPC], "gt": gt[r0:r0 + RPC],
            "sd": sd[r0:r0 + RPC], "rw": rw[r0:r0 + RPC],
            "ps0": ps0[r0:r0 + RPC], "pw0": pw0[r0:r0 + RPC],
            "ps1": ps1[r0:r0 + RPC], "pw1": pw1[r0:r0 + RPC],
        }
        for lvl in (0, 1):
            idx_pad, emb_pad = hashes[lvl]
            im[f"hi{lvl}"] = np.ascontiguousarray(idx_pad[lo:lo + HSLICE])
            im[f"he{lvl}"] = np.ascontiguousarray(
                emb_pad[lo:lo + HSLICE].reshape(-1))
        im.update(consts)
        in_maps.append(im)
    return in_maps


def kernel(**inputs) -> np.ndarray:
    nc = _get_module()
    in_maps = shard_inputs(inputs)
    res = run_bass_kernel_spmd(nc, in_maps, core_ids=list(range(N_CORES)))
    total = np.float64(0.0)
    for r in res.results:
        total += np.float64(r["out"][0, 0])
    return np.float32(total)


# revision 4
# speedup vs baseline: 1.5125x; 1.0039x over previous
"""Trainium2 Bass kernel v2 for nn_Loss_dict_50646254354805 (NeRF-style loss).

v2 vs baseline:
- bitonic merges on uint16 quantized keys (value*15000 + 2 tag bits) -> DVE
  2x perf mode; keys determine ORDER only.
- exact f32 values (queries/em/ep) and radio reach the merged domain via
  batched u16-half local_scatters through one shared index table (idxcat):
  merged positions come from the C/Cm count scans.
- one merge per level; post-merge work split into two 2-block half-streams
  with per-stream engine maps; all generators emitted stage-interleaved so
  DVE / Pool / Act overlap.
"""
import numpy as np

import concourse.bass as bass
import concourse.mybir as mybir
import concourse.tile as tile
from concourse import bacc
from concourse.bass_utils import run_bass_kernel_spmd

dt = mybir.dt
Alu = mybir.AluOpType
AX = mybir.AxisListType
Act = mybir.ActivationFunctionType
P = 128

PULSE = (0.01, 0.005)
W_RGB, W_INTER, W_DIST, W_HASH = 1.0, 1.0, 0.01, 0.1
NUM_SEGMENTS = 65536
R, N = 4096, 48
M = R * N
N_CORES = 8
RPC = R // N_CORES
NBLK = RPC // P               # 4 ray blocks per core
MPC = M // N_CORES
HALO = 64
HROW = MPC // P
HCOLS = HROW + HALO + 1
HSLICE = HALO + MPC + HALO

VOFF = 0.97
QS = 15000.0                  # key quantization scale
PADK = 0xFFFC

LVL = {0: dict(X=257, n2=512), 1: dict(X=97, n2=256)}
for _L in LVL.values():
    _L["EW"] = ((_L["X"] + 98 + 1 + 7) // 8) * 8        # 360 / 200
    _L["QWS"] = _L["EW"] - 98                           # 262 / 102

NB = 2                        # blocks per half-stream
HALVES = [("l0a", 0, 0), ("l0b", 0, 2), ("l1a", 1, 0), ("l1b", 1, 2)]


def _blk(ap, n):
    return ap.rearrange("p (b n) -> p b n", n=n)


def _merge(eng, bufa, bufb, width, ew=None):
    """Ascending bitonic merge over [P, NBLK*width] u16 ping-pong tiles.

    If ew is given, only outputs [0, ew+2d-1] of each block are needed
    downstream, so late stages skip whole 2d-chunks beyond that window."""
    cur, nxt = bufa, bufb
    d = width // 2
    while d >= 1:
        nch = width // (2 * d)
        keep = nch
        if ew is not None:
            keep = min(nch, -(-(ew + 2 * d - 1) // (2 * d)))
        if keep == nch:
            c3 = cur[:].rearrange("p (c td) -> p c td", td=2 * d)
            n3 = nxt[:].rearrange("p (c td) -> p c td", td=2 * d)
        else:
            c3 = cur[:].rearrange("p (b c td) -> p (b c) td",
                                  td=2 * d, c=nch)[: , 0:0]  # placeholder
        if keep == nch:
            lo_in, hi_in = c3[:, :, 0:d], c3[:, :, d:2 * d]
            eng.tensor_tensor(n3[:, :, 0:d], lo_in, hi_in, Alu.min)
            eng.tensor_tensor(n3[:, :, d:2 * d], lo_in, hi_in, Alu.max)
        else:
            c4 = cur[:].rearrange("p (b c td) -> p b c td", td=2 * d, c=nch)
            n4 = nxt[:].rearrange("p (b c td) -> p b c td", td=2 * d, c=nch)
            lo_in = c4[:, :, 0:keep, 0:d]
            hi_in = c4[:, :, 0:keep, d:2 * d]
            eng.tensor_tensor(n4[:, :, 0:keep, 0:d], lo_in, hi_in, Alu.min)
            eng.tensor_tensor(n4[:, :, 0:keep, d:2 * d], lo_in, hi_in, Alu.max)
        cur, nxt = nxt, cur
        d //= 2
    return cur


def _emit_level_merge(nc, tc, pool, lvl, s_sh, x_ap, pwt_ap, out, eng):
    """Generator: quantize + b1/b2 merges for all 4 blocks of one level."""
    ME, ME2, EE = eng["ME"], eng["ME2"], eng["EE"]
    AE = nc.scalar
    L = LVL[lvl]
    X, n2 = L["X"], L["n2"]
    pw = PULSE[lvl]

    xt = pool.tile([P, NBLK * X], dt.float32, tag="xt")
    nc.sync.dma_start(_blk(xt[:], X), x_ap.rearrange("(b p) x -> p b x", p=P))
    pwt = pool.tile([P, NBLK * (X - 1)], dt.float32, tag="pwt")
    nc.sync.dma_start(_blk(pwt[:], X - 1),
                      pwt_ap.rearrange("(b p) x -> p b x", p=P))
    out["xt"] = xt
    out["pwt"] = pwt

    b2a = pool.tile([P, NBLK * n2], dt.uint16, tag="b2a")
    b2b = pool.tile([P, NBLK * n2], dt.uint16, tag="b2b")
    b2a3 = _blk(b2a[:], n2)
    b1a = pool.tile([P, NBLK * 128], dt.uint16, tag="b1a")
    b1b = pool.tile([P, NBLK * 128], dt.uint16, tag="b1b")
    nc.gpsimd.memset(b1a[:], PADK)
    b1a3 = _blk(b1a[:], 128)
    emq = pool.tile([P, NBLK * 49], dt.uint16, tag="emq")
    EE.tensor_scalar(emq[:], s_sh[:], QS, (1.0 - pw - VOFF) * QS + 0.5,
                     Alu.mult, Alu.add)
    epq = pool.tile([P, NBLK * 49], dt.uint16, tag="epq")
    EE.tensor_scalar(epq[:], s_sh[:], QS, (1.0 + pw - VOFF) * QS + 0.5,
                     Alu.mult, Alu.add)
    EE.tensor_scalar(b1a3[:, :, 0:49], _blk(emq[:], 49), 4, 1,
                     Alu.mult, Alu.add)
    EE.tensor_scalar(b1a3[:, :, 79:128][:, :, ::-1], _blk(epq[:], 49), 4, 3,
                     Alu.mult, Alu.add)
    yield
    b1 = _merge(ME, b1a, b1b, 128, ew=98)
    yield
    nc.gpsimd.memset(b2a3[:, :, X:n2 - 128], PADK)
    xq = pool.tile([P, NBLK * X], dt.uint16, tag="xq")
    EE.tensor_scalar(xq[:], xt[:], QS, (1.0 - VOFF) * QS + 0.5,
                     Alu.mult, Alu.add)
    EE.tensor_scalar(b2a3[:, :, 0:X], _blk(xq[:], X), 4, None, Alu.mult)
    EE.tensor_copy(b2a3[:, :, n2 - 128:n2][:, :, ::-1], _blk(b1[:], 128))
    yield
    out["m"] = _merge(ME2, b2a, b2b, n2, ew=L["EW"])
    yield


def _emit_half(nc, pool, lvl, b0, s_sh, radio_full, mout, consts, acc, eng):
    """Generator: post-merge pipeline for blocks [b0, b0+NB) of one level."""
    SE, XE, EE, FE = (eng[k] for k in ("SE", "XE", "EE", "FE"))
    TE = eng.get("TE", EE)
    SE2 = eng.get("SE2", SE)
    AE = nc.scalar
    L = LVL[lvl]
    X, n2, EW, QWS = L["X"], L["n2"], L["EW"], L["QWS"]
    NL = NB * EW
    NQ = NB * QWS
    NE = NB * 49
    VW = NQ + 2 * NE          # vcat width: [x | em | ep]
    pw = PULSE[lvl]
    maskf, mask_cnt, io49p, ioG, ioQ2 = consts

    def blkE(ap):
        return ap.rearrange("p (b n) -> p b n", b=NB)

    ss = s_sh[:][:, b0 * 49:(b0 + NB) * 49]

    # ---------- sources: exact values + radio (independent of merge) ----------
    vcat = pool.tile([P, VW], dt.float32, tag="vcat")
    nc.gpsimd.memset(_blk(vcat[:, 0:NQ], QWS)[:, :, X:QWS], 0.0)
    radcat = pool.tile([P, 2 * NE], dt.float32, tag="radcat")
    rsl_ = radio_full[:][:, b0 * 49:(b0 + NB) * 49]
    FE.tensor_copy(radcat[:, 0:NE], rsl_)
    FE.tensor_scalar(radcat[:, NE:2 * NE], radcat[:, 0:NE], -1.0, None, Alu.mult)
    yield
    # wait for merge result
    while "m" not in mout:
        yield
    m = mout["m"]
    xt, pwt_full = mout["xt"], mout["pwt"]
    mSh = _blk(m[:], n2)[:, b0:b0 + NB, 0:EW]       # [P, NB, EW] strided
    xts = _blk(xt[:], X)[:, b0:b0 + NB]             # [P, NB, X]
    pwt = _blk(pwt_full[:], X - 1)[:, b0:b0 + NB]
    AE.activation(_blk(vcat[:, 0:NQ], QWS)[:, :, 0:X], xts, Act.Copy)
    AE.activation(_blk(vcat[:, NQ:NQ + NE], 49), _blk(ss, 49), Act.Copy, bias=-pw)
    AE.activation(_blk(vcat[:, NQ + NE:VW], 49), _blk(ss, 49), Act.Copy, bias=pw)
    yield

    # ---------- tags + counts ----------
    tagb = pool.tile([P, NL], dt.uint16, tag="tagb")
    XE.tensor_scalar(blkE(tagb[:]), mSh, 3, None, Alu.bitwise_and)
    ev_f = pool.tile([P, NL], dt.uint16, tag="ev_f")
    TE.tensor_scalar(ev_f[:], tagb[:], 1, None, Alu.bitwise_and)
    em_f = pool.tile([P, NL], dt.uint16, tag="em_f")
    TE.tensor_scalar(em_f[:], tagb[:], 1, None, Alu.is_equal)
    ep_f = pool.tile([P, NL], dt.uint16, tag="ep_f")
    TE.tensor_scalar(ep_f[:], tagb[:], 3, None, Alu.is_equal)
    yield
    C = pool.tile([P, NL], dt.uint16, tag="C")
    SE.tensor_tensor_scan(C[:], mask_cnt, ev_f[:], 0.0, Alu.mult, Alu.add)
    Cm = pool.tile([P, NL], dt.uint16, tag="Cm")
    SE.tensor_tensor_scan(Cm[:], mask_cnt, em_f[:], 0.0, Alu.mult, Alu.add)
    yield

    # ---------- idxcat: merged position of every source element ----------
    t1 = tagb                                       # tagb dead after masks
    t2 = pool.tile([P, NL], dt.uint16, tag="t2")
    t3 = pool.tile([P, NL], dt.uint16, tag="t3")
    # block offsets (49b/98b) ride in from the mask_cnt scan carry; section
    # offsets NQ / NQ+NE are flat immediates. One combined scatter:
    # t1 = (Cm'+NQ)*em + (C'-Cm'+NQ+NE)*ep + (ioQ2-C')*qf - 1
    EE.tensor_tensor(t2[:], C[:], Cm[:], Alu.subtract)
    EE.tensor_scalar(t2[:], t2[:], NQ + NE, None, Alu.add)
    EE.tensor_tensor(t2[:], t2[:], ep_f[:], Alu.mult)
    EE.tensor_scalar(t1[:], Cm[:], NQ, None, Alu.add)
    EE.tensor_tensor(t1[:], t1[:], em_f[:], Alu.mult)
    EE.tensor_tensor(t1[:], t1[:], t2[:], Alu.add)
    qf = em_f                                       # em_f dead after t1
    TE.tensor_scalar(qf[:], ev_f[:], 0, None, Alu.is_equal)
    EE.tensor_tensor(t3[:], ioQ2, C[:], Alu.subtract)
    EE.tensor_tensor(t3[:], t3[:], qf[:], Alu.mult)
    EE.tensor_tensor(t1[:], t1[:], t3[:], Alu.add)
    EE.tensor_scalar(t1[:], t1[:], 1, None, Alu.subtract)       # idx all
    EE.tensor_scalar(t2[:], t3[:], 1, None, Alu.subtract)       # idxq
    yield
    idxcat = pool.tile([P, VW], dt.uint16, tag="idxcat")
    nc.gpsimd.local_scatter(idxcat[:].bitcast(dt.int16), ioG,
                            t1[:].bitcast(dt.int16), channels=P,
                            num_elems=VW, num_idxs=NL)
    idx2 = pool.tile([P, 2 * VW], dt.uint16, tag="idx2")
    i2v = idx2[:].rearrange("p (n two) -> p n two", two=2)
    AE.activation(i2v[:, :, 0], idxcat[:], Act.Copy)
    AE.activation(i2v[:, :, 1], idxcat[:], Act.Copy, bias=1.0)
    yield

    # ---------- pair-scatter exact values + radio into merged domain ----------
    v = pool.tile([P, NL], dt.float32, tag="v")
    nc.gpsimd.local_scatter(v[:].bitcast(dt.int16),
                            vcat[:].bitcast(dt.int16),
                            idx2[:].bitcast(dt.int16), channels=P,
                            num_elems=2 * NL, num_idxs=2 * VW)
    F1 = pool.tile([P, NL], dt.float32, tag="F1")   # radio_m
    nc.gpsimd.local_scatter(F1[:].bitcast(dt.int16),
                            radcat[:].bitcast(dt.int16),
                            idx2[:, 2 * NQ:2 * VW].bitcast(dt.int16), channels=P,
                            num_elems=2 * NL, num_idxs=4 * NE)
    yield

    # ---------- density reconstruction ----------
    F2 = pool.tile([P, NL], dt.float32, tag="F2")
    SE2.tensor_tensor_scan(F2[:], maskf, F1[:], 0.0, Alu.mult, Alu.add)  # g
    dv = pool.tile([P, NL], dt.float32, tag="dv")
    dv3 = blkE(dv[:])
    v3 = blkE(v[:])
    nc.gpsimd.memset(dv3[:, :, 0:1], 0.0)
    FE.tensor_tensor(dv3[:, :, 1:EW], v3[:, :, 1:EW], v3[:, :, 0:EW - 1],
                     Alu.subtract)
    yield
    wg = v                                          # v dead after dv
    wg3 = blkE(wg[:])
    nc.gpsimd.memset(wg3[:, :, 0:1], 0.0)
    FE.tensor_tensor(wg3[:, :, 1:EW], dv3[:, :, 1:EW],
                     blkE(F2[:])[:, :, 0:EW - 1], Alu.mult)
    w_t = F1                                        # radio dead after g
    SE2.tensor_tensor_scan(w_t[:], maskf, wg[:], 0.0, Alu.mult, Alu.add)
    yield
    wc = wg                                         # wg dead
    AE.activation(wc[:], w_t[:], Act.Relu, scale=0.5)
    scr = pool.tile([P, NL], dt.float32, tag="scr")
    wc3 = blkE(wc[:])
    s3_ = blkE(scr[:])
    nc.gpsimd.memset(s3_[:, :, 0:1], 0.0)
    FE.tensor_tensor(s3_[:, :, 1:EW], wc3[:, :, 1:EW], wc3[:, :, 0:EW - 1],
                     Alu.add)
    area = w_t                                      # w dead after wc
    a3 = blkE(area[:])
    nc.gpsimd.memset(a3[:, :, 0:1], 0.0)
    FE.tensor_tensor(a3[:, :, 1:EW], s3_[:, :, 1:EW],
                     dv3[:, :, 1:EW], Alu.mult)
    cdf = F2                                        # g dead after wg
    SE2.tensor_tensor_scan(cdf[:], maskf, area[:], 0.0, Alu.mult, Alu.add)
    yield

    # ---------- compact cdf at query slots (pair-scatter) ----------
    idx2q = idx2                                    # idx2 dead after scatters
    i2qv = idx2q[:][:, 0:2 * NL].rearrange("p (n two) -> p n two", two=2)
    tq = tagb                                       # dead
    AE.activation(tq[:], t2[:], Act.Copy, scale=2.0)
    AE.activation(i2qv[:, :, 0], tq[:], Act.Copy)
    AE.activation(i2qv[:, :, 1], tq[:], Act.Copy, bias=1.0)
    cdfq = vcat                                     # vcat dead after v scatter
    cqn = cdfq[:][:, 0:NQ]
    nc.gpsimd.local_scatter(cqn.bitcast(dt.int16),
                            cdf[:].bitcast(dt.int16),
                            idx2q[:][:, 0:2 * NL].bitcast(dt.int16), channels=P,
                            num_elems=2 * NQ, num_idxs=2 * NL)
    yield

    # ---------- loss tail ----------
    NW = NB * (X - 1)
    ws = scr                                        # dead after area
    ws2 = ws[:][:, 0:NW]
    cqf = _blk(cdfq[:][:, 0:NQ], QWS)
    FE.tensor_tensor(_blk(ws2, X - 1), cqf[:, :, 1:X], cqf[:, :, 0:X - 1],
                     Alu.subtract)
    FE.tensor_tensor(_blk(ws2, X - 1), _blk(ws2, X - 1), pwt, Alu.subtract)
    den = area                                      # dead after cdf
    den2 = den[:][:, 0:NW]
    AE.activation(_blk(den2, X - 1), pwt, Act.Copy, bias=1e-5)
    nc.vector.reciprocal(den2, den2)
    rsl = dv                                        # dead after area
    AE.activation(rsl[:][:, 0:NW], ws2, Act.Relu)
    FE.tensor_tensor(ws2, ws2, rsl[:][:, 0:NW], Alu.mult)
    FE.tensor_tensor(ws2, ws2, den2, Alu.mult)
    nc.vector.tensor_reduce(acc[:], _blk(ws2, X - 1), AX.XY, Alu.add)
    yield


def _emit_setup(nc, pool, s_sh, radios, accs, mask48, aps):
    V, G = nc.vector, nc.gpsimd
    rw_sh = pool.tile([P, NBLK * 48], dt.float32, tag="rw_sh")
    nc.sync.dma_start(_blk(rw_sh[:], 48),
                      aps["rw"].rearrange("(b p) x -> p b x", p=P))
    s3 = _blk(s_sh[:], 49)
    ds = pool.tile([P, NBLK * 48], dt.float32, tag="ds")
    V.tensor_tensor(_blk(ds[:], 48), s3[:, :, 1:49], s3[:, :, 0:48], Alu.subtract)
    dse = pool.tile([P, NBLK * 48], dt.float32, tag="dse")
    nc.scalar.activation(dse[:], ds[:], Act.Copy, bias=1e-8)
    V.reciprocal(dse[:], dse[:])
    wnorm = pool.tile([P, NBLK * 48], dt.float32, tag="wnorm")
    V.tensor_tensor(wnorm[:], rw_sh[:], dse[:], Alu.mult)
    wnp = pool.tile([P, NBLK * 50], dt.float32, tag="wnp")
    G.memset(wnp[:], 0.0)
    V.tensor_copy(_blk(wnp[:], 50)[:, :, 1:49], _blk(wnorm[:], 48))
    diff = pool.tile([P, NBLK * 49], dt.float32, tag="diff")
    wnp3 = _blk(wnp[:], 50)
    V.tensor_tensor(_blk(diff[:], 49), wnp3[:, :, 1:50], wnp3[:, :, 0:49],
                    Alu.subtract)
    for lvl in (0, 1):
        V.tensor_scalar(radios[lvl][:], diff[:], 1.0 / (2 * PULSE[lvl]), None,
                        Alu.mult)
    yield

    mid = pool.tile([P, NBLK * 48], dt.float32, tag="mid")
    V.tensor_tensor(_blk(mid[:], 48), s3[:, :, 1:49], s3[:, :, 0:48], Alu.add)
    wm = pool.tile([P, NBLK * 48], dt.float32, tag="wm")
    V.scalar_tensor_tensor(wm[:], mid[:], 0.5, rw_sh[:], Alu.mult, Alu.mult)
    Cin = pool.tile([P, NBLK * 48], dt.float32, tag="Cin")
    V.tensor_tensor_scan(Cin[:], mask48, rw_sh[:], 0.0, Alu.mult, Alu.add)
    Sin = pool.tile([P, NBLK * 48], dt.float32, tag="Sin")
    V.tensor_tensor_scan(Sin[:], mask48, wm[:], 0.0, Alu.mult, Alu.add)
    yield
    A = pool.tile([P, NBLK * 47], dt.float32, tag="A47")
    m3 = _blk(mid[:], 48)
    c3 = _blk(Cin[:], 48)
    sw3 = _blk(Sin[:], 48)
    rw3 = _blk(rw_sh[:], 48)
    A3 = _blk(A[:], 47)
    V.scalar_tensor_tensor(A3, m3[:, :, 1:48], 0.5, c3[:, :, 0:47],
                           Alu.mult, Alu.mult)
    V.tensor_tensor(A3, A3, sw3[:, :, 0:47], Alu.subtract)
    V.tensor_tensor(A3, A3, rw3[:, :, 1:48], Alu.mult)
    V.tensor_reduce(accs["p1"][:], A3, AX.XY, Alu.add)
    t2 = pool.tile([P, NBLK * 48], dt.float32, tag="t2d")
    G.tensor_tensor(t2[:], rw_sh[:], rw_sh[:], Alu.mult)
    G.tensor_tensor(t2[:], t2[:], ds[:], Alu.mult)
    V.tensor_reduce(accs["p2"][:], _blk(t2[:], 48), AX.XY, Alu.add)
    yield

    pdt = pool.tile([P, NBLK * 3], dt.float32, tag="pdt")
    gtt = pool.tile([P, NBLK * 3], dt.float32, tag="gtt")
    nc.sync.dma_start(_blk(pdt[:], 3), aps["pd"].rearrange("(b p) c -> p b c", p=P))
    nc.sync.dma_start(_blk(gtt[:], 3), aps["gt"].rearrange("(b p) c -> p b c", p=P))
    d = pool.tile([P, NBLK * 3], dt.float32, tag="rgbd")
    V.tensor_tensor(d[:], pdt[:], gtt[:], Alu.subtract)
    V.tensor_tensor(d[:], d[:], d[:], Alu.mult)
    V.tensor_reduce(accs["rgb"][:], d[:], AX.X, Alu.add)
    yield


def _emit_hash(nc, pool, lvl, ones_h, acc, aps, first):
    E = nc.gpsimd if lvl == 0 else nc.vector
    idx = pool.tile([P, HCOLS], dt.int32, tag="hidx")
    src = aps[f"hi{lvl}"]
    nc.sync.dma_start(idx[:], bass.AP(tensor=src.tensor, offset=src.offset,
                                      ap=[[HROW, P], [1, HCOLS]]))
    emb = pool.tile([P, HCOLS * 2], dt.float32, tag="hemb")
    esrc = aps[f"he{lvl}"]
    nc.sync.dma_start(emb[:], bass.AP(tensor=esrc.tensor, offset=esrc.offset,
                                      ap=[[HROW * 2, P], [1, HCOLS * 2]]))
    sq = pool.tile([P, HCOLS * 2], dt.float32, tag="hsq")
    E.tensor_tensor(sq[:], emb[:], emb[:], Alu.mult)
    wv = pool.tile([P, HCOLS], dt.float32, tag="hw")
    sq3 = sq[:].rearrange("p (n two) -> p n two", two=2)
    E.tensor_tensor(wv[:], sq3[:, :, 0], sq3[:, :, 1], Alu.add)
    eq = pool.tile([P, HCOLS], dt.float32, tag="heq")
    nc.gpsimd.memset(eq[:, 0:1], 0.0)
    nc.vector.tensor_tensor(eq[:, 1:HCOLS], idx[:, 1:HCOLS], idx[:, 0:HCOLS - 1],
                             Alu.is_equal)
    yield
    S = pool.tile([P, HCOLS], dt.float32, tag="hS")
    nc.vector.tensor_tensor_scan(S[:], eq[:], wv[:], 0.0, Alu.mult, Alu.add)
    cc = pool.tile([P, HCOLS], dt.float32, tag="hcc")
    nc.vector.tensor_tensor_scan(cc[:], eq[:], ones_h, 0.0, Alu.mult, Alu.add)
    yield
    ratio = pool.tile([P, HCOLS], dt.float32, tag="hr")
    nc.vector.reciprocal(cc[:], cc[:])
    E.tensor_tensor(ratio[:], S[:], cc[:], Alu.mult)
    me = pool.tile([P, HCOLS], dt.float32, tag="hme")
    nc.scalar.activation(me[:, 0:HCOLS - 1], eq[:, 1:HCOLS], Act.Copy,
                         bias=1.0, scale=-1.0)
    E.tensor_tensor(ratio[:, HALO:HALO + HROW], ratio[:, HALO:HALO + HROW],
                    me[:, HALO:HALO + HROW], Alu.mult)
    if first:
        nc.vector.tensor_reduce(acc[:], ratio[:, HALO:HALO + HROW], AX.X, Alu.add)
    else:
        part = pool.tile([P, 1], dt.float32, tag="hpart")
        nc.vector.tensor_reduce(part[:], ratio[:, HALO:HALO + HROW], AX.X,
                                Alu.add)
        E.tensor_tensor(acc[:], acc[:], part[:], Alu.add)
    yield


def build_module(parts=("rgb", "dist", "hash", "l0", "l1")):
    nc = bacc.Bacc("TRN2", target_bir_lowering=False, debug=False,
                   enable_asserts=False, num_devices=N_CORES)
    aps = {}

    def din(name, shape, dtype=dt.float32):
        aps[name] = nc.dram_tensor(name, shape, dtype, kind="ExternalInput").ap()
    din("pd", [RPC, 3]); din("gt", [RPC, 3])
    din("sd", [RPC, 49]); din("rw", [RPC, 48])
    din("ps0", [RPC, 257]); din("pw0", [RPC, 256])
    din("ps1", [RPC, 97]); din("pw1", [RPC, 96])
    din("hi0", [HSLICE], dt.int32); din("he0", [HSLICE * 2])
    din("hi1", [HSLICE], dt.int32); din("he1", [HSLICE * 2])
    for lvl, L in LVL.items():
        NL = NB * L["EW"]
        din(f"c_u16_l{lvl}", [P, 4 * NL], dt.int16)
        din(f"c_maskf_l{lvl}", [P, 2 * NL])
    din("c_mask48", [P, NBLK * 48]); din("c_ones", [P, HCOLS])
    out_ap = nc.dram_tensor("out", [1, 1], dt.float32, kind="ExternalOutput").ap()

    with tile.TileContext(nc) as tc:
        _emit(nc, tc, aps, out_ap, parts)
    nc.compile()
    return nc


def _emit(nc, tc, aps, out_ap, parts=("rgb", "dist", "hash", "l0", "l1")):
    import contextlib
    V, G = nc.vector, nc.gpsimd
    with contextlib.ExitStack() as ctx:
        spool = ctx.enter_context(tc.tile_pool(name="shared", bufs=1))
        s_sh = spool.tile([P, NBLK * 49], dt.float32, tag="s_sh")
        nc.sync.dma_start(_blk(s_sh[:], 49),
                          aps["sd"].rearrange("(b p) x -> p b x", p=P))
        radios = {l: spool.tile([P, NBLK * 49], dt.float32, tag=f"radio{l}",
                                name=f"radio{l}")
                  for l in (0, 1)}

        cpool = ctx.enter_context(tc.tile_pool(name="consts", bufs=1))
        mask48 = cpool.tile([P, NBLK * 48], dt.float32, tag="mask48")
        ones_h = cpool.tile([P, HCOLS], dt.float32, tag="ones_h")
        lvl_consts = {}
        cdma = []
        cdma.append((mask48[:], aps["c_mask48"]))
        cdma.append((ones_h[:], aps["c_ones"]))
        for lvl, L in LVL.items():
            NL = NB * L["EW"]
            cu = cpool.tile([P, 4 * NL], dt.int16, tag=f"cu16_{lvl}",
                            name=f"cu16_{lvl}")
            mf = cpool.tile([P, 2 * NL], dt.float32, tag=f"maskf_{lvl}",
                            name=f"maskf_{lvl}")
            cdma.append((cu[:], aps[f"c_u16_l{lvl}"]))
            cdma.append((mf[:], aps[f"c_maskf_l{lvl}"]))
            cuv = cu[:].bitcast(dt.uint16)
            lvl_consts[lvl] = (mf[:][:, 0:NL], mf[:][:, NL:2 * NL],
                               cuv[:, NL:2 * NL],
                               cu[:][:, 2 * NL:3 * NL], cuv[:, 3 * NL:4 * NL])
            # (maskf, mask_cnt(f32), io49p, ioG(i16), ioQ2)

        def _emit_consts():
            for dst, src_ap in cdma:
                nc.sync.dma_start(dst, src_ap)
            yield

        accs = {}
        for name in ("rgb", "p1", "p2", "hash", "l0a", "l0b", "l1a", "l1b"):
            accs[name] = cpool.tile([P, 1], dt.float32, tag=f"acc_{name}",
                                    name=f"acc_{name}")
            V.memset(accs[name][:], 0.0)

        spool = ctx.enter_context(tc.tile_pool(name="shared", bufs=1))
        s_sh = spool.tile([P, NBLK * 49], dt.float32, tag="s_sh")
        nc.sync.dma_start(_blk(s_sh[:], 49),
                          aps["sd"].rearrange("(b p) x -> p b x", p=P))
        radios = {l: spool.tile([P, NBLK * 49], dt.float32, tag=f"radio{l}",
                                name=f"radio{l}")
                  for l in (0, 1)}

        MRG = {0: dict(ME=V, ME2=V, EE=V),
               1: dict(ME=V, ME2=V, EE=V)}
        HEM = {
            "l0a": dict(SE=V, SE2=V, XE=V, EE=V, TE=V, FE=G),
            "l0b": dict(SE=V, SE2=V, XE=V, EE=V, TE=V, FE=G),
            "l1a": dict(SE=V, SE2=V, XE=V, EE=V, TE=V, FE=G),
            "l1b": dict(SE=V, SE2=V, XE=V, EE=V, TE=V, FE=G),
        }

        gens = []
        setup_pool = ctx.enter_context(tc.tile_pool(name="setup", bufs=1))
        gens.append(_emit_setup(nc, setup_pool, s_sh, radios, accs, mask48[:],
                                aps))
        mouts = {}
        for lvl in (0, 1):
            if f"l{lvl}" not in parts:
                continue
            mouts[lvl] = {}
            mp = ctx.enter_context(tc.tile_pool(name=f"mrg{lvl}", bufs=1))
            gens.append(_emit_level_merge(nc, tc, mp, lvl, s_sh,
                                          aps[f"ps{lvl}"], aps[f"pw{lvl}"],
                                          mouts[lvl], MRG[lvl]))
        gens.append(_emit_consts())
        if "hash" in parts:
            for lvl in (0, 1):
                hp2 = ctx.enter_context(tc.tile_pool(name=f"hash{lvl}", bufs=1))
                gens.append(_emit_hash(nc, hp2, lvl, ones_h[:], accs["hash"],
                                       aps, first=(lvl == 0)))
        for name, lvl, b0 in HALVES:
            if f"l{lvl}" not in parts:
                continue
            hp = ctx.enter_context(tc.tile_pool(name=name, bufs=1))
            gens.append(_emit_half(nc, hp, lvl, b0, s_sh, radios[lvl],
                                   mouts[lvl], lvl_consts[lvl], accs[name],
                                   HEM[name]))

        while gens:
            nxt = []
            for g in gens:
                try:
                    next(g)
                    nxt.append(g)
                except StopIteration:
                    pass
            gens = nxt

        with tc.tile_pool(name="fin", bufs=1) as pool:
            tot = pool.tile([P, 1], dt.float32, tag="tot")
            V.tensor_scalar(tot[:], accs["rgb"][:], W_RGB / (R * 3), None,
                            Alu.mult)
            for snm, lvl, _ in HALVES:
                V.scalar_tensor_tensor(tot[:], accs[snm][:],
                                       W_INTER / (R * (LVL[lvl]["X"] - 1)),
                                       tot[:], Alu.mult, Alu.add)
            V.scalar_tensor_tensor(tot[:], accs["p1"][:], W_DIST * 2.0 / R,
                                   tot[:], Alu.mult, Alu.add)
            V.scalar_tensor_tensor(tot[:], accs["p2"][:], W_DIST / (3.0 * R),
                                   tot[:], Alu.mult, Alu.add)
            V.scalar_tensor_tensor(tot[:], accs["hash"][:],
                                   W_HASH / (NUM_SEGMENTS * 2.0), tot[:],
                                   Alu.mult, Alu.add)
            res = pool.tile([1, 1], dt.float32, tag="res")
            G.tensor_reduce(res[:], tot[:], AX.C, Alu.add)
            nc.sync.dma_start(out_ap, res[:])


# ---------------- host side ----------------
_module_cache = {}


def _get_module():
    if "nc" not in _module_cache:
        _module_cache["nc"] = build_module()
    return _module_cache["nc"]


def shard_inputs(inputs):
    f32 = np.float32
    pd = np.ascontiguousarray(inputs["pd_rgbs"], f32)
    gt = np.ascontiguousarray(inputs["gt_rgbs"], f32)
    sd = np.ascontiguousarray(inputs["render_sdist"], f32)
    rw = np.ascontiguousarray(inputs["render_weights"], f32)
    ps0 = np.ascontiguousarray(inputs["prop_sdist_0"], f32)
    pw0 = np.ascontiguousarray(inputs["prop_weights_0"], f32)
    ps1 = np.ascontiguousarray(inputs["prop_sdist_1"], f32)
    pw1 = np.ascontiguousarray(inputs["prop_weights_1"], f32)
    hashes = {}
    for lvl in (0, 1):
        idx = np.asarray(inputs[f"enc_idx_{lvl}"]).astype(np.int32)
        emb = np.ascontiguousarray(inputs[f"enc_embds_{lvl}"], f32)
        idx_pad = np.full(M + 2 * HALO, -1, np.int32)
        idx_pad[HALO:HALO + M] = idx
        emb_pad = np.zeros((M + 2 * HALO, 2), f32)
        emb_pad[HALO:HALO + M] = emb
        hashes[lvl] = (idx_pad, emb_pad)

    consts = {}
    rep = lambda row: np.ascontiguousarray(np.tile(row, (P, 1)))
    for lvl, L in LVL.items():
        EW, QWS, X = L["EW"], L["QWS"], L["X"]
        NL = NB * EW
        NQ = NB * QWS
        io49m = np.zeros(NB * EW, np.uint16)            # unused slot
        io49p = np.full(NB * EW, NB * 49, np.uint16)    # ep dest offset
        ioG = np.concatenate([2 * np.arange(b * EW, (b + 1) * EW,
                                            dtype=np.uint16)
                              for b in range(NB)])
        # query dest: rank-1 + b*QWS ... C' = C + 98b so fold +98b here; the
        # combined-scatter also needs the em/ep region offset handled via
        # t1's own terms, and query dests must land in [0, NQ): ioQ2 value
        # = i+1 + b*QWS + 98b  (so (ioQ2 - C')*qf - 1 = rank-1 + b*QWS)
        ioQ2 = np.concatenate([np.arange(1, EW + 1, dtype=np.uint16)
                               + b * QWS + 98 * b for b in range(NB)])
        packed = np.concatenate([io49m, io49p, ioG, ioQ2]).astype(np.uint16)
        consts[f"c_u16_l{lvl}"] = rep(packed.view(np.int16))
        msk = np.ones(NL, f32)
        msk[::EW] = 0.0
        mcnt = np.ones(NL, f32)
        for b in range(NB):
            mcnt[b * EW] = b
        consts[f"c_maskf_l{lvl}"] = rep(np.concatenate([msk, mcnt]))
    m48 = np.ones(NBLK * 48, f32)
    m48[::48] = 0.0
    consts["c_mask48"] = rep(m48)
    consts["c_ones"] = rep(np.ones(HCOLS, f32))

    in_maps = []
    for c in range(N_CORES):
        r0 = c * RPC
        lo = c * MPC
        im = {
            "pd": pd[r0:r0 + RPC], "gt": gt[r0:r0 + RPC],
            "sd": sd[r0:r0 + RPC], "rw": rw[r0:r0 + RPC],
            "ps0": ps0[r0:r0 + RPC], "pw0": pw0[r0:r0 + RPC],
            "ps1": ps1[r0:r0 + RPC], "pw1": pw1[r0:r0 + RPC],
        }
        for lvl in (0, 1):
            idx_pad, emb_pad = hashes[lvl]
            im[f"hi{lvl}"] = np.ascontiguousarray(idx_pad[lo:lo + HSLICE])
            im[f"he{lvl}"] = np.ascontiguousarray(
                emb_pad[lo:lo + HSLICE].reshape(-1))
        im.update(consts)
        in_maps.append(im)
    return in_maps


def kernel(**inputs) -> np.ndarray:
    nc = _get_module()
    in_maps = shard_inputs(inputs)
    res = run_bass_kernel_spmd(nc, in_maps, core_ids=list(range(N_CORES)))
    total = np.float64(0.0)
    for r in res.results:
        total += np.float64(r["out"][0, 0])
    return np.float32(total)


# revision 5
# speedup vs baseline: 1.5205x; 1.0053x over previous
"""Trainium2 Bass kernel v2 for nn_Loss_dict_50646254354805 (NeRF-style loss).

v2 vs baseline:
- bitonic merges on uint16 quantized keys (value*15000 + 2 tag bits) -> DVE
  2x perf mode; keys determine ORDER only.
- exact f32 values (queries/em/ep) and radio reach the merged domain via
  batched u16-half local_scatters through one shared index table (idxcat):
  merged positions come from the C/Cm count scans.
- one merge per level; post-merge work split into two 2-block half-streams
  with per-stream engine maps; all generators emitted stage-interleaved so
  DVE / Pool / Act overlap.
"""
import numpy as np

import concourse.bass as bass
import concourse.mybir as mybir
import concourse.tile as tile
from concourse import bacc
from concourse.bass_utils import run_bass_kernel_spmd

dt = mybir.dt
Alu = mybir.AluOpType
AX = mybir.AxisListType
Act = mybir.ActivationFunctionType
P = 128

PULSE = (0.01, 0.005)
W_RGB, W_INTER, W_DIST, W_HASH = 1.0, 1.0, 0.01, 0.1
NUM_SEGMENTS = 65536
R, N = 4096, 48
M = R * N
N_CORES = 8
RPC = R // N_CORES
NBLK = RPC // P               # 4 ray blocks per core
MPC = M // N_CORES
HALO = 64
HROW = MPC // P
HCOLS = HROW + HALO + 1
HSLICE = HALO + MPC + HALO

VOFF = 0.97
QS = 15000.0                  # key quantization scale
PADK = 0xFFFC

LVL = {0: dict(X=257, n2=512), 1: dict(X=97, n2=256)}
for _L in LVL.values():
    _L["EW"] = ((_L["X"] + 98 + 1 + 7) // 8) * 8        # 360 / 200
    _L["QWS"] = _L["EW"] - 98                           # 262 / 102

NB = 2                        # blocks per half-stream
HALVES = [("l0a", 0, 0), ("l0b", 0, 2), ("l1a", 1, 0), ("l1b", 1, 2)]


def _blk(ap, n):
    return ap.rearrange("p (b n) -> p b n", n=n)


def _ts_int(eng, out, in0, imm1, op0, imm2=None, op1=None):
    ins_ = [eng.lower_ap(in0), mybir.ImmediateValue(dtype=dt.int32, value=int(imm1))]
    kw = dict(op0=op0)
    if imm2 is not None:
        ins_.append(mybir.ImmediateValue(dtype=dt.int32, value=int(imm2)))
        kw["op1"] = op1
    return eng.add_instruction(mybir.InstTensorScalarPtr(
        name=eng.bass.get_next_instruction_name(),
        ins=ins_, outs=[eng.lower_ap(out)], **kw))

BIGF = 3.0


def _merge(eng, bufa, bufb, width, ew=None, trim4d=True):
    """Ascending bitonic merge over [P, NBLK*width] u16 ping-pong tiles.

    If ew is given, only outputs [0, ew+2d-1] of each block are needed
    downstream, so late stages skip whole 2d-chunks beyond that window."""
    cur, nxt = bufa, bufb
    d = width // 2
    while d >= 1:
        nch = width // (2 * d)
        keep = nch
        if ew is not None and trim4d:
            keep = min(nch, -(-(ew + 2 * d - 1) // (2 * d)))
        if keep == nch:
            c3 = cur[:].rearrange("p (c td) -> p c td", td=2 * d)
            n3 = nxt[:].rearrange("p (c td) -> p c td", td=2 * d)
        else:
            c3 = cur[:].rearrange("p (b c td) -> p (b c) td",
                                  td=2 * d, c=nch)[: , 0:0]  # placeholder
        if keep == nch:
            lo_in, hi_in = c3[:, :, 0:d], c3[:, :, d:2 * d]
            eng.tensor_tensor(n3[:, :, 0:d], lo_in, hi_in, Alu.min)
            eng.tensor_tensor(n3[:, :, d:2 * d], lo_in, hi_in, Alu.max)
        else:
            c4 = cur[:].rearrange("p (b c td) -> p b c td", td=2 * d, c=nch)
            n4 = nxt[:].rearrange("p (b c td) -> p b c td", td=2 * d, c=nch)
            lo_in = c4[:, :, 0:keep, 0:d]
            hi_in = c4[:, :, 0:keep, d:2 * d]
            eng.tensor_tensor(n4[:, :, 0:keep, 0:d], lo_in, hi_in, Alu.min)
            eng.tensor_tensor(n4[:, :, 0:keep, d:2 * d], lo_in, hi_in, Alu.max)
        cur, nxt = nxt, cur
        d //= 2
    return cur


def _emit_level_merge(nc, tc, pool, lvl, s_sh, x_ap, pwt_ap, out, eng):
    """Generator: quantize + b1/b2 merges for all 4 blocks of one level.

    lvl 0: uint16 quantized keys, merged on DVE (2x mode).
    lvl 1: f32-bitcast tagged keys (baseline-style), merged on Pool where
    f32 min/max is legal -- frees DVE during the big level-0 merge."""
    ME, ME2, EE = eng["ME"], eng["ME2"], eng["EE"]
    AE = nc.scalar
    fkeys = eng.get("fkeys", False)
    L = LVL[lvl]
    X, n2 = L["X"], L["n2"]
    pw = PULSE[lvl]
    kdt = dt.float32 if fkeys else dt.uint16

    xt = pool.tile([P, NBLK * X], dt.float32, tag="xt")
    nc.sync.dma_start(_blk(xt[:], X), x_ap.rearrange("(b p) x -> p b x", p=P))
    pwt = pool.tile([P, NBLK * (X - 1)], dt.float32, tag="pwt")
    nc.sync.dma_start(_blk(pwt[:], X - 1),
                      pwt_ap.rearrange("(b p) x -> p b x", p=P))
    out["xt"] = xt
    out["pwt"] = pwt

    b2a = pool.tile([P, NBLK * n2], kdt, tag="b2a")
    b2b = pool.tile([P, NBLK * n2], kdt, tag="b2b")
    b2a3 = _blk(b2a[:], n2)
    b1a = pool.tile([P, NBLK * 128], kdt, tag="b1a")
    b1b = pool.tile([P, NBLK * 128], kdt, tag="b1b")
    b1a3 = _blk(b1a[:], 128)
    if fkeys:
        nc.gpsimd.memset(b1a[:], BIGF)
        emsh = pool.tile([P, NBLK * 49], dt.float32, tag="emsh")
        AE.activation(emsh[:], s_sh[:], Act.Copy, bias=1.0 - pw)
        epsh = pool.tile([P, NBLK * 49], dt.float32, tag="epsh")
        AE.activation(epsh[:], s_sh[:], Act.Copy, bias=1.0 + pw)
        _ts_int(EE, b1a3[:, :, 0:49].bitcast(dt.int32),
                _blk(emsh[:], 49).bitcast(dt.int32), ~3, Alu.bitwise_and,
                1, Alu.bitwise_or)
        _ts_int(EE, b1a3[:, :, 79:128][:, :, ::-1].bitcast(dt.int32),
                _blk(epsh[:], 49).bitcast(dt.int32), ~3, Alu.bitwise_and,
                3, Alu.bitwise_or)
    else:
        nc.gpsimd.memset(b1a[:], PADK)
        emq = pool.tile([P, NBLK * 49], dt.uint16, tag="emq")
        EE.tensor_scalar(emq[:], s_sh[:], QS, (1.0 - pw - VOFF) * QS + 0.5,
                         Alu.mult, Alu.add)
        epq = pool.tile([P, NBLK * 49], dt.uint16, tag="epq")
        EE.tensor_scalar(epq[:], s_sh[:], QS, (1.0 + pw - VOFF) * QS + 0.5,
                         Alu.mult, Alu.add)
        EE.tensor_scalar(b1a3[:, :, 0:49], _blk(emq[:], 49), 4, 1,
                         Alu.mult, Alu.add)
        EE.tensor_scalar(b1a3[:, :, 79:128][:, :, ::-1], _blk(epq[:], 49), 4, 3,
                         Alu.mult, Alu.add)
    yield
    b1 = _merge(ME, b1a, b1b, 128, ew=98, trim4d=not fkeys)
    yield
    if fkeys:
        nc.gpsimd.memset(b2a3[:, :, X:n2 - 128], BIGF)
        xsh = pool.tile([P, NBLK * X], dt.float32, tag="xsh")
        AE.activation(xsh[:], xt[:], Act.Copy, bias=1.0)
        _ts_int(EE, b2a3[:, :, 0:X].bitcast(dt.int32),
                _blk(xsh[:], X).bitcast(dt.int32), ~3, Alu.bitwise_and)
        EE.tensor_copy(b2a3[:, :, n2 - 128:n2][:, :, ::-1], _blk(b1[:], 128))
    else:
        nc.gpsimd.memset(b2a3[:, :, X:n2 - 128], PADK)
        xq = pool.tile([P, NBLK * X], dt.uint16, tag="xq")
        EE.tensor_scalar(xq[:], xt[:], QS, (1.0 - VOFF) * QS + 0.5,
                         Alu.mult, Alu.add)
        EE.tensor_scalar(b2a3[:, :, 0:X], _blk(xq[:], X), 4, None, Alu.mult)
        EE.tensor_copy(b2a3[:, :, n2 - 128:n2][:, :, ::-1], _blk(b1[:], 128))
    yield
    out["m"] = _merge(ME2, b2a, b2b, n2, ew=L["EW"],
                      trim4d=not fkeys)
    yield


def _emit_half(nc, pool, lvl, b0, s_sh, radio_full, mout, consts, acc, eng):
    """Generator: post-merge pipeline for blocks [b0, b0+NB) of one level."""
    SE, XE, EE, FE = (eng[k] for k in ("SE", "XE", "EE", "FE"))
    TE = eng.get("TE", EE)
    SE2 = eng.get("SE2", SE)
    fkeys = eng.get("fkeys", False)
    mdt = dt.uint16 if not fkeys else dt.float32
    AE = nc.scalar
    L = LVL[lvl]
    X, n2, EW, QWS = L["X"], L["n2"], L["EW"], L["QWS"]
    NL = NB * EW
    NQ = NB * QWS
    NE = NB * 49
    VW = NQ + 2 * NE          # vcat width: [x | em | ep]
    pw = PULSE[lvl]
    maskf, mask_cnt, io49p, ioG, ioQ2 = consts

    def blkE(ap):
        return ap.rearrange("p (b n) -> p b n", b=NB)

    ss = s_sh[:][:, b0 * 49:(b0 + NB) * 49]

    # ---------- sources: exact values + radio (independent of merge) ----------
    vcat = pool.tile([P, VW], dt.float32, tag="vcat")
    nc.gpsimd.memset(_blk(vcat[:, 0:NQ], QWS)[:, :, X:QWS], 0.0)
    radcat = pool.tile([P, 2 * NE], dt.float32, tag="radcat")
    rsl_ = radio_full[:][:, b0 * 49:(b0 + NB) * 49]
    FE.tensor_copy(radcat[:, 0:NE], rsl_)
    FE.tensor_scalar(radcat[:, NE:2 * NE], radcat[:, 0:NE], -1.0, None, Alu.mult)
    yield
    # wait for merge result
    while "m" not in mout:
        yield
    m = mout["m"]
    xt, pwt_full = mout["xt"], mout["pwt"]
    mSh = _blk(m[:], n2)[:, b0:b0 + NB, 0:EW]       # [P, NB, EW] strided
    xts = _blk(xt[:], X)[:, b0:b0 + NB]             # [P, NB, X]
    pwt = _blk(pwt_full[:], X - 1)[:, b0:b0 + NB]
    AE.activation(_blk(vcat[:, 0:NQ], QWS)[:, :, 0:X], xts, Act.Copy)
    AE.activation(_blk(vcat[:, NQ:NQ + NE], 49), _blk(ss, 49), Act.Copy, bias=-pw)
    AE.activation(_blk(vcat[:, NQ + NE:VW], 49), _blk(ss, 49), Act.Copy, bias=pw)
    yield

    # ---------- tags + counts ----------
    if fkeys:
        tag32 = pool.tile([P, NL], dt.int32, tag="tag32")
        _ts_int(XE, blkE(tag32[:]), mSh.bitcast(dt.int32), 3, Alu.bitwise_and)
        ev_f = pool.tile([P, NL], dt.float32, tag="ev_f")
        em_f = pool.tile([P, NL], dt.float32, tag="em_f")
        ep_f = pool.tile([P, NL], dt.float32, tag="ep_f")
        _ts_int(TE, em_f[:], tag32[:], 1, Alu.is_equal)
        _ts_int(TE, ep_f[:], tag32[:], 3, Alu.is_equal)
        FE.tensor_tensor(ev_f[:], em_f[:], ep_f[:], Alu.add)
    else:
        tagb_t = pool.tile([P, NL], dt.uint16, tag="tagb")
        tagb = tagb_t[:]
        XE.tensor_scalar(blkE(tagb), mSh, 3, None, Alu.bitwise_and)
        ev_f = pool.tile([P, NL], dt.uint16, tag="ev_f")
        TE.tensor_scalar(ev_f[:], tagb, 1, None, Alu.bitwise_and)
        em_f = pool.tile([P, NL], dt.uint16, tag="em_f")
        TE.tensor_scalar(em_f[:], tagb, 1, None, Alu.is_equal)
        ep_f = pool.tile([P, NL], dt.uint16, tag="ep_f")
        TE.tensor_scalar(ep_f[:], tagb, 3, None, Alu.is_equal)
    yield
    C = pool.tile([P, NL], mdt, tag="C")
    SE.tensor_tensor_scan(C[:], mask_cnt, ev_f[:], 0.0, Alu.mult, Alu.add)
    Cm = pool.tile([P, NL], mdt, tag="Cm")
    SE.tensor_tensor_scan(Cm[:], mask_cnt, em_f[:], 0.0, Alu.mult, Alu.add)
    yield

    # ---------- idxcat: merged position of every source element ----------
    t2 = pool.tile([P, NL], mdt, tag="t2")
    t3 = pool.tile([P, NL], mdt, tag="t3")
    if fkeys:
        t1 = tag32                                  # dead after masks
    else:
        t1 = tagb_t                                 # dead after masks
    # block offsets (49b/98b) ride in from the mask_cnt scan carry; section
    # offsets NQ / NQ+NE are flat immediates. One combined scatter:
    # t1 = (Cm'+NQ)*em + (C'-Cm'+NQ+NE)*ep + (ioQ2-C')*qf - 1
    EE.tensor_tensor(t2[:], C[:], Cm[:], Alu.subtract)
    EE.tensor_scalar(t2[:], t2[:], NQ + NE, None, Alu.add)
    EE.tensor_tensor(t2[:], t2[:], ep_f[:], Alu.mult)
    t1v = t1[:].bitcast(dt.float32) if fkeys else t1[:]
    EE.tensor_scalar(t1v, Cm[:], NQ, None, Alu.add)
    EE.tensor_tensor(t1v, t1v, em_f[:], Alu.mult)
    EE.tensor_tensor(t1v, t1v, t2[:], Alu.add)
    qf = em_f                                       # em_f dead after t1
    TE2 = FE if fkeys else TE
    TE2.tensor_scalar(qf[:], ev_f[:], 0, None, Alu.is_equal)
    EE.tensor_tensor(t3[:], ioQ2, C[:], Alu.subtract)
    EE.tensor_tensor(t3[:], t3[:], qf[:], Alu.mult)
    EE.tensor_tensor(t1v, t1v, t3[:], Alu.add)
    EE.tensor_scalar(t1v, t1v, 1, None, Alu.subtract)       # idx all
    EE.tensor_scalar(t2[:], t3[:], 1, None, Alu.subtract)       # idxq
    if fkeys:
        t1s = ev_f[:].bitcast(dt.int16)[:, 0:NL]    # ev_f dead after qf
        AE.activation(t1s, t1v, Act.Copy)
        t2s = ep_f[:].bitcast(dt.int16)[:, 0:NL]    # ep_f dead after t2
        AE.activation(t2s, t2[:], Act.Copy)
    else:
        t1s = t1[:].bitcast(dt.int16)
        t2s = t2[:].bitcast(dt.int16)
    yield
    idxcat = pool.tile([P, VW], dt.uint16, tag="idxcat")
    nc.gpsimd.local_scatter(idxcat[:].bitcast(dt.int16), ioG,
                            t1s, channels=P,
                            num_elems=VW, num_idxs=NL)
    idx2 = pool.tile([P, 2 * VW], dt.uint16, tag="idx2")
    i2v = idx2[:].rearrange("p (n two) -> p n two", two=2)
    AE.activation(i2v[:, :, 0], idxcat[:], Act.Copy)
    AE.activation(i2v[:, :, 1], idxcat[:], Act.Copy, bias=1.0)
    yield

    # ---------- pair-scatter exact values + radio into merged domain ----------
    v = pool.tile([P, NL], dt.float32, tag="v")
    nc.gpsimd.local_scatter(v[:].bitcast(dt.int16),
                            vcat[:].bitcast(dt.int16),
                            idx2[:].bitcast(dt.int16), channels=P,
                            num_elems=2 * NL, num_idxs=2 * VW)
    F1 = pool.tile([P, NL], dt.float32, tag="F1")   # radio_m
    nc.gpsimd.local_scatter(F1[:].bitcast(dt.int16),
                            radcat[:].bitcast(dt.int16),
                            idx2[:, 2 * NQ:2 * VW].bitcast(dt.int16), channels=P,
                            num_elems=2 * NL, num_idxs=4 * NE)
    yield

    # ---------- density reconstruction ----------
    F2 = pool.tile([P, NL], dt.float32, tag="F2")
    SE2.tensor_tensor_scan(F2[:], maskf, F1[:], 0.0, Alu.mult, Alu.add)  # g
    if fkeys:
        dv = t3                                     # t3 dead after idx phase
    else:
        dv = pool.tile([P, NL], dt.float32, tag="dv")
    dv3 = blkE(dv[:])
    v3 = blkE(v[:])
    nc.gpsimd.memset(dv3[:, :, 0:1], 0.0)
    FE.tensor_tensor(dv3[:, :, 1:EW], v3[:, :, 1:EW], v3[:, :, 0:EW - 1],
                     Alu.subtract)
    yield
    wg = v                                          # v dead after dv
    wg3 = blkE(wg[:])
    nc.gpsimd.memset(wg3[:, :, 0:1], 0.0)
    FE.tensor_tensor(wg3[:, :, 1:EW], dv3[:, :, 1:EW],
                     blkE(F2[:])[:, :, 0:EW - 1], Alu.mult)
    w_t = F1                                        # radio dead after g
    SE2.tensor_tensor_scan(w_t[:], maskf, wg[:], 0.0, Alu.mult, Alu.add)
    yield
    wc = wg                                         # wg dead
    AE.activation(wc[:], w_t[:], Act.Relu, scale=0.5)
    scr = pool.tile([P, NL], dt.float32, tag="scr")
    wc3 = blkE(wc[:])
    s3_ = blkE(scr[:])
    nc.gpsimd.memset(s3_[:, :, 0:1], 0.0)
    FE.tensor_tensor(s3_[:, :, 1:EW], wc3[:, :, 1:EW], wc3[:, :, 0:EW - 1],
                     Alu.add)
    area = w_t                                      # w dead after wc
    a3 = blkE(area[:])
    nc.gpsimd.memset(a3[:, :, 0:1], 0.0)
    FE.tensor_tensor(a3[:, :, 1:EW], s3_[:, :, 1:EW],
                     dv3[:, :, 1:EW], Alu.mult)
    cdf = F2                                        # g dead after wg
    SE2.tensor_tensor_scan(cdf[:], maskf, area[:], 0.0, Alu.mult, Alu.add)
    yield

    # ---------- compact cdf at query slots (pair-scatter) ----------
    idx2q = idx2                                    # idx2 dead after scatters
    i2qv = idx2q[:][:, 0:2 * NL].rearrange("p (n two) -> p n two", two=2)
    tqu = Cm[:].bitcast(dt.uint16)[:, 0:NL]         # Cm dead after t1
    if fkeys:
        AE.activation(tqu.bitcast(dt.int16), t2[:], Act.Copy, scale=2.0)
    else:
        EE.tensor_scalar(tqu, t2[:], 2, None, Alu.mult)
    AE.activation(i2qv[:, :, 0], tqu, Act.Copy)
    AE.activation(i2qv[:, :, 1], tqu, Act.Copy, bias=1.0)
    cdfq = vcat                                     # vcat dead after v scatter
    cqn = cdfq[:][:, 0:NQ]
    nc.gpsimd.local_scatter(cqn.bitcast(dt.int16),
                            cdf[:].bitcast(dt.int16),
                            idx2q[:][:, 0:2 * NL].bitcast(dt.int16), channels=P,
                            num_elems=2 * NQ, num_idxs=2 * NL)
    del t2s
    yield

    # ---------- loss tail ----------
    NW = NB * (X - 1)
    ws = scr                                        # dead after area
    ws2 = ws[:][:, 0:NW]
    cqf = _blk(cdfq[:][:, 0:NQ], QWS)
    FE.tensor_tensor(_blk(ws2, X - 1), cqf[:, :, 1:X], cqf[:, :, 0:X - 1],
                     Alu.subtract)
    FE.tensor_tensor(_blk(ws2, X - 1), _blk(ws2, X - 1), pwt, Alu.subtract)
    den = area                                      # dead after cdf
    den2 = den[:][:, 0:NW]
    AE.activation(_blk(den2, X - 1), pwt, Act.Copy, bias=1e-5)
    nc.vector.reciprocal(den2, den2)
    rsl = dv                                        # dead after area
    AE.activation(rsl[:][:, 0:NW], ws2, Act.Relu)
    FE.tensor_tensor(ws2, ws2, rsl[:][:, 0:NW], Alu.mult)
    FE.tensor_tensor(ws2, ws2, den2, Alu.mult)
    nc.vector.tensor_reduce(acc[:], _blk(ws2, X - 1), AX.XY, Alu.add)
    yield


def _emit_setup(nc, pool, s_sh, radios, accs, mask48, aps):
    V, G = nc.vector, nc.gpsimd
    rw_sh = pool.tile([P, NBLK * 48], dt.float32, tag="rw_sh")
    nc.sync.dma_start(_blk(rw_sh[:], 48),
                      aps["rw"].rearrange("(b p) x -> p b x", p=P))
    s3 = _blk(s_sh[:], 49)
    ds = pool.tile([P, NBLK * 48], dt.float32, tag="ds")
    V.tensor_tensor(_blk(ds[:], 48), s3[:, :, 1:49], s3[:, :, 0:48], Alu.subtract)
    dse = pool.tile([P, NBLK * 48], dt.float32, tag="dse")
    nc.scalar.activation(dse[:], ds[:], Act.Copy, bias=1e-8)
    V.reciprocal(dse[:], dse[:])
    wnorm = pool.tile([P, NBLK * 48], dt.float32, tag="wnorm")
    V.tensor_tensor(wnorm[:], rw_sh[:], dse[:], Alu.mult)
    wnp = pool.tile([P, NBLK * 50], dt.float32, tag="wnp")
    G.memset(wnp[:], 0.0)
    V.tensor_copy(_blk(wnp[:], 50)[:, :, 1:49], _blk(wnorm[:], 48))
    diff = pool.tile([P, NBLK * 49], dt.float32, tag="diff")
    wnp3 = _blk(wnp[:], 50)
    V.tensor_tensor(_blk(diff[:], 49), wnp3[:, :, 1:50], wnp3[:, :, 0:49],
                    Alu.subtract)
    for lvl in (0, 1):
        V.tensor_scalar(radios[lvl][:], diff[:], 1.0 / (2 * PULSE[lvl]), None,
                        Alu.mult)
    yield

    mid = pool.tile([P, NBLK * 48], dt.float32, tag="mid")
    V.tensor_tensor(_blk(mid[:], 48), s3[:, :, 1:49], s3[:, :, 0:48], Alu.add)
    wm = pool.tile([P, NBLK * 48], dt.float32, tag="wm")
    V.scalar_tensor_tensor(wm[:], mid[:], 0.5, rw_sh[:], Alu.mult, Alu.mult)
    Cin = pool.tile([P, NBLK * 48], dt.float32, tag="Cin")
    V.tensor_tensor_scan(Cin[:], mask48, rw_sh[:], 0.0, Alu.mult, Alu.add)
    Sin = pool.tile([P, NBLK * 48], dt.float32, tag="Sin")
    V.tensor_tensor_scan(Sin[:], mask48, wm[:], 0.0, Alu.mult, Alu.add)
    yield
    A = pool.tile([P, NBLK * 47], dt.float32, tag="A47")
    m3 = _blk(mid[:], 48)
    c3 = _blk(Cin[:], 48)
    sw3 = _blk(Sin[:], 48)
    rw3 = _blk(rw_sh[:], 48)
    A3 = _blk(A[:], 47)
    V.scalar_tensor_tensor(A3, m3[:, :, 1:48], 0.5, c3[:, :, 0:47],
                           Alu.mult, Alu.mult)
    V.tensor_tensor(A3, A3, sw3[:, :, 0:47], Alu.subtract)
    V.tensor_tensor(A3, A3, rw3[:, :, 1:48], Alu.mult)
    V.tensor_reduce(accs["p1"][:], A3, AX.XY, Alu.add)
    t2 = pool.tile([P, NBLK * 48], dt.float32, tag="t2d")
    G.tensor_tensor(t2[:], rw_sh[:], rw_sh[:], Alu.mult)
    G.tensor_tensor(t2[:], t2[:], ds[:], Alu.mult)
    V.tensor_reduce(accs["p2"][:], _blk(t2[:], 48), AX.XY, Alu.add)
    yield

    pdt = pool.tile([P, NBLK * 3], dt.float32, tag="pdt")
    gtt = pool.tile([P, NBLK * 3], dt.float32, tag="gtt")
    nc.sync.dma_start(_blk(pdt[:], 3), aps["pd"].rearrange("(b p) c -> p b c", p=P))
    nc.sync.dma_start(_blk(gtt[:], 3), aps["gt"].rearrange("(b p) c -> p b c", p=P))
    d = pool.tile([P, NBLK * 3], dt.float32, tag="rgbd")
    V.tensor_tensor(d[:], pdt[:], gtt[:], Alu.subtract)
    V.tensor_tensor(d[:], d[:], d[:], Alu.mult)
    V.tensor_reduce(accs["rgb"][:], d[:], AX.X, Alu.add)
    yield


def _emit_hash(nc, pool, lvl, ones_h, acc, aps, first):
    E = nc.gpsimd if lvl == 0 else nc.vector
    idx = pool.tile([P, HCOLS], dt.int32, tag="hidx")
    src = aps[f"hi{lvl}"]
    nc.sync.dma_start(idx[:], bass.AP(tensor=src.tensor, offset=src.offset,
                                      ap=[[HROW, P], [1, HCOLS]]))
    emb = pool.tile([P, HCOLS * 2], dt.float32, tag="hemb")
    esrc = aps[f"he{lvl}"]
    nc.sync.dma_start(emb[:], bass.AP(tensor=esrc.tensor, offset=esrc.offset,
                                      ap=[[HROW * 2, P], [1, HCOLS * 2]]))
    sq = pool.tile([P, HCOLS * 2], dt.float32, tag="hsq")
    E.tensor_tensor(sq[:], emb[:], emb[:], Alu.mult)
    wv = pool.tile([P, HCOLS], dt.float32, tag="hw")
    sq3 = sq[:].rearrange("p (n two) -> p n two", two=2)
    E.tensor_tensor(wv[:], sq3[:, :, 0], sq3[:, :, 1], Alu.add)
    eq = pool.tile([P, HCOLS], dt.float32, tag="heq")
    nc.gpsimd.memset(eq[:, 0:1], 0.0)
    nc.vector.tensor_tensor(eq[:, 1:HCOLS], idx[:, 1:HCOLS], idx[:, 0:HCOLS - 1],
                             Alu.is_equal)
    yield
    S = pool.tile([P, HCOLS], dt.float32, tag="hS")
    nc.vector.tensor_tensor_scan(S[:], eq[:], wv[:], 0.0, Alu.mult, Alu.add)
    cc = pool.tile([P, HCOLS], dt.float32, tag="hcc")
    nc.vector.tensor_tensor_scan(cc[:], eq[:], ones_h, 0.0, Alu.mult, Alu.add)
    yield
    ratio = pool.tile([P, HCOLS], dt.float32, tag="hr")
    nc.vector.reciprocal(cc[:], cc[:])
    E.tensor_tensor(ratio[:], S[:], cc[:], Alu.mult)
    me = pool.tile([P, HCOLS], dt.float32, tag="hme")
    nc.scalar.activation(me[:, 0:HCOLS - 1], eq[:, 1:HCOLS], Act.Copy,
                         bias=1.0, scale=-1.0)
    E.tensor_tensor(ratio[:, HALO:HALO + HROW], ratio[:, HALO:HALO + HROW],
                    me[:, HALO:HALO + HROW], Alu.mult)
    if first:
        nc.vector.tensor_reduce(acc[:], ratio[:, HALO:HALO + HROW], AX.X, Alu.add)
    else:
        part = pool.tile([P, 1], dt.float32, tag="hpart")
        nc.vector.tensor_reduce(part[:], ratio[:, HALO:HALO + HROW], AX.X,
                                Alu.add)
        E.tensor_tensor(acc[:], acc[:], part[:], Alu.add)
    yield


def build_module(parts=("rgb", "dist", "hash", "l0", "l1")):
    nc = bacc.Bacc("TRN2", target_bir_lowering=False, debug=False,
                   enable_asserts=False, num_devices=N_CORES)
    aps = {}

    def din(name, shape, dtype=dt.float32):
        aps[name] = nc.dram_tensor(name, shape, dtype, kind="ExternalInput").ap()
    din("pd", [RPC, 3]); din("gt", [RPC, 3])
    din("sd", [RPC, 49]); din("rw", [RPC, 48])
    din("ps0", [RPC, 257]); din("pw0", [RPC, 256])
    din("ps1", [RPC, 97]); din("pw1", [RPC, 96])
    din("hi0", [HSLICE], dt.int32); din("he0", [HSLICE * 2])
    din("hi1", [HSLICE], dt.int32); din("he1", [HSLICE * 2])
    for lvl, L in LVL.items():
        NL = NB * L["EW"]
        din(f"c_u16_l{lvl}", [P, 2 * NL], dt.int16)
        din(f"c_maskf_l{lvl}", [P, (3 if lvl == 1 else 2) * NL])
    din("c_mask48", [P, NBLK * 48]); din("c_ones", [P, HCOLS])
    out_ap = nc.dram_tensor("out", [1, 1], dt.float32, kind="ExternalOutput").ap()

    with tile.TileContext(nc) as tc:
        _emit(nc, tc, aps, out_ap, parts)
    nc.compile()
    return nc


def _emit(nc, tc, aps, out_ap, parts=("rgb", "dist", "hash", "l0", "l1")):
    import contextlib
    V, G = nc.vector, nc.gpsimd
    with contextlib.ExitStack() as ctx:
        spool = ctx.enter_context(tc.tile_pool(name="shared", bufs=1))
        s_sh = spool.tile([P, NBLK * 49], dt.float32, tag="s_sh")
        nc.sync.dma_start(_blk(s_sh[:], 49),
                          aps["sd"].rearrange("(b p) x -> p b x", p=P))
        radios = {l: spool.tile([P, NBLK * 49], dt.float32, tag=f"radio{l}",
                                name=f"radio{l}")
                  for l in (0, 1)}

        cpool = ctx.enter_context(tc.tile_pool(name="consts", bufs=1))
        mask48 = cpool.tile([P, NBLK * 48], dt.float32, tag="mask48")
        ones_h = cpool.tile([P, HCOLS], dt.float32, tag="ones_h")
        lvl_consts = {}
        cdma = []
        cdma.append((mask48[:], aps["c_mask48"]))
        cdma.append((ones_h[:], aps["c_ones"]))
        for lvl, L in LVL.items():
            NL = NB * L["EW"]
            cu = cpool.tile([P, 2 * NL], dt.int16, tag=f"cu16_{lvl}",
                            name=f"cu16_{lvl}")
            mf = cpool.tile([P, (3 if lvl == 1 else 2) * NL], dt.float32,
                            tag=f"maskf_{lvl}", name=f"maskf_{lvl}")
            cdma.append((cu[:], aps[f"c_u16_l{lvl}"]))
            cdma.append((mf[:], aps[f"c_maskf_l{lvl}"]))
            cuv = cu[:].bitcast(dt.uint16)
            ioq2 = (mf[:][:, 2 * NL:3 * NL] if lvl == 1
                    else cuv[:, NL:2 * NL])
            lvl_consts[lvl] = (mf[:][:, 0:NL], mf[:][:, NL:2 * NL],
                               None, cu[:][:, 0:NL], ioq2)
            # (maskf, mask_cnt(f32), unused, ioG(i16), ioQ2)

        def _emit_consts():
            for dst, src_ap in cdma:
                nc.sync.dma_start(dst, src_ap)
            yield

        accs = {}
        for name in ("rgb", "p1", "p2", "hash", "l0a", "l0b", "l1a", "l1b"):
            accs[name] = cpool.tile([P, 1], dt.float32, tag=f"acc_{name}",
                                    name=f"acc_{name}")
            V.memset(accs[name][:], 0.0)

        spool = ctx.enter_context(tc.tile_pool(name="shared", bufs=1))
        s_sh = spool.tile([P, NBLK * 49], dt.float32, tag="s_sh")
        nc.sync.dma_start(_blk(s_sh[:], 49),
                          aps["sd"].rearrange("(b p) x -> p b x", p=P))
        radios = {l: spool.tile([P, NBLK * 49], dt.float32, tag=f"radio{l}",
                                name=f"radio{l}")
                  for l in (0, 1)}

        MRG = {0: dict(ME=V, ME2=V, EE=V),
               1: dict(ME=V, ME2=V, EE=V)}
        HEM = {
            "l0a": dict(SE=V, SE2=V, XE=V, EE=V, TE=V, FE=G),
            "l0b": dict(SE=V, SE2=V, XE=V, EE=V, TE=V, FE=G),
            "l1a": dict(SE=V, SE2=V, XE=V, EE=V, TE=V, FE=G),
            "l1b": dict(SE=V, SE2=V, XE=V, EE=V, TE=V, FE=G),
        }

        gens = []
        setup_pool = ctx.enter_context(tc.tile_pool(name="setup", bufs=1))
        gens.append(_emit_setup(nc, setup_pool, s_sh, radios, accs, mask48[:],
                                aps))
        mouts = {}
        for lvl in (0, 1):
            if f"l{lvl}" not in parts:
                continue
            mouts[lvl] = {}
            mp = ctx.enter_context(tc.tile_pool(name=f"mrg{lvl}", bufs=1))
            gens.append(_emit_level_merge(nc, tc, mp, lvl, s_sh,
                                          aps[f"ps{lvl}"], aps[f"pw{lvl}"],
                                          mouts[lvl], MRG[lvl]))
        gens.append(_emit_consts())
        if "hash" in parts:
            for lvl in (0, 1):
                hp2 = ctx.enter_context(tc.tile_pool(name=f"hash{lvl}", bufs=1))
                gens.append(_emit_hash(nc, hp2, lvl, ones_h[:], accs["hash"],
                                       aps, first=(lvl == 0)))
        for name, lvl, b0 in HALVES:
            if f"l{lvl}" not in parts:
                continue
            hp = ctx.enter_context(tc.tile_pool(name=name, bufs=1))
            gens.append(_emit_half(nc, hp, lvl, b0, s_sh, radios[lvl],
                                   mouts[lvl], lvl_consts[lvl], accs[name],
                                   HEM[name]))

        while gens:
            nxt = []
            for g in gens:
                try:
                    next(g)
                    nxt.append(g)
                except StopIteration:
                    pass
            gens = nxt

        with tc.tile_pool(name="fin", bufs=1) as pool:
            tot = pool.tile([P, 1], dt.float32, tag="tot")
            V.tensor_scalar(tot[:], accs["rgb"][:], W_RGB / (R * 3), None,
                            Alu.mult)
            for snm, lvl, _ in HALVES:
                V.scalar_tensor_tensor(tot[:], accs[snm][:],
                                       W_INTER / (R * (LVL[lvl]["X"] - 1)),
                                       tot[:], Alu.mult, Alu.add)
            V.scalar_tensor_tensor(tot[:], accs["p1"][:], W_DIST * 2.0 / R,
                                   tot[:], Alu.mult, Alu.add)
            V.scalar_tensor_tensor(tot[:], accs["p2"][:], W_DIST / (3.0 * R),
                                   tot[:], Alu.mult, Alu.add)
            V.scalar_tensor_tensor(tot[:], accs["hash"][:],
                                   W_HASH / (NUM_SEGMENTS * 2.0), tot[:],
                                   Alu.mult, Alu.add)
            res = pool.tile([1, 1], dt.float32, tag="res")
            G.tensor_reduce(res[:], tot[:], AX.C, Alu.add)
            nc.sync.dma_start(out_ap, res[:])


# ---------------- host side ----------------
_module_cache = {}


def _get_module():
    if "nc" not in _module_cache:
        _module_cache["nc"] = build_module()
    return _module_cache["nc"]


def shard_inputs(inputs):
    f32 = np.float32
    pd = np.ascontiguousarray(inputs["pd_rgbs"], f32)
    gt = np.ascontiguousarray(inputs["gt_rgbs"], f32)
    sd = np.ascontiguousarray(inputs["render_sdist"], f32)
    rw = np.ascontiguousarray(inputs["render_weights"], f32)
    ps0 = np.ascontiguousarray(inputs["prop_sdist_0"], f32)
    pw0 = np.ascontiguousarray(inputs["prop_weights_0"], f32)
    ps1 = np.ascontiguousarray(inputs["prop_sdist_1"], f32)
    pw1 = np.ascontiguousarray(inputs["prop_weights_1"], f32)
    hashes = {}
    for lvl in (0, 1):
        idx = np.asarray(inputs[f"enc_idx_{lvl}"]).astype(np.int32)
        emb = np.ascontiguousarray(inputs[f"enc_embds_{lvl}"], f32)
        idx_pad = np.full(M + 2 * HALO, -1, np.int32)
        idx_pad[HALO:HALO + M] = idx
        emb_pad = np.zeros((M + 2 * HALO, 2), f32)
        emb_pad[HALO:HALO + M] = emb
        hashes[lvl] = (idx_pad, emb_pad)

    consts = {}
    rep = lambda row: np.ascontiguousarray(np.tile(row, (P, 1)))
    for lvl, L in LVL.items():
        EW, QWS, X = L["EW"], L["QWS"], L["X"]
        NL = NB * EW
        NQ = NB * QWS
        ioG = np.concatenate([2 * np.arange(b * EW, (b + 1) * EW,
                                            dtype=np.uint16)
                              for b in range(NB)])
        # query dest: rank-1 + b*QWS; C' = C + 98b so fold +98b here:
        # ioQ2 = i+1 + b*QWS + 98b -> (ioQ2 - C')*qf - 1 = rank-1 + b*QWS
        ioQ2 = np.concatenate([np.arange(1, EW + 1, dtype=np.uint16)
                               + b * QWS + 98 * b for b in range(NB)])
        packed = np.concatenate([ioG, ioQ2]).astype(np.uint16)
        consts[f"c_u16_l{lvl}"] = rep(packed.view(np.int16))
        msk = np.ones(NL, f32)
        msk[::EW] = 0.0
        mcnt = np.ones(NL, f32)
        for b in range(NB):
            mcnt[b * EW] = b
        parts_ = [msk, mcnt]
        if lvl == 1:
            parts_.append(np.concatenate([np.arange(1, EW + 1, dtype=f32)
                                          + b * QWS + 98 * b
                                          for b in range(NB)]))
        consts[f"c_maskf_l{lvl}"] = rep(np.concatenate(parts_))
    m48 = np.ones(NBLK * 48, f32)
    m48[::48] = 0.0
    consts["c_mask48"] = rep(m48)
    consts["c_ones"] = rep(np.ones(HCOLS, f32))

    in_maps = []
    for c in range(N_CORES):
        r0 = c * RPC
        lo = c * MPC
        im = {
            "pd": pd[r0:r0 + RPC], "gt": gt[r0:r0 + RPC],
            "sd": sd[r0:r0 + RPC], "rw": rw[r0:r0 + RPC],
            "ps0": ps0[r0:r0 + RPC], "pw0": pw0[r0:r0 + RPC],
            "ps1": ps1[r0:r0 + RPC], "pw1": pw1[r0:r0 + RPC],
        }
        for lvl in (0, 1):
            idx_pad, emb_pad = hashes[lvl]
            im[f"hi{lvl}"] = np.ascontiguousarray(idx_pad[lo:lo + HSLICE])
            im[f"he{lvl}"] = np.ascontiguousarray(
                emb_pad[lo:lo + HSLICE].reshape(-1))
        im.update(consts)
        in_maps.append(im)
    return in_maps


def kernel(**inputs) -> np.ndarray:
    nc = _get_module()
    in_maps = shard_inputs(inputs)
    res = run_bass_kernel_spmd(nc, in_maps, core_ids=list(range(N_CORES)))
    total = np.float64(0.0)
    for r in res.results:
        total += np.float64(r["out"][0, 0])
    return np.float32(total)


# revision 6
# speedup vs baseline: 1.6138x; 1.0614x over previous
"""Trainium2 Bass kernel v2 for nn_Loss_dict_50646254354805 (NeRF-style loss).

v2 vs baseline:
- bitonic merges on uint16 quantized keys (value*15000 + 2 tag bits) -> DVE
  2x perf mode; keys determine ORDER only.
- exact f32 values (queries/em/ep) and radio reach the merged domain via
  batched u16-half local_scatters through one shared index table (idxcat):
  merged positions come from the C/Cm count scans.
- one merge per level; post-merge work split into two 2-block half-streams
  with per-stream engine maps; all generators emitted stage-interleaved so
  DVE / Pool / Act overlap.
"""
import numpy as np

import concourse.bass as bass
import concourse.mybir as mybir
import concourse.tile as tile
from concourse import bacc
from concourse.bass_utils import run_bass_kernel_spmd

dt = mybir.dt
Alu = mybir.AluOpType
AX = mybir.AxisListType
Act = mybir.ActivationFunctionType
P = 128

PULSE = (0.01, 0.005)
W_RGB, W_INTER, W_DIST, W_HASH = 1.0, 1.0, 0.01, 0.1
NUM_SEGMENTS = 65536
R, N = 4096, 48
M = R * N
N_CORES = 8
RPC = R // N_CORES
NBLK = RPC // P               # 4 ray blocks per core
MPC = M // N_CORES
HALO = 64
HROW = MPC // P
HCOLS = HROW + HALO + 1
HSLICE = HALO + MPC + HALO

VOFF = 0.97
QS = 15000.0                  # key quantization scale
PADK = 0xFFFC

LVL = {0: dict(X=257, n2=512), 1: dict(X=97, n2=256)}
for _L in LVL.values():
    _L["EW"] = ((_L["X"] + 98 + 1 + 7) // 8) * 8        # 360 / 200
    _L["QWS"] = _L["EW"] - 98                           # 262 / 102

NB = 2                        # blocks per half-stream
HALVES = [("l0a", 0, 0), ("l0b", 0, 2), ("l1a", 1, 0), ("l1b", 1, 2)]


def _blk(ap, n):
    return ap.rearrange("p (b n) -> p b n", n=n)


def _ts_int(eng, out, in0, imm1, op0, imm2=None, op1=None):
    ins_ = [eng.lower_ap(in0), mybir.ImmediateValue(dtype=dt.int32, value=int(imm1))]
    kw = dict(op0=op0)
    if imm2 is not None:
        ins_.append(mybir.ImmediateValue(dtype=dt.int32, value=int(imm2)))
        kw["op1"] = op1
    return eng.add_instruction(mybir.InstTensorScalarPtr(
        name=eng.bass.get_next_instruction_name(),
        ins=ins_, outs=[eng.lower_ap(out)], **kw))

BIGF = 3.0


def _merge_gen(eng, bufa, bufb, width, out, ew=None, trim4d=True):
    """Ascending bitonic merge over [P, NBLK*width] u16 ping-pong tiles.
    Generator: yields after each stage so two levels' merges interleave in
    the engine queue. Result tile is appended to `out`.

    If ew is given, only outputs [0, ew+2d-1] of each block are needed
    downstream, so late stages skip whole 2d-chunks beyond that window."""
    cur, nxt = bufa, bufb
    d = width // 2
    while d >= 1:
        nch = width // (2 * d)
        keep = nch
        if ew is not None and trim4d:
            keep = min(nch, -(-(ew + 2 * d - 1) // (2 * d)))
        if keep == nch:
            c3 = cur[:].rearrange("p (c td) -> p c td", td=2 * d)
            n3 = nxt[:].rearrange("p (c td) -> p c td", td=2 * d)
        else:
            c3 = cur[:].rearrange("p (b c td) -> p (b c) td",
                                  td=2 * d, c=nch)[: , 0:0]  # placeholder
        if keep == nch:
            lo_in, hi_in = c3[:, :, 0:d], c3[:, :, d:2 * d]
            eng.tensor_tensor(n3[:, :, 0:d], lo_in, hi_in, Alu.min)
            eng.tensor_tensor(n3[:, :, d:2 * d], lo_in, hi_in, Alu.max)
        else:
            c4 = cur[:].rearrange("p (b c td) -> p b c td", td=2 * d, c=nch)
            n4 = nxt[:].rearrange("p (b c td) -> p b c td", td=2 * d, c=nch)
            lo_in = c4[:, :, 0:keep, 0:d]
            hi_in = c4[:, :, 0:keep, d:2 * d]
            eng.tensor_tensor(n4[:, :, 0:keep, 0:d], lo_in, hi_in, Alu.min)
            eng.tensor_tensor(n4[:, :, 0:keep, d:2 * d], lo_in, hi_in, Alu.max)
        cur, nxt = nxt, cur
        d //= 2
        if d >= 1:
            yield
    out.append(cur)


def _emit_level_merge(nc, tc, pool, lvl, s_sh, x_ap, pwt_ap, out, eng):
    """Generator: quantize + b1/b2 merges for all 4 blocks of one level.

    lvl 0: uint16 quantized keys, merged on DVE (2x mode).
    lvl 1: f32-bitcast tagged keys (baseline-style), merged on Pool where
    f32 min/max is legal -- frees DVE during the big level-0 merge."""
    ME, ME2, EE = eng["ME"], eng["ME2"], eng["EE"]
    AE = nc.scalar
    fkeys = eng.get("fkeys", False)
    L = LVL[lvl]
    X, n2 = L["X"], L["n2"]
    pw = PULSE[lvl]
    kdt = dt.float32 if fkeys else dt.uint16

    xt = pool.tile([P, NBLK * X], dt.float32, tag="xt")
    nc.sync.dma_start(_blk(xt[:], X), x_ap.rearrange("(b p) x -> p b x", p=P))
    pwt = pool.tile([P, NBLK * (X - 1)], dt.float32, tag="pwt")
    nc.sync.dma_start(_blk(pwt[:], X - 1),
                      pwt_ap.rearrange("(b p) x -> p b x", p=P))
    out["xt"] = xt
    out["pwt"] = pwt

    b2a = pool.tile([P, NBLK * n2], kdt, tag="b2a")
    b2b = pool.tile([P, NBLK * n2], kdt, tag="b2b")
    b2a3 = _blk(b2a[:], n2)
    b1a = pool.tile([P, NBLK * 128], kdt, tag="b1a")
    b1b = pool.tile([P, NBLK * 128], kdt, tag="b1b")
    b1a3 = _blk(b1a[:], 128)
    if fkeys:
        nc.gpsimd.memset(b1a[:], BIGF)
        emsh = pool.tile([P, NBLK * 49], dt.float32, tag="emsh")
        AE.activation(emsh[:], s_sh[:], Act.Copy, bias=1.0 - pw)
        epsh = pool.tile([P, NBLK * 49], dt.float32, tag="epsh")
        AE.activation(epsh[:], s_sh[:], Act.Copy, bias=1.0 + pw)
        _ts_int(EE, b1a3[:, :, 0:49].bitcast(dt.int32),
                _blk(emsh[:], 49).bitcast(dt.int32), ~3, Alu.bitwise_and,
                1, Alu.bitwise_or)
        _ts_int(EE, b1a3[:, :, 79:128][:, :, ::-1].bitcast(dt.int32),
                _blk(epsh[:], 49).bitcast(dt.int32), ~3, Alu.bitwise_and,
                3, Alu.bitwise_or)
    else:
        nc.gpsimd.memset(b1a[:], PADK)
        emq = pool.tile([P, NBLK * 49], dt.uint16, tag="emq")
        EE.tensor_scalar(emq[:], s_sh[:], QS, (1.0 - pw - VOFF) * QS + 0.5,
                         Alu.mult, Alu.add)
        epq = pool.tile([P, NBLK * 49], dt.uint16, tag="epq")
        EE.tensor_scalar(epq[:], s_sh[:], QS, (1.0 + pw - VOFF) * QS + 0.5,
                         Alu.mult, Alu.add)
        EE.tensor_scalar(b1a3[:, :, 0:49], _blk(emq[:], 49), 4, 1,
                         Alu.mult, Alu.add)
        EE.tensor_scalar(b1a3[:, :, 79:128][:, :, ::-1], _blk(epq[:], 49), 4, 3,
                         Alu.mult, Alu.add)
    yield
    _r1 = []
    yield from _merge_gen(ME, b1a, b1b, 128, _r1, ew=98, trim4d=not fkeys)
    b1 = _r1[0]
    yield
    if fkeys:
        nc.gpsimd.memset(b2a3[:, :, X:n2 - 128], BIGF)
        xsh = pool.tile([P, NBLK * X], dt.float32, tag="xsh")
        AE.activation(xsh[:], xt[:], Act.Copy, bias=1.0)
        _ts_int(EE, b2a3[:, :, 0:X].bitcast(dt.int32),
                _blk(xsh[:], X).bitcast(dt.int32), ~3, Alu.bitwise_and)
        EE.tensor_copy(b2a3[:, :, n2 - 128:n2][:, :, ::-1], _blk(b1[:], 128))
    else:
        nc.gpsimd.memset(b2a3[:, :, X:n2 - 128], PADK)
        xq = pool.tile([P, NBLK * X], dt.uint16, tag="xq")
        EE.tensor_scalar(xq[:], xt[:], QS, (1.0 - VOFF) * QS + 0.5,
                         Alu.mult, Alu.add)
        EE.tensor_scalar(b2a3[:, :, 0:X], _blk(xq[:], X), 4, None, Alu.mult)
        EE.tensor_copy(b2a3[:, :, n2 - 128:n2][:, :, ::-1], _blk(b1[:], 128))
    yield
    _r2 = []
    yield from _merge_gen(ME2, b2a, b2b, n2, _r2, ew=L["EW"],
                          trim4d=not fkeys)
    out["m"] = _r2[0]
    yield


def _emit_half(nc, pool, lvl, b0, s_sh, radio_full, mout, consts, acc, eng):
    """Generator: post-merge pipeline for blocks [b0, b0+NB) of one level."""
    SE, XE, EE, FE = (eng[k] for k in ("SE", "XE", "EE", "FE"))
    TE = eng.get("TE", EE)
    SE2 = eng.get("SE2", SE)
    fkeys = eng.get("fkeys", False)
    fchain = eng.get("fchain", False)
    mdt = dt.float32 if (fkeys or fchain) else dt.uint16
    AE = nc.scalar
    L = LVL[lvl]
    X, n2, EW, QWS = L["X"], L["n2"], L["EW"], L["QWS"]
    NL = NB * EW
    NQ = NB * QWS
    NE = NB * 49
    VW = NQ + 2 * NE          # vcat width: [x | em | ep]
    pw = PULSE[lvl]
    maskf, mask_cnt, io49p, ioG, ioQ2 = consts

    def blkE(ap):
        return ap.rearrange("p (b n) -> p b n", b=NB)

    ss = s_sh[:][:, b0 * 49:(b0 + NB) * 49]

    # ---------- sources: exact values + radio (independent of merge) ----------
    vcat = pool.tile([P, VW], dt.float32, tag="vcat")
    nc.gpsimd.memset(_blk(vcat[:, 0:NQ], QWS)[:, :, X:QWS], 0.0)
    radcat = pool.tile([P, 2 * NE], dt.float32, tag="radcat")
    rsl_ = radio_full[:][:, b0 * 49:(b0 + NB) * 49]
    FE.tensor_copy(radcat[:, 0:NE], rsl_)
    FE.tensor_scalar(radcat[:, NE:2 * NE], radcat[:, 0:NE], -1.0, None, Alu.mult)
    yield
    # wait for merge result
    while "m" not in mout:
        yield
    m = mout["m"]
    xt, pwt_full = mout["xt"], mout["pwt"]
    mSh = _blk(m[:], n2)[:, b0:b0 + NB, 0:EW]       # [P, NB, EW] strided
    xts = _blk(xt[:], X)[:, b0:b0 + NB]             # [P, NB, X]
    pwt = _blk(pwt_full[:], X - 1)[:, b0:b0 + NB]
    AE.activation(_blk(vcat[:, 0:NQ], QWS)[:, :, 0:X], xts, Act.Copy)
    AE.activation(_blk(vcat[:, NQ:NQ + NE], 49), _blk(ss, 49), Act.Copy, bias=-pw)
    AE.activation(_blk(vcat[:, NQ + NE:VW], 49), _blk(ss, 49), Act.Copy, bias=pw)
    yield

    # ---------- tags + counts ----------
    if fkeys:
        tag32 = pool.tile([P, NL], dt.int32, tag="tag32")
        _ts_int(XE, blkE(tag32[:]), mSh.bitcast(dt.int32), 3, Alu.bitwise_and)
        ev_f = pool.tile([P, NL], dt.float32, tag="ev_f")
        em_f = pool.tile([P, NL], dt.float32, tag="em_f")
        ep_f = pool.tile([P, NL], dt.float32, tag="ep_f")
        _ts_int(TE, em_f[:], tag32[:], 1, Alu.is_equal)
        _ts_int(TE, ep_f[:], tag32[:], 3, Alu.is_equal)
        FE.tensor_tensor(ev_f[:], em_f[:], ep_f[:], Alu.add)
    elif fchain:
        tagb_t = pool.tile([P, NL], dt.uint16, tag="tagb")
        tagb = tagb_t[:]
        XE.tensor_scalar(blkE(tagb), mSh, 3, None, Alu.bitwise_and)
        em_f = pool.tile([P, NL], dt.float32, tag="em_f")
        TE.tensor_scalar(em_f[:], tagb, 1, None, Alu.is_equal)
        ep_f = pool.tile([P, NL], dt.float32, tag="ep_f")
        TE.tensor_scalar(ep_f[:], tagb, 3, None, Alu.is_equal)
        ev_f = pool.tile([P, NL], dt.float32, tag="ev_f")
        FE.tensor_tensor(ev_f[:], em_f[:], ep_f[:], Alu.add)
    else:
        tagb_t = pool.tile([P, NL], dt.uint16, tag="tagb")
        tagb = tagb_t[:]
        XE.tensor_scalar(blkE(tagb), mSh, 3, None, Alu.bitwise_and)
        ev_f = pool.tile([P, NL], dt.uint16, tag="ev_f")
        TE.tensor_scalar(ev_f[:], tagb, 1, None, Alu.bitwise_and)
        em_f = pool.tile([P, NL], dt.uint16, tag="em_f")
        TE.tensor_scalar(em_f[:], tagb, 1, None, Alu.is_equal)
        ep_f = pool.tile([P, NL], dt.uint16, tag="ep_f")
        TE.tensor_scalar(ep_f[:], tagb, 3, None, Alu.is_equal)
    yield
    C = pool.tile([P, NL], mdt, tag="C")
    SE.tensor_tensor_scan(C[:], mask_cnt, ev_f[:], 0.0, Alu.mult, Alu.add)
    Cm = pool.tile([P, NL], mdt, tag="Cm")
    SE.tensor_tensor_scan(Cm[:], mask_cnt, em_f[:], 0.0, Alu.mult, Alu.add)
    yield

    # ---------- idxcat: merged position of every source element ----------
    t2 = pool.tile([P, NL], mdt, tag="t2")
    t3 = pool.tile([P, NL], mdt, tag="t3")
    if fkeys:
        t1 = tag32                                  # dead after masks
    elif fchain:
        t1 = pool.tile([P, NL], dt.float32, tag="t1f")
    else:
        t1 = tagb_t                                 # dead after masks
    # block offsets (49b/98b) ride in from the mask_cnt scan carry; section
    # offsets NQ / NQ+NE are flat immediates. One combined scatter:
    # t1 = (Cm'+NQ)*em + (C'-Cm'+NQ+NE)*ep + (ioQ2-C')*qf - 1
    EE.tensor_tensor(t2[:], C[:], Cm[:], Alu.subtract)
    EE.tensor_scalar(t2[:], t2[:], NQ + NE, None, Alu.add)
    EE.tensor_tensor(t2[:], t2[:], ep_f[:], Alu.mult)
    t1v = t1[:].bitcast(dt.float32) if fkeys else t1[:]
    ffull = fkeys or fchain
    EE.tensor_scalar(t1v, Cm[:], NQ, None, Alu.add)
    EE.tensor_tensor(t1v, t1v, em_f[:], Alu.mult)
    EE.tensor_tensor(t1v, t1v, t2[:], Alu.add)
    qf = em_f                                       # em_f dead after t1
    TE2 = FE if ffull else TE
    TE2.tensor_scalar(qf[:], ev_f[:], 0, None, Alu.is_equal)
    EE.tensor_tensor(t3[:], ioQ2, C[:], Alu.subtract)
    EE.tensor_tensor(t3[:], t3[:], qf[:], Alu.mult)
    EE.tensor_tensor(t1v, t1v, t3[:], Alu.add)
    EE.tensor_scalar(t1v, t1v, 1, None, Alu.subtract)       # idx all
    EE.tensor_scalar(t2[:], t3[:], 1, None, Alu.subtract)       # idxq
    if ffull:
        t1s = ev_f[:].bitcast(dt.int16)[:, 0:NL]    # ev_f dead after qf
        AE.activation(t1s, t1v, Act.Copy)
        t2s = ep_f[:].bitcast(dt.int16)[:, 0:NL]    # ep_f dead after t2
        AE.activation(t2s, t2[:], Act.Copy)
    else:
        t1s = t1[:].bitcast(dt.int16)
        t2s = t2[:].bitcast(dt.int16)
    yield
    idxcat = pool.tile([P, VW], dt.uint16, tag="idxcat")
    nc.gpsimd.local_scatter(idxcat[:].bitcast(dt.int16), ioG,
                            t1s, channels=P,
                            num_elems=VW, num_idxs=NL)
    idx2 = pool.tile([P, 2 * VW], dt.uint16, tag="idx2")
    i2v = idx2[:].rearrange("p (n two) -> p n two", two=2)
    AE.activation(i2v[:, :, 0], idxcat[:], Act.Copy)
    AE.activation(i2v[:, :, 1], idxcat[:], Act.Copy, bias=1.0)
    yield

    # ---------- pair-scatter exact values + radio into merged domain ----------
    v = pool.tile([P, NL], dt.float32, tag="v")
    nc.gpsimd.local_scatter(v[:].bitcast(dt.int16),
                            vcat[:].bitcast(dt.int16),
                            idx2[:].bitcast(dt.int16), channels=P,
                            num_elems=2 * NL, num_idxs=2 * VW)
    F1 = pool.tile([P, NL], dt.float32, tag="F1")   # radio_m
    nc.gpsimd.local_scatter(F1[:].bitcast(dt.int16),
                            radcat[:].bitcast(dt.int16),
                            idx2[:, 2 * NQ:2 * VW].bitcast(dt.int16), channels=P,
                            num_elems=2 * NL, num_idxs=4 * NE)
    yield

    # ---------- density reconstruction ----------
    F2 = pool.tile([P, NL], dt.float32, tag="F2")
    SE2.tensor_tensor_scan(F2[:], maskf, F1[:], 0.0, Alu.mult, Alu.add)  # g
    if ffull:
        dv = t3                                     # t3 dead after idx phase
    else:
        dv = pool.tile([P, NL], dt.float32, tag="dv")
    dv3 = blkE(dv[:])
    v3 = blkE(v[:])
    nc.gpsimd.memset(dv3[:, :, 0:1], 0.0)
    FE.tensor_tensor(dv3[:, :, 1:EW], v3[:, :, 1:EW], v3[:, :, 0:EW - 1],
                     Alu.subtract)
    yield
    wg = v                                          # v dead after dv
    wg3 = blkE(wg[:])
    nc.gpsimd.memset(wg3[:, :, 0:1], 0.0)
    FE.tensor_tensor(wg3[:, :, 1:EW], dv3[:, :, 1:EW],
                     blkE(F2[:])[:, :, 0:EW - 1], Alu.mult)
    w_t = F1                                        # radio dead after g
    SE2.tensor_tensor_scan(w_t[:], maskf, wg[:], 0.0, Alu.mult, Alu.add)
    yield
    wc = wg                                         # wg dead
    AE.activation(wc[:], w_t[:], Act.Relu, scale=0.5)
    scr = pool.tile([P, NL], dt.float32, tag="scr")
    wc3 = blkE(wc[:])
    s3_ = blkE(scr[:])
    nc.gpsimd.memset(s3_[:, :, 0:1], 0.0)
    FE.tensor_tensor(s3_[:, :, 1:EW], wc3[:, :, 1:EW], wc3[:, :, 0:EW - 1],
                     Alu.add)
    area = w_t                                      # w dead after wc
    a3 = blkE(area[:])
    nc.gpsimd.memset(a3[:, :, 0:1], 0.0)
    FE.tensor_tensor(a3[:, :, 1:EW], s3_[:, :, 1:EW],
                     dv3[:, :, 1:EW], Alu.mult)
    cdf = F2                                        # g dead after wg
    SE2.tensor_tensor_scan(cdf[:], maskf, area[:], 0.0, Alu.mult, Alu.add)
    yield

    # ---------- compact cdf at query slots (pair-scatter) ----------
    idx2q = idx2                                    # idx2 dead after scatters
    i2qv = idx2q[:][:, 0:2 * NL].rearrange("p (n two) -> p n two", two=2)
    tqu = Cm[:].bitcast(dt.uint16)[:, 0:NL]         # Cm dead after t1
    if ffull:
        AE.activation(tqu.bitcast(dt.int16), t2[:], Act.Copy, scale=2.0)
    else:
        EE.tensor_scalar(tqu, t2[:], 2, None, Alu.mult)
    AE.activation(i2qv[:, :, 0], tqu, Act.Copy)
    AE.activation(i2qv[:, :, 1], tqu, Act.Copy, bias=1.0)
    cdfq = vcat                                     # vcat dead after v scatter
    cqn = cdfq[:][:, 0:NQ]
    nc.gpsimd.local_scatter(cqn.bitcast(dt.int16),
                            cdf[:].bitcast(dt.int16),
                            idx2q[:][:, 0:2 * NL].bitcast(dt.int16), channels=P,
                            num_elems=2 * NQ, num_idxs=2 * NL)
    del t2s
    yield

    # ---------- loss tail ----------
    NW = NB * (X - 1)
    ws = scr                                        # dead after area
    ws2 = ws[:][:, 0:NW]
    cqf = _blk(cdfq[:][:, 0:NQ], QWS)
    FE.tensor_tensor(_blk(ws2, X - 1), cqf[:, :, 1:X], cqf[:, :, 0:X - 1],
                     Alu.subtract)
    FE.tensor_tensor(_blk(ws2, X - 1), _blk(ws2, X - 1), pwt, Alu.subtract)
    den = area                                      # dead after cdf
    den2 = den[:][:, 0:NW]
    AE.activation(_blk(den2, X - 1), pwt, Act.Copy, bias=1e-5)
    nc.vector.reciprocal(den2, den2)
    rsl = dv                                        # dead after area
    AE.activation(rsl[:][:, 0:NW], ws2, Act.Relu)
    FE.tensor_tensor(ws2, ws2, rsl[:][:, 0:NW], Alu.mult)
    FE.tensor_tensor(ws2, ws2, den2, Alu.mult)
    nc.vector.tensor_reduce(acc[:], _blk(ws2, X - 1), AX.XY, Alu.add)
    yield


def _emit_setup(nc, pool, s_sh, radios, accs, mask48, aps):
    V, G = nc.vector, nc.gpsimd
    rw_sh = pool.tile([P, NBLK * 48], dt.float32, tag="rw_sh")
    nc.sync.dma_start(_blk(rw_sh[:], 48),
                      aps["rw"].rearrange("(b p) x -> p b x", p=P))
    s3 = _blk(s_sh[:], 49)
    ds = pool.tile([P, NBLK * 48], dt.float32, tag="ds")
    V.tensor_tensor(_blk(ds[:], 48), s3[:, :, 1:49], s3[:, :, 0:48], Alu.subtract)
    dse = pool.tile([P, NBLK * 48], dt.float32, tag="dse")
    nc.scalar.activation(dse[:], ds[:], Act.Copy, bias=1e-8)
    V.reciprocal(dse[:], dse[:])
    wnorm = pool.tile([P, NBLK * 48], dt.float32, tag="wnorm")
    V.tensor_tensor(wnorm[:], rw_sh[:], dse[:], Alu.mult)
    wnp = pool.tile([P, NBLK * 50], dt.float32, tag="wnp")
    G.memset(wnp[:], 0.0)
    V.tensor_copy(_blk(wnp[:], 50)[:, :, 1:49], _blk(wnorm[:], 48))
    diff = pool.tile([P, NBLK * 49], dt.float32, tag="diff")
    wnp3 = _blk(wnp[:], 50)
    V.tensor_tensor(_blk(diff[:], 49), wnp3[:, :, 1:50], wnp3[:, :, 0:49],
                    Alu.subtract)
    for lvl in (0, 1):
        V.tensor_scalar(radios[lvl][:], diff[:], 1.0 / (2 * PULSE[lvl]), None,
                        Alu.mult)
    yield

    mid = pool.tile([P, NBLK * 48], dt.float32, tag="mid")
    V.tensor_tensor(_blk(mid[:], 48), s3[:, :, 1:49], s3[:, :, 0:48], Alu.add)
    wm = pool.tile([P, NBLK * 48], dt.float32, tag="wm")
    V.scalar_tensor_tensor(wm[:], mid[:], 0.5, rw_sh[:], Alu.mult, Alu.mult)
    Cin = pool.tile([P, NBLK * 48], dt.float32, tag="Cin")
    V.tensor_tensor_scan(Cin[:], mask48, rw_sh[:], 0.0, Alu.mult, Alu.add)
    Sin = pool.tile([P, NBLK * 48], dt.float32, tag="Sin")
    V.tensor_tensor_scan(Sin[:], mask48, wm[:], 0.0, Alu.mult, Alu.add)
    yield
    A = pool.tile([P, NBLK * 47], dt.float32, tag="A47")
    m3 = _blk(mid[:], 48)
    c3 = _blk(Cin[:], 48)
    sw3 = _blk(Sin[:], 48)
    rw3 = _blk(rw_sh[:], 48)
    A3 = _blk(A[:], 47)
    V.scalar_tensor_tensor(A3, m3[:, :, 1:48], 0.5, c3[:, :, 0:47],
                           Alu.mult, Alu.mult)
    V.tensor_tensor(A3, A3, sw3[:, :, 0:47], Alu.subtract)
    V.tensor_tensor(A3, A3, rw3[:, :, 1:48], Alu.mult)
    V.tensor_reduce(accs["p1"][:], A3, AX.XY, Alu.add)
    t2 = pool.tile([P, NBLK * 48], dt.float32, tag="t2d")
    G.tensor_tensor(t2[:], rw_sh[:], rw_sh[:], Alu.mult)
    G.tensor_tensor(t2[:], t2[:], ds[:], Alu.mult)
    V.tensor_reduce(accs["p2"][:], _blk(t2[:], 48), AX.XY, Alu.add)
    yield

    pdt = pool.tile([P, NBLK * 3], dt.float32, tag="pdt")
    gtt = pool.tile([P, NBLK * 3], dt.float32, tag="gtt")
    nc.sync.dma_start(_blk(pdt[:], 3), aps["pd"].rearrange("(b p) c -> p b c", p=P))
    nc.sync.dma_start(_blk(gtt[:], 3), aps["gt"].rearrange("(b p) c -> p b c", p=P))
    d = pool.tile([P, NBLK * 3], dt.float32, tag="rgbd")
    V.tensor_tensor(d[:], pdt[:], gtt[:], Alu.subtract)
    V.tensor_tensor(d[:], d[:], d[:], Alu.mult)
    V.tensor_reduce(accs["rgb"][:], d[:], AX.X, Alu.add)
    yield


def _emit_hash(nc, pool, lvl, ones_h, acc, aps, first):
    E = nc.gpsimd if lvl == 0 else nc.vector
    idx = pool.tile([P, HCOLS], dt.int32, tag="hidx")
    src = aps[f"hi{lvl}"]
    nc.sync.dma_start(idx[:], bass.AP(tensor=src.tensor, offset=src.offset,
                                      ap=[[HROW, P], [1, HCOLS]]))
    emb = pool.tile([P, HCOLS * 2], dt.float32, tag="hemb")
    esrc = aps[f"he{lvl}"]
    nc.sync.dma_start(emb[:], bass.AP(tensor=esrc.tensor, offset=esrc.offset,
                                      ap=[[HROW * 2, P], [1, HCOLS * 2]]))
    sq = pool.tile([P, HCOLS * 2], dt.float32, tag="hsq")
    E.tensor_tensor(sq[:], emb[:], emb[:], Alu.mult)
    wv = pool.tile([P, HCOLS], dt.float32, tag="hw")
    sq3 = sq[:].rearrange("p (n two) -> p n two", two=2)
    E.tensor_tensor(wv[:], sq3[:, :, 0], sq3[:, :, 1], Alu.add)
    eq = pool.tile([P, HCOLS], dt.float32, tag="heq")
    nc.gpsimd.memset(eq[:, 0:1], 0.0)
    nc.vector.tensor_tensor(eq[:, 1:HCOLS], idx[:, 1:HCOLS], idx[:, 0:HCOLS - 1],
                             Alu.is_equal)
    yield
    S = pool.tile([P, HCOLS], dt.float32, tag="hS")
    nc.vector.tensor_tensor_scan(S[:], eq[:], wv[:], 0.0, Alu.mult, Alu.add)
    cc = pool.tile([P, HCOLS], dt.float32, tag="hcc")
    nc.vector.tensor_tensor_scan(cc[:], eq[:], ones_h, 0.0, Alu.mult, Alu.add)
    yield
    ratio = pool.tile([P, HCOLS], dt.float32, tag="hr")
    nc.vector.reciprocal(cc[:], cc[:])
    E.tensor_tensor(ratio[:], S[:], cc[:], Alu.mult)
    me = pool.tile([P, HCOLS], dt.float32, tag="hme")
    nc.scalar.activation(me[:, 0:HCOLS - 1], eq[:, 1:HCOLS], Act.Copy,
                         bias=1.0, scale=-1.0)
    E.tensor_tensor(ratio[:, HALO:HALO + HROW], ratio[:, HALO:HALO + HROW],
                    me[:, HALO:HALO + HROW], Alu.mult)
    if first:
        nc.vector.tensor_reduce(acc[:], ratio[:, HALO:HALO + HROW], AX.X, Alu.add)
    else:
        part = pool.tile([P, 1], dt.float32, tag="hpart")
        nc.vector.tensor_reduce(part[:], ratio[:, HALO:HALO + HROW], AX.X,
                                Alu.add)
        E.tensor_tensor(acc[:], acc[:], part[:], Alu.add)
    yield


def build_module(parts=("rgb", "dist", "hash", "l0", "l1")):
    nc = bacc.Bacc("TRN2", target_bir_lowering=False, debug=False,
                   enable_asserts=False, num_devices=N_CORES)
    aps = {}

    def din(name, shape, dtype=dt.float32):
        aps[name] = nc.dram_tensor(name, shape, dtype, kind="ExternalInput").ap()
    din("pd", [RPC, 3]); din("gt", [RPC, 3])
    din("sd", [RPC, 49]); din("rw", [RPC, 48])
    din("ps0", [RPC, 257]); din("pw0", [RPC, 256])
    din("ps1", [RPC, 97]); din("pw1", [RPC, 96])
    din("hi0", [HSLICE], dt.int32); din("he0", [HSLICE * 2])
    din("hi1", [HSLICE], dt.int32); din("he1", [HSLICE * 2])
    for lvl, L in LVL.items():
        NL = NB * L["EW"]
        din(f"c_u16_l{lvl}", [P, 2 * NL], dt.int16)
        din(f"c_maskf_l{lvl}", [P, (3 if lvl == 1 else 2) * NL])
    din("c_mask48", [P, NBLK * 48]); din("c_ones", [P, HCOLS])
    out_ap = nc.dram_tensor("out", [1, 1], dt.float32, kind="ExternalOutput").ap()

    with tile.TileContext(nc) as tc:
        _emit(nc, tc, aps, out_ap, parts)
    nc.compile()
    return nc


def _emit(nc, tc, aps, out_ap, parts=("rgb", "dist", "hash", "l0", "l1")):
    import contextlib
    V, G = nc.vector, nc.gpsimd
    with contextlib.ExitStack() as ctx:
        spool = ctx.enter_context(tc.tile_pool(name="shared", bufs=1))
        s_sh = spool.tile([P, NBLK * 49], dt.float32, tag="s_sh")
        nc.sync.dma_start(_blk(s_sh[:], 49),
                          aps["sd"].rearrange("(b p) x -> p b x", p=P))
        radios = {l: spool.tile([P, NBLK * 49], dt.float32, tag=f"radio{l}",
                                name=f"radio{l}")
                  for l in (0, 1)}

        cpool = ctx.enter_context(tc.tile_pool(name="consts", bufs=1))
        mask48 = cpool.tile([P, NBLK * 48], dt.float32, tag="mask48")
        ones_h = cpool.tile([P, HCOLS], dt.float32, tag="ones_h")
        lvl_consts = {}
        cdma = []
        cdma.append((mask48[:], aps["c_mask48"]))
        cdma.append((ones_h[:], aps["c_ones"]))
        for lvl, L in LVL.items():
            NL = NB * L["EW"]
            cu = cpool.tile([P, 2 * NL], dt.int16, tag=f"cu16_{lvl}",
                            name=f"cu16_{lvl}")
            mf = cpool.tile([P, (3 if lvl == 1 else 2) * NL], dt.float32,
                            tag=f"maskf_{lvl}", name=f"maskf_{lvl}")
            cdma.append((cu[:], aps[f"c_u16_l{lvl}"]))
            cdma.append((mf[:], aps[f"c_maskf_l{lvl}"]))
            cuv = cu[:].bitcast(dt.uint16)
            ioq2 = (mf[:][:, 2 * NL:3 * NL] if lvl == 1
                    else cuv[:, NL:2 * NL])
            lvl_consts[lvl] = (mf[:][:, 0:NL], mf[:][:, NL:2 * NL],
                               None, cu[:][:, 0:NL], ioq2)
            # (maskf, mask_cnt(f32), unused, ioG(i16), ioQ2)

        def _emit_consts():
            for dst, src_ap in cdma:
                nc.sync.dma_start(dst, src_ap)
            yield

        accs = {}
        for name in ("rgb", "p1", "p2", "hash", "l0a", "l0b", "l1a", "l1b"):
            accs[name] = cpool.tile([P, 1], dt.float32, tag=f"acc_{name}",
                                    name=f"acc_{name}")
            V.memset(accs[name][:], 0.0)

        spool = ctx.enter_context(tc.tile_pool(name="shared", bufs=1))
        s_sh = spool.tile([P, NBLK * 49], dt.float32, tag="s_sh")
        nc.sync.dma_start(_blk(s_sh[:], 49),
                          aps["sd"].rearrange("(b p) x -> p b x", p=P))
        radios = {l: spool.tile([P, NBLK * 49], dt.float32, tag=f"radio{l}",
                                name=f"radio{l}")
                  for l in (0, 1)}

        MRG = {0: dict(ME=V, ME2=V, EE=V),
               1: dict(ME=V, ME2=V, EE=V)}
        HEM = {
            "l0a": dict(SE=V, SE2=V, XE=V, EE=V, TE=V, FE=G),
            "l0b": dict(SE=V, SE2=V, XE=V, EE=V, TE=V, FE=G),
            "l1a": dict(SE=V, SE2=V, XE=V, EE=G, TE=G, FE=G, fchain=True),
            "l1b": dict(SE=V, SE2=V, XE=V, EE=G, TE=G, FE=G, fchain=True),
        }

        gens = []
        setup_pool = ctx.enter_context(tc.tile_pool(name="setup", bufs=1))
        gens.append(_emit_setup(nc, setup_pool, s_sh, radios, accs, mask48[:],
                                aps))
        mouts = {}
        for lvl in (0, 1):
            if f"l{lvl}" not in parts:
                continue
            mouts[lvl] = {}
            mp = ctx.enter_context(tc.tile_pool(name=f"mrg{lvl}", bufs=1))
            gens.append(_emit_level_merge(nc, tc, mp, lvl, s_sh,
                                          aps[f"ps{lvl}"], aps[f"pw{lvl}"],
                                          mouts[lvl], MRG[lvl]))
        gens.append(_emit_consts())
        if "hash" in parts:
            for lvl in (0, 1):
                hp2 = ctx.enter_context(tc.tile_pool(name=f"hash{lvl}", bufs=1))
                gens.append(_emit_hash(nc, hp2, lvl, ones_h[:], accs["hash"],
                                       aps, first=(lvl == 0)))
        for name, lvl, b0 in HALVES:
            if f"l{lvl}" not in parts:
                continue
            hp = ctx.enter_context(tc.tile_pool(name=name, bufs=1))
            gens.append(_emit_half(nc, hp, lvl, b0, s_sh, radios[lvl],
                                   mouts[lvl], lvl_consts[lvl], accs[name],
                                   HEM[name]))

        while gens:
            nxt = []
            for g in gens:
                try:
                    next(g)
                    nxt.append(g)
                except StopIteration:
                    pass
            gens = nxt

        with tc.tile_pool(name="fin", bufs=1) as pool:
            tot = pool.tile([P, 1], dt.float32, tag="tot")
            V.tensor_scalar(tot[:], accs["rgb"][:], W_RGB / (R * 3), None,
                            Alu.mult)
            for snm, lvl, _ in HALVES:
                V.scalar_tensor_tensor(tot[:], accs[snm][:],
                                       W_INTER / (R * (LVL[lvl]["X"] - 1)),
                                       tot[:], Alu.mult, Alu.add)
            V.scalar_tensor_tensor(tot[:], accs["p1"][:], W_DIST * 2.0 / R,
                                   tot[:], Alu.mult, Alu.add)
            V.scalar_tensor_tensor(tot[:], accs["p2"][:], W_DIST / (3.0 * R),
                                   tot[:], Alu.mult, Alu.add)
            V.scalar_tensor_tensor(tot[:], accs["hash"][:],
                                   W_HASH / (NUM_SEGMENTS * 2.0), tot[:],
                                   Alu.mult, Alu.add)
            res = pool.tile([1, 1], dt.float32, tag="res")
            G.tensor_reduce(res[:], tot[:], AX.C, Alu.add)
            nc.sync.dma_start(out_ap, res[:])


# ---------------- host side ----------------
_module_cache = {}


def _get_module():
    if "nc" not in _module_cache:
        _module_cache["nc"] = build_module()
    return _module_cache["nc"]


def shard_inputs(inputs):
    f32 = np.float32
    pd = np.ascontiguousarray(inputs["pd_rgbs"], f32)
    gt = np.ascontiguousarray(inputs["gt_rgbs"], f32)
    sd = np.ascontiguousarray(inputs["render_sdist"], f32)
    rw = np.ascontiguousarray(inputs["render_weights"], f32)
    ps0 = np.ascontiguousarray(inputs["prop_sdist_0"], f32)
    pw0 = np.ascontiguousarray(inputs["prop_weights_0"], f32)
    ps1 = np.ascontiguousarray(inputs["prop_sdist_1"], f32)
    pw1 = np.ascontiguousarray(inputs["prop_weights_1"], f32)
    hashes = {}
    for lvl in (0, 1):
        idx = np.asarray(inputs[f"enc_idx_{lvl}"]).astype(np.int32)
        emb = np.ascontiguousarray(inputs[f"enc_embds_{lvl}"], f32)
        idx_pad = np.full(M + 2 * HALO, -1, np.int32)
        idx_pad[HALO:HALO + M] = idx
        emb_pad = np.zeros((M + 2 * HALO, 2), f32)
        emb_pad[HALO:HALO + M] = emb
        hashes[lvl] = (idx_pad, emb_pad)

    consts = {}
    rep = lambda row: np.ascontiguousarray(np.tile(row, (P, 1)))
    for lvl, L in LVL.items():
        EW, QWS, X = L["EW"], L["QWS"], L["X"]
        NL = NB * EW
        NQ = NB * QWS
        ioG = np.concatenate([2 * np.arange(b * EW, (b + 1) * EW,
                                            dtype=np.uint16)
                              for b in range(NB)])
        # query dest: rank-1 + b*QWS; C' = C + 98b so fold +98b here:
        # ioQ2 = i+1 + b*QWS + 98b -> (ioQ2 - C')*qf - 1 = rank-1 + b*QWS
        ioQ2 = np.concatenate([np.arange(1, EW + 1, dtype=np.uint16)
                               + b * QWS + 98 * b for b in range(NB)])
        packed = np.concatenate([ioG, ioQ2]).astype(np.uint16)
        consts[f"c_u16_l{lvl}"] = rep(packed.view(np.int16))
        msk = np.ones(NL, f32)
        msk[::EW] = 0.0
        mcnt = np.ones(NL, f32)
        for b in range(NB):
            mcnt[b * EW] = b
        parts_ = [msk, mcnt]
        if lvl == 1:
            parts_.append(np.concatenate([np.arange(1, EW + 1, dtype=f32)
                                          + b * QWS + 98 * b
                                          for b in range(NB)]))
        consts[f"c_maskf_l{lvl}"] = rep(np.concatenate(parts_))
    m48 = np.ones(NBLK * 48, f32)
    m48[::48] = 0.0
    consts["c_mask48"] = rep(m48)
    consts["c_ones"] = rep(np.ones(HCOLS, f32))

    in_maps = []
    for c in range(N_CORES):
        r0 = c * RPC
        lo = c * MPC
        im = {
            "pd": pd[r0:r0 + RPC], "gt": gt[r0:r0 + RPC],
            "sd": sd[r0:r0 + RPC], "rw": rw[r0:r0 + RPC],
            "ps0": ps0[r0:r0 + RPC], "pw0": pw0[r0:r0 + RPC],
            "ps1": ps1[r0:r0 + RPC], "pw1": pw1[r0:r0 + RPC],
        }
        for lvl in (0, 1):
            idx_pad, emb_pad = hashes[lvl]
            im[f"hi{lvl}"] = np.ascontiguousarray(idx_pad[lo:lo + HSLICE])
            im[f"he{lvl}"] = np.ascontiguousarray(
                emb_pad[lo:lo + HSLICE].reshape(-1))
        im.update(consts)
        in_maps.append(im)
    return in_maps


def kernel(**inputs) -> np.ndarray:
    nc = _get_module()
    in_maps = shard_inputs(inputs)
    res = run_bass_kernel_spmd(nc, in_maps, core_ids=list(range(N_CORES)))
    total = np.float64(0.0)
    for r in res.results:
        total += np.float64(r["out"][0, 0])
    return np.float32(total)


# revision 7
# speedup vs baseline: 1.6569x; 1.0267x over previous
"""Trainium2 Bass kernel v2 for nn_Loss_dict_50646254354805 (NeRF-style loss).

v2 vs baseline:
- bitonic merges on uint16 quantized keys (value*15000 + 2 tag bits) -> DVE
  2x perf mode; keys determine ORDER only.
- exact f32 values (queries/em/ep) and radio reach the merged domain via
  batched u16-half local_scatters through one shared index table (idxcat):
  merged positions come from the C/Cm count scans.
- one merge per level; post-merge work split into two 2-block half-streams
  with per-stream engine maps; all generators emitted stage-interleaved so
  DVE / Pool / Act overlap.
"""
import numpy as np

import concourse.bass as bass
import concourse.mybir as mybir
import concourse.tile as tile
from concourse import bacc
from concourse.bass_utils import run_bass_kernel_spmd

dt = mybir.dt
Alu = mybir.AluOpType
AX = mybir.AxisListType
Act = mybir.ActivationFunctionType
P = 128

PULSE = (0.01, 0.005)
W_RGB, W_INTER, W_DIST, W_HASH = 1.0, 1.0, 0.01, 0.1
NUM_SEGMENTS = 65536
R, N = 4096, 48
M = R * N
N_CORES = 8
RPC = R // N_CORES
NBLK = RPC // P               # 4 ray blocks per core
MPC = M // N_CORES
HALO = 64
HROW = MPC // P
HCOLS = HROW + HALO + 1
HSLICE = HALO + MPC + HALO

VOFF = 0.97
QS = 15000.0                  # key quantization scale
PADK = 0xFFFC

LVL = {0: dict(X=257, n2=512), 1: dict(X=97, n2=256)}
for _L in LVL.values():
    _L["EW"] = ((_L["X"] + 98 + 1 + 7) // 8) * 8        # 360 / 200
    _L["QWS"] = _L["EW"] - 98                           # 262 / 102

NB = 2                        # blocks per half-stream
HALVES = [("l0a", 0, 0), ("l0b", 0, 2), ("l1a", 1, 0), ("l1b", 1, 2)]


def _blk(ap, n):
    return ap.rearrange("p (b n) -> p b n", n=n)


def _ts_int(eng, out, in0, imm1, op0, imm2=None, op1=None):
    ins_ = [eng.lower_ap(in0), mybir.ImmediateValue(dtype=dt.int32, value=int(imm1))]
    kw = dict(op0=op0)
    if imm2 is not None:
        ins_.append(mybir.ImmediateValue(dtype=dt.int32, value=int(imm2)))
        kw["op1"] = op1
    return eng.add_instruction(mybir.InstTensorScalarPtr(
        name=eng.bass.get_next_instruction_name(),
        ins=ins_, outs=[eng.lower_ap(out)], **kw))

BIGF = 3.0


def _merge_gen(eng, bufa, bufb, width, out, ew=None, trim4d=True):
    """Ascending bitonic merge over [P, NBLK*width] u16 ping-pong tiles.
    Generator: yields after each stage so two levels' merges interleave in
    the engine queue. Result tile is appended to `out`.

    If ew is given, only outputs [0, ew+2d-1] of each block are needed
    downstream, so late stages skip whole 2d-chunks beyond that window."""
    cur, nxt = bufa, bufb
    d = width // 2
    while d >= 1:
        nch = width // (2 * d)
        keep = nch
        if ew is not None and trim4d:
            keep = min(nch, -(-(ew + 2 * d - 1) // (2 * d)))
        if keep == nch:
            c3 = cur[:].rearrange("p (c td) -> p c td", td=2 * d)
            n3 = nxt[:].rearrange("p (c td) -> p c td", td=2 * d)
        else:
            c3 = cur[:].rearrange("p (b c td) -> p (b c) td",
                                  td=2 * d, c=nch)[: , 0:0]  # placeholder
        if keep == nch:
            lo_in, hi_in = c3[:, :, 0:d], c3[:, :, d:2 * d]
            eng.tensor_tensor(n3[:, :, 0:d], lo_in, hi_in, Alu.min)
            eng.tensor_tensor(n3[:, :, d:2 * d], lo_in, hi_in, Alu.max)
        else:
            c4 = cur[:].rearrange("p (b c td) -> p b c td", td=2 * d, c=nch)
            n4 = nxt[:].rearrange("p (b c td) -> p b c td", td=2 * d, c=nch)
            lo_in = c4[:, :, 0:keep, 0:d]
            hi_in = c4[:, :, 0:keep, d:2 * d]
            eng.tensor_tensor(n4[:, :, 0:keep, 0:d], lo_in, hi_in, Alu.min)
            eng.tensor_tensor(n4[:, :, 0:keep, d:2 * d], lo_in, hi_in, Alu.max)
        cur, nxt = nxt, cur
        d //= 2
        if d >= 1:
            yield
    out.append(cur)


def _emit_level_merge(nc, tc, pool, lvl, s_sh, x_ap, pwt_ap, out, eng):
    """Generator: quantize + b1/b2 merges for all 4 blocks of one level.

    lvl 0: uint16 quantized keys, merged on DVE (2x mode).
    lvl 1: f32-bitcast tagged keys (baseline-style), merged on Pool where
    f32 min/max is legal -- frees DVE during the big level-0 merge."""
    ME, ME2, EE = eng["ME"], eng["ME2"], eng["EE"]
    AE = nc.scalar
    fkeys = eng.get("fkeys", False)
    L = LVL[lvl]
    X, n2 = L["X"], L["n2"]
    pw = PULSE[lvl]
    kdt = dt.float32 if fkeys else dt.uint16

    xt = pool.tile([P, NBLK * X], dt.float32, tag="xt")
    nc.sync.dma_start(_blk(xt[:], X), x_ap.rearrange("(b p) x -> p b x", p=P))
    pwt = pool.tile([P, NBLK * (X - 1)], dt.float32, tag="pwt")
    nc.sync.dma_start(_blk(pwt[:], X - 1),
                      pwt_ap.rearrange("(b p) x -> p b x", p=P))
    out["xt"] = xt
    out["pwt"] = pwt

    b2a = pool.tile([P, NBLK * n2], kdt, tag="b2a")
    b2b = pool.tile([P, NBLK * n2], kdt, tag="b2b")
    b2a3 = _blk(b2a[:], n2)
    b1a = pool.tile([P, NBLK * 128], kdt, tag="b1a")
    b1b = pool.tile([P, NBLK * 128], kdt, tag="b1b")
    b1a3 = _blk(b1a[:], 128)
    if fkeys:
        nc.gpsimd.memset(b1a[:], BIGF)
        emsh = pool.tile([P, NBLK * 49], dt.float32, tag="emsh")
        AE.activation(emsh[:], s_sh[:], Act.Copy, bias=1.0 - pw)
        epsh = pool.tile([P, NBLK * 49], dt.float32, tag="epsh")
        AE.activation(epsh[:], s_sh[:], Act.Copy, bias=1.0 + pw)
        _ts_int(EE, b1a3[:, :, 0:49].bitcast(dt.int32),
                _blk(emsh[:], 49).bitcast(dt.int32), ~3, Alu.bitwise_and,
                1, Alu.bitwise_or)
        _ts_int(EE, b1a3[:, :, 79:128][:, :, ::-1].bitcast(dt.int32),
                _blk(epsh[:], 49).bitcast(dt.int32), ~3, Alu.bitwise_and,
                3, Alu.bitwise_or)
    else:
        nc.gpsimd.memset(b1a[:], PADK)
        emq = pool.tile([P, NBLK * 49], dt.uint16, tag="emq")
        EE.tensor_scalar(emq[:], s_sh[:], QS, (1.0 - pw - VOFF) * QS + 0.5,
                         Alu.mult, Alu.add)
        epq = pool.tile([P, NBLK * 49], dt.uint16, tag="epq")
        EE.tensor_scalar(epq[:], s_sh[:], QS, (1.0 + pw - VOFF) * QS + 0.5,
                         Alu.mult, Alu.add)
        EE.tensor_scalar(b1a3[:, :, 0:49], _blk(emq[:], 49), 4, 1,
                         Alu.mult, Alu.add)
        EE.tensor_scalar(b1a3[:, :, 79:128][:, :, ::-1], _blk(epq[:], 49), 4, 3,
                         Alu.mult, Alu.add)
    yield
    _r1 = []
    yield from _merge_gen(ME, b1a, b1b, 128, _r1, ew=98, trim4d=not fkeys)
    b1 = _r1[0]
    yield
    if fkeys:
        nc.gpsimd.memset(b2a3[:, :, X:n2 - 128], BIGF)
        xsh = pool.tile([P, NBLK * X], dt.float32, tag="xsh")
        AE.activation(xsh[:], xt[:], Act.Copy, bias=1.0)
        _ts_int(EE, b2a3[:, :, 0:X].bitcast(dt.int32),
                _blk(xsh[:], X).bitcast(dt.int32), ~3, Alu.bitwise_and)
        EE.tensor_copy(b2a3[:, :, n2 - 128:n2][:, :, ::-1], _blk(b1[:], 128))
    else:
        nc.gpsimd.memset(b2a3[:, :, X:n2 - 128], PADK)
        xq = pool.tile([P, NBLK * X], dt.uint16, tag="xq")
        EE.tensor_scalar(xq[:], xt[:], QS, (1.0 - VOFF) * QS + 0.5,
                         Alu.mult, Alu.add)
        EE.tensor_scalar(b2a3[:, :, 0:X], _blk(xq[:], X), 4, None, Alu.mult)
        EE.tensor_copy(b2a3[:, :, n2 - 128:n2][:, :, ::-1], _blk(b1[:], 128))
    yield
    _r2 = []
    yield from _merge_gen(ME2, b2a, b2b, n2, _r2, ew=L["EW"],
                          trim4d=not fkeys)
    out["m"] = _r2[0]
    yield


def _emit_half(nc, pool, lvl, b0, s_sh, radio_full, mout, consts, acc, eng):
    """Generator: post-merge pipeline for blocks [b0, b0+NB) of one level."""
    SE, XE, EE, FE = (eng[k] for k in ("SE", "XE", "EE", "FE"))
    TE = eng.get("TE", EE)
    SE2 = eng.get("SE2", SE)
    fkeys = eng.get("fkeys", False)
    fchain = eng.get("fchain", False)
    mdt = dt.float32 if (fkeys or fchain) else dt.uint16
    AE = nc.scalar
    L = LVL[lvl]
    X, n2, EW, QWS = L["X"], L["n2"], L["EW"], L["QWS"]
    NL = NB * EW
    NQ = NB * QWS
    NE = NB * 49
    VW = NQ + 2 * NE          # vcat width: [x | em | ep]
    pw = PULSE[lvl]
    maskf, mask_cnt, io49p, ioG, ioQ2 = consts

    def blkE(ap):
        return ap.rearrange("p (b n) -> p b n", b=NB)

    ss = s_sh[:][:, b0 * 49:(b0 + NB) * 49]

    # ---------- sources: exact values + radio (independent of merge) ----------
    vcat = pool.tile([P, VW], dt.float32, tag="vcat")
    nc.gpsimd.memset(_blk(vcat[:, 0:NQ], QWS)[:, :, X:QWS], 0.0)
    radcat = pool.tile([P, 2 * NE], dt.float32, tag="radcat")
    rsl_ = radio_full[:][:, b0 * 49:(b0 + NB) * 49]
    FE.tensor_copy(radcat[:, 0:NE], rsl_)
    FE.tensor_scalar(radcat[:, NE:2 * NE], radcat[:, 0:NE], -1.0, None, Alu.mult)
    yield
    # wait for merge result
    while "m" not in mout:
        yield
    m = mout["m"]
    xt, pwt_full = mout["xt"], mout["pwt"]
    mSh = _blk(m[:], n2)[:, b0:b0 + NB, 0:EW]       # [P, NB, EW] strided
    xts = _blk(xt[:], X)[:, b0:b0 + NB]             # [P, NB, X]
    pwt = _blk(pwt_full[:], X - 1)[:, b0:b0 + NB]
    AE.activation(_blk(vcat[:, 0:NQ], QWS)[:, :, 0:X], xts, Act.Copy)
    AE.activation(_blk(vcat[:, NQ:NQ + NE], 49), _blk(ss, 49), Act.Copy, bias=-pw)
    AE.activation(_blk(vcat[:, NQ + NE:VW], 49), _blk(ss, 49), Act.Copy, bias=pw)
    yield

    # ---------- tags + counts ----------
    if fkeys:
        tag32 = pool.tile([P, NL], dt.int32, tag="tag32")
        _ts_int(XE, blkE(tag32[:]), mSh.bitcast(dt.int32), 3, Alu.bitwise_and)
        ev_f = pool.tile([P, NL], dt.float32, tag="ev_f")
        em_f = pool.tile([P, NL], dt.float32, tag="em_f")
        ep_f = pool.tile([P, NL], dt.float32, tag="ep_f")
        _ts_int(TE, em_f[:], tag32[:], 1, Alu.is_equal)
        _ts_int(TE, ep_f[:], tag32[:], 3, Alu.is_equal)
        FE.tensor_tensor(ev_f[:], em_f[:], ep_f[:], Alu.add)
    elif fchain:
        tagb_t = pool.tile([P, NL], dt.uint16, tag="tagb")
        tagb = tagb_t[:]
        XE.tensor_scalar(blkE(tagb), mSh, 3, None, Alu.bitwise_and)
        em_f = pool.tile([P, NL], dt.float32, tag="em_f")
        TE.tensor_scalar(em_f[:], tagb, 1, None, Alu.is_equal)
        ep_f = pool.tile([P, NL], dt.float32, tag="ep_f")
        TE.tensor_scalar(ep_f[:], tagb, 3, None, Alu.is_equal)
        ev_f = pool.tile([P, NL], dt.float32, tag="ev_f")
        FE.tensor_tensor(ev_f[:], em_f[:], ep_f[:], Alu.add)
    else:
        tagb_t = pool.tile([P, NL], dt.uint16, tag="tagb")
        tagb = tagb_t[:]
        XE.tensor_scalar(blkE(tagb), mSh, 3, None, Alu.bitwise_and)
        ev_f = pool.tile([P, NL], dt.uint16, tag="ev_f")
        TE.tensor_scalar(ev_f[:], tagb, 1, None, Alu.bitwise_and)
        em_f = pool.tile([P, NL], dt.uint16, tag="em_f")
        TE.tensor_scalar(em_f[:], tagb, 1, None, Alu.is_equal)
        ep_f = pool.tile([P, NL], dt.uint16, tag="ep_f")
        TE.tensor_scalar(ep_f[:], tagb, 3, None, Alu.is_equal)
    yield
    C = pool.tile([P, NL], mdt, tag="C")
    SE.tensor_tensor_scan(C[:], mask_cnt, ev_f[:], 0.0, Alu.mult, Alu.add)
    Cm = pool.tile([P, NL], mdt, tag="Cm")
    SE.tensor_tensor_scan(Cm[:], mask_cnt, em_f[:], 0.0, Alu.mult, Alu.add)
    yield

    # ---------- idxcat: merged position of every source element ----------
    t2 = pool.tile([P, NL], mdt, tag="t2")
    t3 = pool.tile([P, NL], mdt, tag="t3")
    if fkeys:
        t1 = tag32                                  # dead after masks
    elif fchain:
        t1 = pool.tile([P, NL], dt.float32, tag="t1f")
    else:
        t1 = tagb_t                                 # dead after masks
    # block offsets (49b/98b) ride in from the mask_cnt scan carry; section
    # offsets NQ / NQ+NE are flat immediates. One combined scatter:
    # t1 = (Cm'+NQ)*em + (C'-Cm'+NQ+NE)*ep + (ioQ2-C')*qf - 1
    EE.tensor_tensor(t2[:], C[:], Cm[:], Alu.subtract)
    EE.tensor_scalar(t2[:], t2[:], NQ + NE, None, Alu.add)
    EE.tensor_tensor(t2[:], t2[:], ep_f[:], Alu.mult)
    t1v = t1[:].bitcast(dt.float32) if fkeys else t1[:]
    ffull = fkeys or fchain
    EE.tensor_scalar(t1v, Cm[:], NQ, None, Alu.add)
    EE.tensor_tensor(t1v, t1v, em_f[:], Alu.mult)
    EE.tensor_tensor(t1v, t1v, t2[:], Alu.add)
    qf = em_f                                       # em_f dead after t1
    TE2 = FE if ffull else TE
    TE2.tensor_scalar(qf[:], ev_f[:], 0, None, Alu.is_equal)
    EE.tensor_tensor(t3[:], ioQ2, C[:], Alu.subtract)
    EE.tensor_tensor(t3[:], t3[:], qf[:], Alu.mult)
    EE.tensor_tensor(t1v, t1v, t3[:], Alu.add)
    EE.tensor_scalar(t1v, t1v, 1, None, Alu.subtract)       # idx all
    EE.tensor_scalar(t2[:], t3[:], 1, None, Alu.subtract)       # idxq
    if ffull:
        t1s = ev_f[:].bitcast(dt.int16)[:, 0:NL]    # ev_f dead after qf
        AE.activation(t1s, t1v, Act.Copy)
        t2s = ep_f[:].bitcast(dt.int16)[:, 0:NL]    # ep_f dead after t2
        AE.activation(t2s, t2[:], Act.Copy)
    else:
        t1s = t1[:].bitcast(dt.int16)
        t2s = t2[:].bitcast(dt.int16)
    yield
    idxcat = pool.tile([P, VW], dt.uint16, tag="idxcat")
    nc.gpsimd.local_scatter(idxcat[:].bitcast(dt.int16), ioG,
                            t1s, channels=P,
                            num_elems=VW, num_idxs=NL)
    idx2 = pool.tile([P, 2 * VW], dt.uint16, tag="idx2")
    i2v = idx2[:].rearrange("p (n two) -> p n two", two=2)
    AE.activation(i2v[:, :, 0], idxcat[:], Act.Copy)
    AE.activation(i2v[:, :, 1], idxcat[:], Act.Copy, bias=1.0)
    yield

    # ---------- pair-scatter exact values + radio into merged domain ----------
    v = pool.tile([P, NL], dt.float32, tag="v")
    nc.gpsimd.local_scatter(v[:].bitcast(dt.int16),
                            vcat[:].bitcast(dt.int16),
                            idx2[:].bitcast(dt.int16), channels=P,
                            num_elems=2 * NL, num_idxs=2 * VW)
    F1 = pool.tile([P, NL], dt.float32, tag="F1")   # radio_m
    nc.gpsimd.local_scatter(F1[:].bitcast(dt.int16),
                            radcat[:].bitcast(dt.int16),
                            idx2[:, 2 * NQ:2 * VW].bitcast(dt.int16), channels=P,
                            num_elems=2 * NL, num_idxs=4 * NE)
    yield

    # ---------- density reconstruction ----------
    F2 = pool.tile([P, NL], dt.float32, tag="F2")
    SE2.tensor_tensor_scan(F2[:], maskf, F1[:], 0.0, Alu.mult, Alu.add)  # g
    if ffull:
        dv = t3                                     # t3 dead after idx phase
    else:
        dv = pool.tile([P, NL], dt.float32, tag="dv")
    dv3 = blkE(dv[:])
    v3 = blkE(v[:])
    nc.gpsimd.memset(dv3[:, :, 0:1], 0.0)
    FE.tensor_tensor(dv3[:, :, 1:EW], v3[:, :, 1:EW], v3[:, :, 0:EW - 1],
                     Alu.subtract)
    yield
    wg = v                                          # v dead after dv
    wg3 = blkE(wg[:])
    nc.gpsimd.memset(wg3[:, :, 0:1], 0.0)
    FE.tensor_tensor(wg3[:, :, 1:EW], dv3[:, :, 1:EW],
                     blkE(F2[:])[:, :, 0:EW - 1], Alu.mult)
    w_t = F1                                        # radio dead after g
    SE2.tensor_tensor_scan(w_t[:], maskf, wg[:], 0.0, Alu.mult, Alu.add)
    yield
    wc = wg                                         # wg dead
    AE.activation(wc[:], w_t[:], Act.Relu, scale=0.5)
    scr = pool.tile([P, NL], dt.float32, tag="scr")
    wc3 = blkE(wc[:])
    s3_ = blkE(scr[:])
    nc.gpsimd.memset(s3_[:, :, 0:1], 0.0)
    FE.tensor_tensor(s3_[:, :, 1:EW], wc3[:, :, 1:EW], wc3[:, :, 0:EW - 1],
                     Alu.add)
    area = w_t                                      # w dead after wc
    a3 = blkE(area[:])
    nc.gpsimd.memset(a3[:, :, 0:1], 0.0)
    FE.tensor_tensor(a3[:, :, 1:EW], s3_[:, :, 1:EW],
                     dv3[:, :, 1:EW], Alu.mult)
    cdf = F2                                        # g dead after wg
    SE2.tensor_tensor_scan(cdf[:], maskf, area[:], 0.0, Alu.mult, Alu.add)
    yield

    # ---------- compact cdf at query slots (pair-scatter) ----------
    idx2q = idx2                                    # idx2 dead after scatters
    i2qv = idx2q[:][:, 0:2 * NL].rearrange("p (n two) -> p n two", two=2)
    tqu = Cm[:].bitcast(dt.uint16)[:, 0:NL]         # Cm dead after t1
    if ffull:
        AE.activation(tqu.bitcast(dt.int16), t2[:], Act.Copy, scale=2.0)
    else:
        EE.tensor_scalar(tqu, t2[:], 2, None, Alu.mult)
    AE.activation(i2qv[:, :, 0], tqu, Act.Copy)
    AE.activation(i2qv[:, :, 1], tqu, Act.Copy, bias=1.0)
    cdfq = vcat                                     # vcat dead after v scatter
    cqn = cdfq[:][:, 0:NQ]
    nc.gpsimd.local_scatter(cqn.bitcast(dt.int16),
                            cdf[:].bitcast(dt.int16),
                            idx2q[:][:, 0:2 * NL].bitcast(dt.int16), channels=P,
                            num_elems=2 * NQ, num_idxs=2 * NL)
    del t2s
    yield

    # ---------- loss tail ----------
    NW = NB * (X - 1)
    ws = scr                                        # dead after area
    ws2 = ws[:][:, 0:NW]
    cqf = _blk(cdfq[:][:, 0:NQ], QWS)
    FE.tensor_tensor(_blk(ws2, X - 1), cqf[:, :, 1:X], cqf[:, :, 0:X - 1],
                     Alu.subtract)
    FE.tensor_tensor(_blk(ws2, X - 1), _blk(ws2, X - 1), pwt, Alu.subtract)
    den = area                                      # dead after cdf
    den2 = den[:][:, 0:NW]
    AE.activation(_blk(den2, X - 1), pwt, Act.Copy, bias=1e-5)
    nc.vector.reciprocal(den2, den2)
    rsl = dv                                        # dead after area
    AE.activation(rsl[:][:, 0:NW], ws2, Act.Relu)
    FE.tensor_tensor(ws2, ws2, rsl[:][:, 0:NW], Alu.mult)
    FE.tensor_tensor(ws2, ws2, den2, Alu.mult)
    nc.vector.tensor_reduce(acc[:], _blk(ws2, X - 1), AX.XY, Alu.add)
    yield


def _emit_setup(nc, pool, s_sh, radios, accs, mask48, aps):
    V, G = nc.vector, nc.gpsimd
    rw_sh = pool.tile([P, NBLK * 48], dt.float32, tag="rw_sh")
    nc.sync.dma_start(_blk(rw_sh[:], 48),
                      aps["rw"].rearrange("(b p) x -> p b x", p=P))
    s3 = _blk(s_sh[:], 49)
    ds = pool.tile([P, NBLK * 48], dt.float32, tag="ds")
    V.tensor_tensor(_blk(ds[:], 48), s3[:, :, 1:49], s3[:, :, 0:48], Alu.subtract)
    dse = pool.tile([P, NBLK * 48], dt.float32, tag="dse")
    nc.scalar.activation(dse[:], ds[:], Act.Copy, bias=1e-8)
    V.reciprocal(dse[:], dse[:])
    wnorm = pool.tile([P, NBLK * 48], dt.float32, tag="wnorm")
    V.tensor_tensor(wnorm[:], rw_sh[:], dse[:], Alu.mult)
    wnp = pool.tile([P, NBLK * 50], dt.float32, tag="wnp")
    G.memset(wnp[:], 0.0)
    V.tensor_copy(_blk(wnp[:], 50)[:, :, 1:49], _blk(wnorm[:], 48))
    diff = pool.tile([P, NBLK * 49], dt.float32, tag="diff")
    wnp3 = _blk(wnp[:], 50)
    V.tensor_tensor(_blk(diff[:], 49), wnp3[:, :, 1:50], wnp3[:, :, 0:49],
                    Alu.subtract)
    for lvl in (0, 1):
        V.tensor_scalar(radios[lvl][:], diff[:], 1.0 / (2 * PULSE[lvl]), None,
                        Alu.mult)
    yield

    mid = pool.tile([P, NBLK * 48], dt.float32, tag="mid")
    V.tensor_tensor(_blk(mid[:], 48), s3[:, :, 1:49], s3[:, :, 0:48], Alu.add)
    wm = pool.tile([P, NBLK * 48], dt.float32, tag="wm")
    V.scalar_tensor_tensor(wm[:], mid[:], 0.5, rw_sh[:], Alu.mult, Alu.mult)
    Cin = pool.tile([P, NBLK * 48], dt.float32, tag="Cin")
    V.tensor_tensor_scan(Cin[:], mask48, rw_sh[:], 0.0, Alu.mult, Alu.add)
    Sin = pool.tile([P, NBLK * 48], dt.float32, tag="Sin")
    V.tensor_tensor_scan(Sin[:], mask48, wm[:], 0.0, Alu.mult, Alu.add)
    yield
    A = pool.tile([P, NBLK * 47], dt.float32, tag="A47")
    m3 = _blk(mid[:], 48)
    c3 = _blk(Cin[:], 48)
    sw3 = _blk(Sin[:], 48)
    rw3 = _blk(rw_sh[:], 48)
    A3 = _blk(A[:], 47)
    V.scalar_tensor_tensor(A3, m3[:, :, 1:48], 0.5, c3[:, :, 0:47],
                           Alu.mult, Alu.mult)
    V.tensor_tensor(A3, A3, sw3[:, :, 0:47], Alu.subtract)
    V.tensor_tensor(A3, A3, rw3[:, :, 1:48], Alu.mult)
    V.tensor_reduce(accs["p1"][:], A3, AX.XY, Alu.add)
    t2 = pool.tile([P, NBLK * 48], dt.float32, tag="t2d")
    G.tensor_tensor(t2[:], rw_sh[:], rw_sh[:], Alu.mult)
    G.tensor_tensor(t2[:], t2[:], ds[:], Alu.mult)
    V.tensor_reduce(accs["p2"][:], _blk(t2[:], 48), AX.XY, Alu.add)
    yield

    pdt = pool.tile([P, NBLK * 3], dt.float32, tag="pdt")
    gtt = pool.tile([P, NBLK * 3], dt.float32, tag="gtt")
    nc.sync.dma_start(_blk(pdt[:], 3), aps["pd"].rearrange("(b p) c -> p b c", p=P))
    nc.sync.dma_start(_blk(gtt[:], 3), aps["gt"].rearrange("(b p) c -> p b c", p=P))
    d = pool.tile([P, NBLK * 3], dt.float32, tag="rgbd")
    V.tensor_tensor(d[:], pdt[:], gtt[:], Alu.subtract)
    V.tensor_tensor(d[:], d[:], d[:], Alu.mult)
    V.tensor_reduce(accs["rgb"][:], d[:], AX.X, Alu.add)
    yield


def _emit_hash(nc, pool, lvl, ones_h, acc, aps, first):
    E = nc.gpsimd if lvl == 0 else nc.vector
    idx = pool.tile([P, HCOLS], dt.int32, tag="hidx")
    src = aps[f"hi{lvl}"]
    nc.sync.dma_start(idx[:], bass.AP(tensor=src.tensor, offset=src.offset,
                                      ap=[[HROW, P], [1, HCOLS]]))
    emb = pool.tile([P, HCOLS * 2], dt.float32, tag="hemb")
    esrc = aps[f"he{lvl}"]
    nc.sync.dma_start(emb[:], bass.AP(tensor=esrc.tensor, offset=esrc.offset,
                                      ap=[[HROW * 2, P], [1, HCOLS * 2]]))
    sq = pool.tile([P, HCOLS * 2], dt.float32, tag="hsq")
    E.tensor_tensor(sq[:], emb[:], emb[:], Alu.mult)
    wv = pool.tile([P, HCOLS], dt.float32, tag="hw")
    sq3 = sq[:].rearrange("p (n two) -> p n two", two=2)
    E.tensor_tensor(wv[:], sq3[:, :, 0], sq3[:, :, 1], Alu.add)
    eq = pool.tile([P, HCOLS], dt.float32, tag="heq")
    nc.gpsimd.memset(eq[:, 0:1], 0.0)
    nc.vector.tensor_tensor(eq[:, 1:HCOLS], idx[:, 1:HCOLS], idx[:, 0:HCOLS - 1],
                             Alu.is_equal)
    yield
    S = pool.tile([P, HCOLS], dt.float32, tag="hS")
    nc.vector.tensor_tensor_scan(S[:], eq[:], wv[:], 0.0, Alu.mult, Alu.add)
    cc = pool.tile([P, HCOLS], dt.float32, tag="hcc")
    nc.vector.tensor_tensor_scan(cc[:], eq[:], ones_h, 0.0, Alu.mult, Alu.add)
    yield
    ratio = pool.tile([P, HCOLS], dt.float32, tag="hr")
    nc.vector.reciprocal(cc[:], cc[:])
    E.tensor_tensor(ratio[:], S[:], cc[:], Alu.mult)
    me = pool.tile([P, HCOLS], dt.float32, tag="hme")
    nc.scalar.activation(me[:, 0:HCOLS - 1], eq[:, 1:HCOLS], Act.Copy,
                         bias=1.0, scale=-1.0)
    E.tensor_tensor(ratio[:, HALO:HALO + HROW], ratio[:, HALO:HALO + HROW],
                    me[:, HALO:HALO + HROW], Alu.mult)
    if first:
        nc.vector.tensor_reduce(acc[:], ratio[:, HALO:HALO + HROW], AX.X, Alu.add)
    else:
        part = pool.tile([P, 1], dt.float32, tag="hpart")
        nc.vector.tensor_reduce(part[:], ratio[:, HALO:HALO + HROW], AX.X,
                                Alu.add)
        E.tensor_tensor(acc[:], acc[:], part[:], Alu.add)
    yield


def build_module(parts=("rgb", "dist", "hash", "l0", "l1")):
    nc = bacc.Bacc("TRN2", target_bir_lowering=False, debug=False,
                   enable_asserts=False, num_devices=N_CORES)
    aps = {}

    def din(name, shape, dtype=dt.float32):
        aps[name] = nc.dram_tensor(name, shape, dtype, kind="ExternalInput").ap()
    din("pd", [RPC, 3]); din("gt", [RPC, 3])
    din("sd", [RPC, 49]); din("rw", [RPC, 48])
    din("ps0", [RPC, 257]); din("pw0", [RPC, 256])
    din("ps1", [RPC, 97]); din("pw1", [RPC, 96])
    din("hi0", [HSLICE], dt.int32); din("he0", [HSLICE * 2])
    din("hi1", [HSLICE], dt.int32); din("he1", [HSLICE * 2])
    for lvl, L in LVL.items():
        NL = NB * L["EW"]
        din(f"c_u16_l{lvl}", [P, 2 * NL], dt.int16)
        din(f"c_maskf_l{lvl}", [P, (3 if lvl == 1 else 2) * NL])
    din("c_mask48", [P, NBLK * 48]); din("c_ones", [P, HCOLS])
    out_ap = nc.dram_tensor("out", [1, 1], dt.float32, kind="ExternalOutput").ap()

    with tile.TileContext(nc) as tc:
        _emit(nc, tc, aps, out_ap, parts)
    nc.compile()
    return nc


def _emit(nc, tc, aps, out_ap, parts=("rgb", "dist", "hash", "l0", "l1")):
    import contextlib
    V, G = nc.vector, nc.gpsimd
    with contextlib.ExitStack() as ctx:
        spool = ctx.enter_context(tc.tile_pool(name="shared", bufs=1))
        s_sh = spool.tile([P, NBLK * 49], dt.float32, tag="s_sh")
        nc.sync.dma_start(_blk(s_sh[:], 49),
                          aps["sd"].rearrange("(b p) x -> p b x", p=P))
        radios = {l: spool.tile([P, NBLK * 49], dt.float32, tag=f"radio{l}",
                                name=f"radio{l}")
                  for l in (0, 1)}

        cpool = ctx.enter_context(tc.tile_pool(name="consts", bufs=1))
        mask48 = cpool.tile([P, NBLK * 48], dt.float32, tag="mask48")
        ones_h = cpool.tile([P, HCOLS], dt.float32, tag="ones_h")
        lvl_consts = {}
        cdma = []
        cdma.append((mask48[:], aps["c_mask48"]))
        cdma.append((ones_h[:], aps["c_ones"]))
        for lvl, L in LVL.items():
            NL = NB * L["EW"]
            cu = cpool.tile([P, 2 * NL], dt.int16, tag=f"cu16_{lvl}",
                            name=f"cu16_{lvl}")
            mf = cpool.tile([P, (3 if lvl == 1 else 2) * NL], dt.float32,
                            tag=f"maskf_{lvl}", name=f"maskf_{lvl}")
            cdma.append((cu[:], aps[f"c_u16_l{lvl}"]))
            cdma.append((mf[:], aps[f"c_maskf_l{lvl}"]))
            cuv = cu[:].bitcast(dt.uint16)
            ioq2 = (mf[:][:, 2 * NL:3 * NL] if lvl == 1
                    else cuv[:, NL:2 * NL])
            lvl_consts[lvl] = (mf[:][:, 0:NL], mf[:][:, NL:2 * NL],
                               None, cu[:][:, 0:NL], ioq2)
            # (maskf, mask_cnt(f32), unused, ioG(i16), ioQ2)

        def _emit_consts():
            for dst, src_ap in cdma:
                nc.sync.dma_start(dst, src_ap)
            yield

        accs = {}
        for name in ("rgb", "p1", "p2", "hash", "l0a", "l0b", "l1a", "l1b"):
            accs[name] = cpool.tile([P, 1], dt.float32, tag=f"acc_{name}",
                                    name=f"acc_{name}")
            V.memset(accs[name][:], 0.0)

        spool = ctx.enter_context(tc.tile_pool(name="shared", bufs=1))
        s_sh = spool.tile([P, NBLK * 49], dt.float32, tag="s_sh")
        nc.sync.dma_start(_blk(s_sh[:], 49),
                          aps["sd"].rearrange("(b p) x -> p b x", p=P))
        radios = {l: spool.tile([P, NBLK * 49], dt.float32, tag=f"radio{l}",
                                name=f"radio{l}")
                  for l in (0, 1)}

        MRG = {0: dict(ME=V, ME2=V, EE=V),
               1: dict(ME=V, ME2=V, EE=V)}
        HEM = {
            "l0a": dict(SE=V, SE2=V, XE=V, EE=V, TE=V, FE=V),
            "l0b": dict(SE=V, SE2=V, XE=V, EE=V, TE=V, FE=G),
            "l1a": dict(SE=V, SE2=V, XE=V, EE=G, TE=G, FE=G, fchain=True),
            "l1b": dict(SE=V, SE2=V, XE=V, EE=G, TE=G, FE=G, fchain=True),
        }

        gens = []
        setup_pool = ctx.enter_context(tc.tile_pool(name="setup", bufs=1))
        gens.append(_emit_setup(nc, setup_pool, s_sh, radios, accs, mask48[:],
                                aps))
        mouts = {}
        for lvl in (0, 1):
            if f"l{lvl}" not in parts:
                continue
            mouts[lvl] = {}
            mp = ctx.enter_context(tc.tile_pool(name=f"mrg{lvl}", bufs=1))
            gens.append(_emit_level_merge(nc, tc, mp, lvl, s_sh,
                                          aps[f"ps{lvl}"], aps[f"pw{lvl}"],
                                          mouts[lvl], MRG[lvl]))
        gens.append(_emit_consts())
        if "hash" in parts:
            for lvl in (0, 1):
                hp2 = ctx.enter_context(tc.tile_pool(name=f"hash{lvl}", bufs=1))
                gens.append(_emit_hash(nc, hp2, lvl, ones_h[:], accs["hash"],
                                       aps, first=(lvl == 0)))
        for name, lvl, b0 in HALVES:
            if f"l{lvl}" not in parts:
                continue
            hp = ctx.enter_context(tc.tile_pool(name=name, bufs=1))
            gens.append(_emit_half(nc, hp, lvl, b0, s_sh, radios[lvl],
                                   mouts[lvl], lvl_consts[lvl], accs[name],
                                   HEM[name]))

        while gens:
            nxt = []
            for g in gens:
                try:
                    next(g)
                    nxt.append(g)
                except StopIteration:
                    pass
            gens = nxt

        with tc.tile_pool(name="fin", bufs=1) as pool:
            tot = pool.tile([P, 1], dt.float32, tag="tot")
            V.tensor_scalar(tot[:], accs["rgb"][:], W_RGB / (R * 3), None,
                            Alu.mult)
            for snm, lvl, _ in HALVES:
                V.scalar_tensor_tensor(tot[:], accs[snm][:],
                                       W_INTER / (R * (LVL[lvl]["X"] - 1)),
                                       tot[:], Alu.mult, Alu.add)
            V.scalar_tensor_tensor(tot[:], accs["p1"][:], W_DIST * 2.0 / R,
                                   tot[:], Alu.mult, Alu.add)
            V.scalar_tensor_tensor(tot[:], accs["p2"][:], W_DIST / (3.0 * R),
                                   tot[:], Alu.mult, Alu.add)
            V.scalar_tensor_tensor(tot[:], accs["hash"][:],
                                   W_HASH / (NUM_SEGMENTS * 2.0), tot[:],
                                   Alu.mult, Alu.add)
            res = pool.tile([1, 1], dt.float32, tag="res")
            G.tensor_reduce(res[:], tot[:], AX.C, Alu.add)
            nc.sync.dma_start(out_ap, res[:])


# ---------------- host side ----------------
_module_cache = {}


def _get_module():
    if "nc" not in _module_cache:
        _module_cache["nc"] = build_module()
    return _module_cache["nc"]


def shard_inputs(inputs):
    f32 = np.float32
    pd = np.ascontiguousarray(inputs["pd_rgbs"], f32)
    gt = np.ascontiguousarray(inputs["gt_rgbs"], f32)
    sd = np.ascontiguousarray(inputs["render_sdist"], f32)
    rw = np.ascontiguousarray(inputs["render_weights"], f32)
    ps0 = np.ascontiguousarray(inputs["prop_sdist_0"], f32)
    pw0 = np.ascontiguousarray(inputs["prop_weights_0"], f32)
    ps1 = np.ascontiguousarray(inputs["prop_sdist_1"], f32)
    pw1 = np.ascontiguousarray(inputs["prop_weights_1"], f32)
    hashes = {}
    for lvl in (0, 1):
        idx = np.asarray(inputs[f"enc_idx_{lvl}"]).astype(np.int32)
        emb = np.ascontiguousarray(inputs[f"enc_embds_{lvl}"], f32)
        idx_pad = np.full(M + 2 * HALO, -1, np.int32)
        idx_pad[HALO:HALO + M] = idx
        emb_pad = np.zeros((M + 2 * HALO, 2), f32)
        emb_pad[HALO:HALO + M] = emb
        hashes[lvl] = (idx_pad, emb_pad)

    consts = {}
    rep = lambda row: np.ascontiguousarray(np.tile(row, (P, 1)))
    for lvl, L in LVL.items():
        EW, QWS, X = L["EW"], L["QWS"], L["X"]
        NL = NB * EW
        NQ = NB * QWS
        ioG = np.concatenate([2 * np.arange(b * EW, (b + 1) * EW,
                                            dtype=np.uint16)
                              for b in range(NB)])
        # query dest: rank-1 + b*QWS; C' = C + 98b so fold +98b here:
        # ioQ2 = i+1 + b*QWS + 98b -> (ioQ2 - C')*qf - 1 = rank-1 + b*QWS
        ioQ2 = np.concatenate([np.arange(1, EW + 1, dtype=np.uint16)
                               + b * QWS + 98 * b for b in range(NB)])
        packed = np.concatenate([ioG, ioQ2]).astype(np.uint16)
        consts[f"c_u16_l{lvl}"] = rep(packed.view(np.int16))
        msk = np.ones(NL, f32)
        msk[::EW] = 0.0
        mcnt = np.ones(NL, f32)
        for b in range(NB):
            mcnt[b * EW] = b
        parts_ = [msk, mcnt]
        if lvl == 1:
            parts_.append(np.concatenate([np.arange(1, EW + 1, dtype=f32)
                                          + b * QWS + 98 * b
                                          for b in range(NB)]))
        consts[f"c_maskf_l{lvl}"] = rep(np.concatenate(parts_))
    m48 = np.ones(NBLK * 48, f32)
    m48[::48] = 0.0
    consts["c_mask48"] = rep(m48)
    consts["c_ones"] = rep(np.ones(HCOLS, f32))

    in_maps = []
    for c in range(N_CORES):
        r0 = c * RPC
        lo = c * MPC
        im = {
            "pd": pd[r0:r0 + RPC], "gt": gt[r0:r0 + RPC],
            "sd": sd[r0:r0 + RPC], "rw": rw[r0:r0 + RPC],
            "ps0": ps0[r0:r0 + RPC], "pw0": pw0[r0:r0 + RPC],
            "ps1": ps1[r0:r0 + RPC], "pw1": pw1[r0:r0 + RPC],
        }
        for lvl in (0, 1):
            idx_pad, emb_pad = hashes[lvl]
            im[f"hi{lvl}"] = np.ascontiguousarray(idx_pad[lo:lo + HSLICE])
            im[f"he{lvl}"] = np.ascontiguousarray(
                emb_pad[lo:lo + HSLICE].reshape(-1))
        im.update(consts)
        in_maps.append(im)
    return in_maps


def kernel(**inputs) -> np.ndarray:
    nc = _get_module()
    in_maps = shard_inputs(inputs)
    res = run_bass_kernel_spmd(nc, in_maps, core_ids=list(range(N_CORES)))
    total = np.float64(0.0)
    for r in res.results:
        total += np.float64(r["out"][0, 0])
    return np.float32(total)


# revision 8
# speedup vs baseline: 1.7238x; 1.0404x over previous
"""Trainium2 Bass kernel v2 for nn_Loss_dict_50646254354805 (NeRF-style loss).

v2 vs baseline:
- bitonic merges on uint16 quantized keys (value*15000 + 2 tag bits) -> DVE
  2x perf mode; keys determine ORDER only.
- exact f32 values (queries/em/ep) and radio reach the merged domain via
  batched u16-half local_scatters through one shared index table (idxcat):
  merged positions come from the C/Cm count scans.
- one merge per level; post-merge work split into two 2-block half-streams
  with per-stream engine maps; all generators emitted stage-interleaved so
  DVE / Pool / Act overlap.
"""
import numpy as np

import concourse.bass as bass
import concourse.mybir as mybir
import concourse.tile as tile
from concourse import bacc
from concourse.bass_utils import run_bass_kernel_spmd

dt = mybir.dt
Alu = mybir.AluOpType
AX = mybir.AxisListType
Act = mybir.ActivationFunctionType
P = 128

PULSE = (0.01, 0.005)
W_RGB, W_INTER, W_DIST, W_HASH = 1.0, 1.0, 0.01, 0.1
NUM_SEGMENTS = 65536
R, N = 4096, 48
M = R * N
N_CORES = 8
RPC = R // N_CORES
NBLK = RPC // P               # 4 ray blocks per core
MPC = M // N_CORES
HALO = 64
HROW = MPC // P
HCOLS = HROW + HALO + 1
HSLICE = HALO + MPC + HALO

VOFF = 0.97
QS = 15000.0                  # key quantization scale
PADK = 0xFFFC

LVL = {0: dict(X=257, n2=512), 1: dict(X=97, n2=256)}
for _L in LVL.values():
    _L["EW"] = ((_L["X"] + 98 + 1 + 7) // 8) * 8        # 360 / 200
    _L["QWS"] = _L["EW"] - 98                           # 262 / 102

NB = 2                        # blocks per half-stream
HALVES = [("l0a", 0, 0), ("l0b", 0, 2), ("l1a", 1, 0), ("l1b", 1, 2)]


def _blk(ap, n):
    return ap.rearrange("p (b n) -> p b n", n=n)


def _ts_int(eng, out, in0, imm1, op0, imm2=None, op1=None):
    ins_ = [eng.lower_ap(in0), mybir.ImmediateValue(dtype=dt.int32, value=int(imm1))]
    kw = dict(op0=op0)
    if imm2 is not None:
        ins_.append(mybir.ImmediateValue(dtype=dt.int32, value=int(imm2)))
        kw["op1"] = op1
    return eng.add_instruction(mybir.InstTensorScalarPtr(
        name=eng.bass.get_next_instruction_name(),
        ins=ins_, outs=[eng.lower_ap(out)], **kw))

BIGF = 3.0


def _merge_gen(eng, bufa, bufb, width, out, ew=None, trim4d=True):
    """Ascending bitonic merge over [P, NBLK*width] u16 ping-pong tiles.
    Generator: yields after each stage so two levels' merges interleave in
    the engine queue. Result tile is appended to `out`.

    If ew is given, only outputs [0, ew+2d-1] of each block are needed
    downstream, so late stages skip whole 2d-chunks beyond that window."""
    cur, nxt = bufa, bufb
    d = width // 2
    while d >= 1:
        nch = width // (2 * d)
        keep = nch
        if ew is not None and trim4d:
            keep = min(nch, -(-(ew + 2 * d - 1) // (2 * d)))
        if keep == nch:
            c3 = cur[:].rearrange("p (c td) -> p c td", td=2 * d)
            n3 = nxt[:].rearrange("p (c td) -> p c td", td=2 * d)
        else:
            c3 = cur[:].rearrange("p (b c td) -> p (b c) td",
                                  td=2 * d, c=nch)[: , 0:0]  # placeholder
        if keep == nch:
            lo_in, hi_in = c3[:, :, 0:d], c3[:, :, d:2 * d]
            eng.tensor_tensor(n3[:, :, 0:d], lo_in, hi_in, Alu.min)
            eng.tensor_tensor(n3[:, :, d:2 * d], lo_in, hi_in, Alu.max)
        else:
            c4 = cur[:].rearrange("p (b c td) -> p b c td", td=2 * d, c=nch)
            n4 = nxt[:].rearrange("p (b c td) -> p b c td", td=2 * d, c=nch)
            lo_in = c4[:, :, 0:keep, 0:d]
            hi_in = c4[:, :, 0:keep, d:2 * d]
            eng.tensor_tensor(n4[:, :, 0:keep, 0:d], lo_in, hi_in, Alu.min)
            eng.tensor_tensor(n4[:, :, 0:keep, d:2 * d], lo_in, hi_in, Alu.max)
        cur, nxt = nxt, cur
        d //= 2
        if d >= 1:
            yield
    out.append(cur)


def _emit_level_merge(nc, tc, pool, lvl, s_sh, x_ap, pwt_ap, out, eng):
    """Generator: quantize + b1/b2 merges for all 4 blocks of one level.

    lvl 0: uint16 quantized keys, merged on DVE (2x mode).
    lvl 1: f32-bitcast tagged keys (baseline-style), merged on Pool where
    f32 min/max is legal -- frees DVE during the big level-0 merge."""
    ME, ME2, EE = eng["ME"], eng["ME2"], eng["EE"]
    AE = nc.scalar
    fkeys = eng.get("fkeys", False)
    L = LVL[lvl]
    X, n2 = L["X"], L["n2"]
    pw = PULSE[lvl]
    kdt = dt.float32 if fkeys else dt.uint16

    xt = pool.tile([P, NBLK * X], dt.float32, tag="xt")
    nc.sync.dma_start(_blk(xt[:], X), x_ap.rearrange("(b p) x -> p b x", p=P))
    pwt = pool.tile([P, NBLK * (X - 1)], dt.float32, tag="pwt")
    nc.sync.dma_start(_blk(pwt[:], X - 1),
                      pwt_ap.rearrange("(b p) x -> p b x", p=P))
    out["xt"] = xt
    out["pwt"] = pwt

    b2a = pool.tile([P, NBLK * n2], kdt, tag="b2a")
    b2b = pool.tile([P, NBLK * n2], kdt, tag="b2b")
    b2a3 = _blk(b2a[:], n2)
    b1a = pool.tile([P, NBLK * 128], kdt, tag="b1a")
    b1b = pool.tile([P, NBLK * 128], kdt, tag="b1b")
    b1a3 = _blk(b1a[:], 128)
    if fkeys:
        nc.gpsimd.memset(b1a[:], BIGF)
        emsh = pool.tile([P, NBLK * 49], dt.float32, tag="emsh")
        AE.activation(emsh[:], s_sh[:], Act.Copy, bias=1.0 - pw)
        epsh = pool.tile([P, NBLK * 49], dt.float32, tag="epsh")
        AE.activation(epsh[:], s_sh[:], Act.Copy, bias=1.0 + pw)
        _ts_int(EE, b1a3[:, :, 0:49].bitcast(dt.int32),
                _blk(emsh[:], 49).bitcast(dt.int32), ~3, Alu.bitwise_and,
                1, Alu.bitwise_or)
        _ts_int(EE, b1a3[:, :, 79:128][:, :, ::-1].bitcast(dt.int32),
                _blk(epsh[:], 49).bitcast(dt.int32), ~3, Alu.bitwise_and,
                3, Alu.bitwise_or)
    else:
        nc.gpsimd.memset(b1a[:], PADK)
        emq = pool.tile([P, NBLK * 49], dt.uint16, tag="emq")
        EE.tensor_scalar(emq[:], s_sh[:], QS, (1.0 - pw - VOFF) * QS + 0.5,
                         Alu.mult, Alu.add)
        epq = pool.tile([P, NBLK * 49], dt.uint16, tag="epq")
        EE.tensor_scalar(epq[:], s_sh[:], QS, (1.0 + pw - VOFF) * QS + 0.5,
                         Alu.mult, Alu.add)
        EE.tensor_scalar(b1a3[:, :, 0:49], _blk(emq[:], 49), 4, 1,
                         Alu.mult, Alu.add)
        EE.tensor_scalar(b1a3[:, :, 79:128][:, :, ::-1], _blk(epq[:], 49), 4, 3,
                         Alu.mult, Alu.add)
    yield
    _r1 = []
    yield from _merge_gen(ME, b1a, b1b, 128, _r1, ew=98, trim4d=not fkeys)
    b1 = _r1[0]
    yield
    if fkeys:
        nc.gpsimd.memset(b2a3[:, :, X:n2 - 128], BIGF)
        xsh = pool.tile([P, NBLK * X], dt.float32, tag="xsh")
        AE.activation(xsh[:], xt[:], Act.Copy, bias=1.0)
        _ts_int(EE, b2a3[:, :, 0:X].bitcast(dt.int32),
                _blk(xsh[:], X).bitcast(dt.int32), ~3, Alu.bitwise_and)
        EE.tensor_copy(b2a3[:, :, n2 - 128:n2][:, :, ::-1], _blk(b1[:], 128))
    else:
        nc.gpsimd.memset(b2a3[:, :, X:n2 - 128], PADK)
        xq = pool.tile([P, NBLK * X], dt.uint16, tag="xq")
        EE.tensor_scalar(xq[:], xt[:], QS, (1.0 - VOFF) * QS + 0.5,
                         Alu.mult, Alu.add)
        EE.tensor_scalar(b2a3[:, :, 0:X], _blk(xq[:], X), 4, None, Alu.mult)
        EE.tensor_copy(b2a3[:, :, n2 - 128:n2][:, :, ::-1], _blk(b1[:], 128))
    yield
    _r2 = []
    yield from _merge_gen(ME2, b2a, b2b, n2, _r2, ew=L["EW"],
                          trim4d=not fkeys)
    out["m"] = _r2[0]
    yield


def _emit_half(nc, pool, lvl, b0, s_sh, radio_full, mout, consts, acc, eng):
    """Generator: post-merge pipeline for blocks [b0, b0+NB) of one level."""
    SE, XE, EE, FE = (eng[k] for k in ("SE", "XE", "EE", "FE"))
    TE = eng.get("TE", EE)
    SE2 = eng.get("SE2", SE)
    fkeys = eng.get("fkeys", False)
    fchain = eng.get("fchain", False)
    mdt = dt.float32 if (fkeys or fchain) else dt.uint16
    AE = nc.scalar
    L = LVL[lvl]
    X, n2, EW, QWS = L["X"], L["n2"], L["EW"], L["QWS"]
    NL = NB * EW
    NQ = NB * QWS
    NE = NB * 49
    VW = NQ + 2 * NE          # vcat width: [x | em | ep]
    pw = PULSE[lvl]
    maskf, mask_cnt, io49p, ioG, ioQ2 = consts

    def blkE(ap):
        return ap.rearrange("p (b n) -> p b n", b=NB)

    ss = s_sh[:][:, b0 * 49:(b0 + NB) * 49]

    # ---------- sources: exact values + radio (independent of merge) ----------
    vcat = pool.tile([P, VW], dt.float32, tag="vcat")
    nc.gpsimd.memset(_blk(vcat[:, 0:NQ], QWS)[:, :, X:QWS], 0.0)
    radcat = pool.tile([P, 2 * NE], dt.float32, tag="radcat")
    rsl_ = radio_full[:][:, b0 * 49:(b0 + NB) * 49]
    FE.tensor_copy(radcat[:, 0:NE], rsl_)
    FE.tensor_scalar(radcat[:, NE:2 * NE], radcat[:, 0:NE], -1.0, None, Alu.mult)
    yield
    # wait for merge result
    while "m" not in mout:
        yield
    m = mout["m"]
    xt, pwt_full = mout["xt"], mout["pwt"]
    mSh = _blk(m[:], n2)[:, b0:b0 + NB, 0:EW]       # [P, NB, EW] strided
    xts = _blk(xt[:], X)[:, b0:b0 + NB]             # [P, NB, X]
    pwt = _blk(pwt_full[:], X - 1)[:, b0:b0 + NB]
    AE.activation(_blk(vcat[:, 0:NQ], QWS)[:, :, 0:X], xts, Act.Copy)
    AE.activation(_blk(vcat[:, NQ:NQ + NE], 49), _blk(ss, 49), Act.Copy, bias=-pw)
    AE.activation(_blk(vcat[:, NQ + NE:VW], 49), _blk(ss, 49), Act.Copy, bias=pw)
    yield

    # ---------- tags + counts ----------
    if fkeys:
        tag32 = pool.tile([P, NL], dt.int32, tag="tag32")
        _ts_int(XE, blkE(tag32[:]), mSh.bitcast(dt.int32), 3, Alu.bitwise_and)
        ev_f = pool.tile([P, NL], dt.float32, tag="ev_f")
        em_f = pool.tile([P, NL], dt.float32, tag="em_f")
        ep_f = pool.tile([P, NL], dt.float32, tag="ep_f")
        _ts_int(TE, em_f[:], tag32[:], 1, Alu.is_equal)
        _ts_int(TE, ep_f[:], tag32[:], 3, Alu.is_equal)
        FE.tensor_tensor(ev_f[:], em_f[:], ep_f[:], Alu.add)
    elif fchain:
        tagb_t = pool.tile([P, NL], dt.uint16, tag="tagb")
        tagb = tagb_t[:]
        XE.tensor_scalar(blkE(tagb), mSh, 3, None, Alu.bitwise_and)
        em_f = pool.tile([P, NL], dt.float32, tag="em_f")
        TE.tensor_scalar(em_f[:], tagb, 1, None, Alu.is_equal)
        ep_f = pool.tile([P, NL], dt.float32, tag="ep_f")
        TE.tensor_scalar(ep_f[:], tagb, 3, None, Alu.is_equal)
        ev_f = pool.tile([P, NL], dt.float32, tag="ev_f")
        FE.tensor_tensor(ev_f[:], em_f[:], ep_f[:], Alu.add)
    else:
        tagb_t = pool.tile([P, NL], dt.uint16, tag="tagb")
        tagb = tagb_t[:]
        XE.tensor_scalar(blkE(tagb), mSh, 3, None, Alu.bitwise_and)
        ev_f = pool.tile([P, NL], dt.uint16, tag="ev_f")
        TE.tensor_scalar(ev_f[:], tagb, 1, None, Alu.bitwise_and)
        em_f = pool.tile([P, NL], dt.uint16, tag="em_f")
        TE.tensor_scalar(em_f[:], tagb, 1, None, Alu.is_equal)
        ep_f = pool.tile([P, NL], dt.uint16, tag="ep_f")
        TE.tensor_scalar(ep_f[:], tagb, 3, None, Alu.is_equal)
    yield
    C = pool.tile([P, NL], mdt, tag="C")
    SE.tensor_tensor_scan(C[:], mask_cnt, ev_f[:], 0.0, Alu.mult, Alu.add)
    Cm = pool.tile([P, NL], mdt, tag="Cm")
    SE.tensor_tensor_scan(Cm[:], mask_cnt, em_f[:], 0.0, Alu.mult, Alu.add)
    yield

    # ---------- idxcat: merged position of every source element ----------
    t2 = pool.tile([P, NL], mdt, tag="t2")
    t3 = pool.tile([P, NL], mdt, tag="t3")
    if fkeys:
        t1 = tag32                                  # dead after masks
    elif fchain:
        t1 = pool.tile([P, NL], dt.float32, tag="t1f")
    else:
        t1 = tagb_t                                 # dead after masks
    # block offsets (49b/98b) ride in from the mask_cnt scan carry; section
    # offsets NQ / NQ+NE are flat immediates. One combined scatter:
    # t1 = (Cm'+NQ)*em + (C'-Cm'+NQ+NE)*ep + (ioQ2-C')*qf - 1
    EE.tensor_tensor(t2[:], C[:], Cm[:], Alu.subtract)
    EE.tensor_scalar(t2[:], t2[:], NQ + NE, None, Alu.add)
    EE.tensor_tensor(t2[:], t2[:], ep_f[:], Alu.mult)
    t1v = t1[:].bitcast(dt.float32) if fkeys else t1[:]
    ffull = fkeys or fchain
    EE.tensor_scalar(t1v, Cm[:], NQ, None, Alu.add)
    EE.tensor_tensor(t1v, t1v, em_f[:], Alu.mult)
    EE.tensor_tensor(t1v, t1v, t2[:], Alu.add)
    qf = em_f                                       # em_f dead after t1
    TE2 = FE if ffull else TE
    TE2.tensor_scalar(qf[:], ev_f[:], 0, None, Alu.is_equal)
    EE.tensor_tensor(t3[:], ioQ2, C[:], Alu.subtract)
    EE.tensor_tensor(t3[:], t3[:], qf[:], Alu.mult)
    EE.tensor_tensor(t1v, t1v, t3[:], Alu.add)
    EE.tensor_scalar(t1v, t1v, 1, None, Alu.subtract)       # idx all
    EE.tensor_scalar(t2[:], t3[:], 1, None, Alu.subtract)       # idxq
    if ffull:
        t1s = ev_f[:].bitcast(dt.int16)[:, 0:NL]    # ev_f dead after qf
        AE.activation(t1s, t1v, Act.Copy)
        t2s = ep_f[:].bitcast(dt.int16)[:, 0:NL]    # ep_f dead after t2
        AE.activation(t2s, t2[:], Act.Copy)
    else:
        t1s = t1[:].bitcast(dt.int16)
        t2s = t2[:].bitcast(dt.int16)
    yield
    idxcat = pool.tile([P, VW], dt.uint16, tag="idxcat")
    nc.gpsimd.local_scatter(idxcat[:].bitcast(dt.int16), ioG,
                            t1s, channels=P,
                            num_elems=VW, num_idxs=NL)
    idx2 = pool.tile([P, 2 * VW], dt.uint16, tag="idx2")
    i2v = idx2[:].rearrange("p (n two) -> p n two", two=2)
    AE.activation(i2v[:, :, 0], idxcat[:], Act.Copy)
    AE.activation(i2v[:, :, 1], idxcat[:], Act.Copy, bias=1.0)
    yield

    # ---------- pair-scatter exact values + radio into merged domain ----------
    v = pool.tile([P, NL], dt.float32, tag="v")
    nc.gpsimd.local_scatter(v[:].bitcast(dt.int16),
                            vcat[:].bitcast(dt.int16),
                            idx2[:].bitcast(dt.int16), channels=P,
                            num_elems=2 * NL, num_idxs=2 * VW)
    F1 = pool.tile([P, NL], dt.float32, tag="F1")   # radio_m
    nc.gpsimd.local_scatter(F1[:].bitcast(dt.int16),
                            radcat[:].bitcast(dt.int16),
                            idx2[:, 2 * NQ:2 * VW].bitcast(dt.int16), channels=P,
                            num_elems=2 * NL, num_idxs=4 * NE)
    yield

    # ---------- density reconstruction ----------
    F2 = pool.tile([P, NL], dt.float32, tag="F2")
    SE2.tensor_tensor_scan(F2[:], maskf, F1[:], 0.0, Alu.mult, Alu.add)  # g
    if ffull:
        dv = t3                                     # t3 dead after idx phase
    else:
        dv = pool.tile([P, NL], dt.float32, tag="dv")
    dv3 = blkE(dv[:])
    v3 = blkE(v[:])
    nc.gpsimd.memset(dv3[:, :, 0:1], 0.0)
    FE.tensor_tensor(dv3[:, :, 1:EW], v3[:, :, 1:EW], v3[:, :, 0:EW - 1],
                     Alu.subtract)
    yield
    wg = v                                          # v dead after dv
    wg3 = blkE(wg[:])
    nc.gpsimd.memset(wg3[:, :, 0:1], 0.0)
    FE.tensor_tensor(wg3[:, :, 1:EW], dv3[:, :, 1:EW],
                     blkE(F2[:])[:, :, 0:EW - 1], Alu.mult)
    w_t = F1                                        # radio dead after g
    SE2.tensor_tensor_scan(w_t[:], maskf, wg[:], 0.0, Alu.mult, Alu.add)
    yield
    wc = wg                                         # wg dead
    AE.activation(wc[:], w_t[:], Act.Relu, scale=0.5)
    scr = pool.tile([P, NL], dt.float32, tag="scr")
    wc3 = blkE(wc[:])
    s3_ = blkE(scr[:])
    nc.gpsimd.memset(s3_[:, :, 0:1], 0.0)
    FE.tensor_tensor(s3_[:, :, 1:EW], wc3[:, :, 1:EW], wc3[:, :, 0:EW - 1],
                     Alu.add)
    area = w_t                                      # w dead after wc
    a3 = blkE(area[:])
    nc.gpsimd.memset(a3[:, :, 0:1], 0.0)
    FE.tensor_tensor(a3[:, :, 1:EW], s3_[:, :, 1:EW],
                     dv3[:, :, 1:EW], Alu.mult)
    cdf = F2                                        # g dead after wg
    SE2.tensor_tensor_scan(cdf[:], maskf, area[:], 0.0, Alu.mult, Alu.add)
    yield

    # ---------- compact cdf at query slots (pair-scatter) ----------
    idx2q = idx2                                    # idx2 dead after scatters
    i2qv = idx2q[:][:, 0:2 * NL].rearrange("p (n two) -> p n two", two=2)
    tqu = Cm[:].bitcast(dt.uint16)[:, 0:NL]         # Cm dead after t1
    if ffull:
        AE.activation(tqu.bitcast(dt.int16), t2[:], Act.Copy, scale=2.0)
    else:
        EE.tensor_scalar(tqu, t2[:], 2, None, Alu.mult)
    AE.activation(i2qv[:, :, 0], tqu, Act.Copy)
    AE.activation(i2qv[:, :, 1], tqu, Act.Copy, bias=1.0)
    cdfq = vcat                                     # vcat dead after v scatter
    cqn = cdfq[:][:, 0:NQ]
    nc.gpsimd.local_scatter(cqn.bitcast(dt.int16),
                            cdf[:].bitcast(dt.int16),
                            idx2q[:][:, 0:2 * NL].bitcast(dt.int16), channels=P,
                            num_elems=2 * NQ, num_idxs=2 * NL)
    del t2s
    yield

    # ---------- loss tail ----------
    NW = NB * (X - 1)
    ws = scr                                        # dead after area
    ws2 = ws[:][:, 0:NW]
    cqf = _blk(cdfq[:][:, 0:NQ], QWS)
    FE.tensor_tensor(_blk(ws2, X - 1), cqf[:, :, 1:X], cqf[:, :, 0:X - 1],
                     Alu.subtract)
    FE.tensor_tensor(_blk(ws2, X - 1), _blk(ws2, X - 1), pwt, Alu.subtract)
    den = area                                      # dead after cdf
    den2 = den[:][:, 0:NW]
    AE.activation(_blk(den2, X - 1), pwt, Act.Copy, bias=1e-5)
    nc.vector.reciprocal(den2, den2)
    rsl = dv                                        # dead after area
    AE.activation(rsl[:][:, 0:NW], ws2, Act.Relu)
    FE.tensor_tensor(ws2, ws2, rsl[:][:, 0:NW], Alu.mult)
    FE.tensor_tensor(ws2, ws2, den2, Alu.mult)
    nc.vector.tensor_reduce(acc[:], _blk(ws2, X - 1), AX.XY, Alu.add)
    yield


def _emit_setup(nc, pool, s_sh, radios, accs, mask48, aps):
    V, G = nc.vector, nc.gpsimd
    rw_sh = pool.tile([P, NBLK * 48], dt.float32, tag="rw_sh")
    nc.sync.dma_start(_blk(rw_sh[:], 48),
                      aps["rw"].rearrange("(b p) x -> p b x", p=P))
    s3 = _blk(s_sh[:], 49)
    ds = pool.tile([P, NBLK * 48], dt.float32, tag="ds")
    G.tensor_tensor(_blk(ds[:], 48), s3[:, :, 1:49], s3[:, :, 0:48], Alu.subtract)
    dse = pool.tile([P, NBLK * 48], dt.float32, tag="dse")
    nc.scalar.activation(dse[:], ds[:], Act.Copy, bias=1e-8)
    V.reciprocal(dse[:], dse[:])
    wnorm = pool.tile([P, NBLK * 48], dt.float32, tag="wnorm")
    G.tensor_tensor(wnorm[:], rw_sh[:], dse[:], Alu.mult)
    wnp = pool.tile([P, NBLK * 50], dt.float32, tag="wnp")
    G.memset(wnp[:], 0.0)
    V.tensor_copy(_blk(wnp[:], 50)[:, :, 1:49], _blk(wnorm[:], 48))
    diff = pool.tile([P, NBLK * 49], dt.float32, tag="diff")
    wnp3 = _blk(wnp[:], 50)
    G.tensor_tensor(_blk(diff[:], 49), wnp3[:, :, 1:50], wnp3[:, :, 0:49],
                    Alu.subtract)
    for lvl in (0, 1):
        G.tensor_scalar(radios[lvl][:], diff[:], 1.0 / (2 * PULSE[lvl]), None,
                        Alu.mult)
    yield

    mid = pool.tile([P, NBLK * 48], dt.float32, tag="mid")
    G.tensor_tensor(_blk(mid[:], 48), s3[:, :, 1:49], s3[:, :, 0:48], Alu.add)
    wm = pool.tile([P, NBLK * 48], dt.float32, tag="wm")
    V.scalar_tensor_tensor(wm[:], mid[:], 0.5, rw_sh[:], Alu.mult, Alu.mult)
    Cin = pool.tile([P, NBLK * 48], dt.float32, tag="Cin")
    V.tensor_tensor_scan(Cin[:], mask48, rw_sh[:], 0.0, Alu.mult, Alu.add)
    Sin = pool.tile([P, NBLK * 48], dt.float32, tag="Sin")
    V.tensor_tensor_scan(Sin[:], mask48, wm[:], 0.0, Alu.mult, Alu.add)
    yield
    A = pool.tile([P, NBLK * 47], dt.float32, tag="A47")
    m3 = _blk(mid[:], 48)
    c3 = _blk(Cin[:], 48)
    sw3 = _blk(Sin[:], 48)
    rw3 = _blk(rw_sh[:], 48)
    A3 = _blk(A[:], 47)
    V.scalar_tensor_tensor(A3, m3[:, :, 1:48], 0.5, c3[:, :, 0:47],
                           Alu.mult, Alu.mult)
    V.tensor_tensor(A3, A3, sw3[:, :, 0:47], Alu.subtract)
    V.tensor_tensor(A3, A3, rw3[:, :, 1:48], Alu.mult)
    V.tensor_reduce(accs["p1"][:], A3, AX.XY, Alu.add)
    t2 = pool.tile([P, NBLK * 48], dt.float32, tag="t2d")
    G.tensor_tensor(t2[:], rw_sh[:], rw_sh[:], Alu.mult)
    G.tensor_tensor(t2[:], t2[:], ds[:], Alu.mult)
    V.tensor_reduce(accs["p2"][:], _blk(t2[:], 48), AX.XY, Alu.add)
    yield

    pdt = pool.tile([P, NBLK * 3], dt.float32, tag="pdt")
    gtt = pool.tile([P, NBLK * 3], dt.float32, tag="gtt")
    nc.sync.dma_start(_blk(pdt[:], 3), aps["pd"].rearrange("(b p) c -> p b c", p=P))
    nc.sync.dma_start(_blk(gtt[:], 3), aps["gt"].rearrange("(b p) c -> p b c", p=P))
    d = pool.tile([P, NBLK * 3], dt.float32, tag="rgbd")
    V.tensor_tensor(d[:], pdt[:], gtt[:], Alu.subtract)
    V.tensor_tensor(d[:], d[:], d[:], Alu.mult)
    V.tensor_reduce(accs["rgb"][:], d[:], AX.X, Alu.add)
    yield


def _emit_hash(nc, pool, lvl, ones_h, acc, aps, first):
    E = nc.gpsimd
    idx = pool.tile([P, HCOLS], dt.int32, tag="hidx")
    src = aps[f"hi{lvl}"]
    nc.sync.dma_start(idx[:], bass.AP(tensor=src.tensor, offset=src.offset,
                                      ap=[[HROW, P], [1, HCOLS]]))
    emb = pool.tile([P, HCOLS * 2], dt.float32, tag="hemb")
    esrc = aps[f"he{lvl}"]
    nc.sync.dma_start(emb[:], bass.AP(tensor=esrc.tensor, offset=esrc.offset,
                                      ap=[[HROW * 2, P], [1, HCOLS * 2]]))
    sq = pool.tile([P, HCOLS * 2], dt.float32, tag="hsq")
    E.tensor_tensor(sq[:], emb[:], emb[:], Alu.mult)
    wv = pool.tile([P, HCOLS], dt.float32, tag="hw")
    sq3 = sq[:].rearrange("p (n two) -> p n two", two=2)
    E.tensor_tensor(wv[:], sq3[:, :, 0], sq3[:, :, 1], Alu.add)
    eq = pool.tile([P, HCOLS], dt.float32, tag="heq")
    nc.gpsimd.memset(eq[:, 0:1], 0.0)
    nc.vector.tensor_tensor(eq[:, 1:HCOLS], idx[:, 1:HCOLS], idx[:, 0:HCOLS - 1],
                             Alu.is_equal)
    yield
    S = pool.tile([P, HCOLS], dt.float32, tag="hS")
    nc.vector.tensor_tensor_scan(S[:], eq[:], wv[:], 0.0, Alu.mult, Alu.add)
    cc = pool.tile([P, HCOLS], dt.float32, tag="hcc")
    nc.vector.tensor_tensor_scan(cc[:], eq[:], ones_h, 0.0, Alu.mult, Alu.add)
    yield
    ratio = pool.tile([P, HCOLS], dt.float32, tag="hr")
    nc.vector.reciprocal(cc[:], cc[:])
    E.tensor_tensor(ratio[:], S[:], cc[:], Alu.mult)
    me = pool.tile([P, HCOLS], dt.float32, tag="hme")
    nc.scalar.activation(me[:, 0:HCOLS - 1], eq[:, 1:HCOLS], Act.Copy,
                         bias=1.0, scale=-1.0)
    E.tensor_tensor(ratio[:, HALO:HALO + HROW], ratio[:, HALO:HALO + HROW],
                    me[:, HALO:HALO + HROW], Alu.mult)
    if first:
        nc.vector.tensor_reduce(acc[:], ratio[:, HALO:HALO + HROW], AX.X, Alu.add)
    else:
        part = pool.tile([P, 1], dt.float32, tag="hpart")
        nc.vector.tensor_reduce(part[:], ratio[:, HALO:HALO + HROW], AX.X,
                                Alu.add)
        E.tensor_tensor(acc[:], acc[:], part[:], Alu.add)
    yield


def build_module(parts=("rgb", "dist", "hash", "l0", "l1")):
    nc = bacc.Bacc("TRN2", target_bir_lowering=False, debug=False,
                   enable_asserts=False, num_devices=N_CORES)
    aps = {}

    def din(name, shape, dtype=dt.float32):
        aps[name] = nc.dram_tensor(name, shape, dtype, kind="ExternalInput").ap()
    din("pd", [RPC, 3]); din("gt", [RPC, 3])
    din("sd", [RPC, 49]); din("rw", [RPC, 48])
    din("ps0", [RPC, 257]); din("pw0", [RPC, 256])
    din("ps1", [RPC, 97]); din("pw1", [RPC, 96])
    din("hi0", [HSLICE], dt.int32); din("he0", [HSLICE * 2])
    din("hi1", [HSLICE], dt.int32); din("he1", [HSLICE * 2])
    for lvl, L in LVL.items():
        NL = NB * L["EW"]
        din(f"c_u16_l{lvl}", [P, 2 * NL], dt.int16)
        din(f"c_maskf_l{lvl}", [P, (3 if lvl == 1 else 2) * NL])
    din("c_mask48", [P, NBLK * 48]); din("c_ones", [P, HCOLS])
    out_ap = nc.dram_tensor("out", [1, 1], dt.float32, kind="ExternalOutput").ap()

    with tile.TileContext(nc) as tc:
        _emit(nc, tc, aps, out_ap, parts)
    nc.compile()
    return nc


def _emit(nc, tc, aps, out_ap, parts=("rgb", "dist", "hash", "l0", "l1")):
    import contextlib
    V, G = nc.vector, nc.gpsimd
    with contextlib.ExitStack() as ctx:
        spool = ctx.enter_context(tc.tile_pool(name="shared", bufs=1))
        s_sh = spool.tile([P, NBLK * 49], dt.float32, tag="s_sh")
        nc.sync.dma_start(_blk(s_sh[:], 49),
                          aps["sd"].rearrange("(b p) x -> p b x", p=P))
        radios = {l: spool.tile([P, NBLK * 49], dt.float32, tag=f"radio{l}",
                                name=f"radio{l}")
                  for l in (0, 1)}

        cpool = ctx.enter_context(tc.tile_pool(name="consts", bufs=1))
        mask48 = cpool.tile([P, NBLK * 48], dt.float32, tag="mask48")
        ones_h = cpool.tile([P, HCOLS], dt.float32, tag="ones_h")
        lvl_consts = {}
        cdma = []
        cdma.append((mask48[:], aps["c_mask48"]))
        cdma.append((ones_h[:], aps["c_ones"]))
        for lvl, L in LVL.items():
            NL = NB * L["EW"]
            cu = cpool.tile([P, 2 * NL], dt.int16, tag=f"cu16_{lvl}",
                            name=f"cu16_{lvl}")
            mf = cpool.tile([P, (3 if lvl == 1 else 2) * NL], dt.float32,
                            tag=f"maskf_{lvl}", name=f"maskf_{lvl}")
            cdma.append((cu[:], aps[f"c_u16_l{lvl}"]))
            cdma.append((mf[:], aps[f"c_maskf_l{lvl}"]))
            cuv = cu[:].bitcast(dt.uint16)
            ioq2 = (mf[:][:, 2 * NL:3 * NL] if lvl == 1
                    else cuv[:, NL:2 * NL])
            lvl_consts[lvl] = (mf[:][:, 0:NL], mf[:][:, NL:2 * NL],
                               None, cu[:][:, 0:NL], ioq2)
            # (maskf, mask_cnt(f32), unused, ioG(i16), ioQ2)

        def _emit_consts():
            for dst, src_ap in cdma:
                nc.sync.dma_start(dst, src_ap)
            yield

        accs = {}
        for name in ("rgb", "p1", "p2", "hash", "l0a", "l0b", "l1a", "l1b"):
            accs[name] = cpool.tile([P, 1], dt.float32, tag=f"acc_{name}",
                                    name=f"acc_{name}")
            V.memset(accs[name][:], 0.0)

        spool = ctx.enter_context(tc.tile_pool(name="shared", bufs=1))
        s_sh = spool.tile([P, NBLK * 49], dt.float32, tag="s_sh")
        nc.sync.dma_start(_blk(s_sh[:], 49),
                          aps["sd"].rearrange("(b p) x -> p b x", p=P))
        radios = {l: spool.tile([P, NBLK * 49], dt.float32, tag=f"radio{l}",
                                name=f"radio{l}")
                  for l in (0, 1)}

        MRG = {0: dict(ME=V, ME2=V, EE=V),
               1: dict(ME=V, ME2=V, EE=V)}
        HEM = {
            "l0a": dict(SE=V, SE2=V, XE=V, EE=V, TE=V, FE=V),
            "l0b": dict(SE=V, SE2=V, XE=V, EE=V, TE=V, FE=G),
            "l1a": dict(SE=V, SE2=V, XE=V, EE=G, TE=G, FE=G, fchain=True),
            "l1b": dict(SE=V, SE2=V, XE=V, EE=G, TE=G, FE=G, fchain=True),
        }

        gens = []
        setup_pool = ctx.enter_context(tc.tile_pool(name="setup", bufs=1))
        gens.append(_emit_setup(nc, setup_pool, s_sh, radios, accs, mask48[:],
                                aps))
        mouts = {}
        for lvl in (0, 1):
            if f"l{lvl}" not in parts:
                continue
            mouts[lvl] = {}
            mp = ctx.enter_context(tc.tile_pool(name=f"mrg{lvl}", bufs=1))
            gens.append(_emit_level_merge(nc, tc, mp, lvl, s_sh,
                                          aps[f"ps{lvl}"], aps[f"pw{lvl}"],
                                          mouts[lvl], MRG[lvl]))
        gens.append(_emit_consts())
        if "hash" in parts:
            for lvl in (0, 1):
                hp2 = ctx.enter_context(tc.tile_pool(name=f"hash{lvl}", bufs=1))
                gens.append(_emit_hash(nc, hp2, lvl, ones_h[:], accs["hash"],
                                       aps, first=(lvl == 0)))
        for name, lvl, b0 in HALVES:
            if f"l{lvl}" not in parts:
                continue
            hp = ctx.enter_context(tc.tile_pool(name=name, bufs=1))
            gens.append(_emit_half(nc, hp, lvl, b0, s_sh, radios[lvl],
                                   mouts[lvl], lvl_consts[lvl], accs[name],
                                   HEM[name]))

        while gens:
            nxt = []
            for g in gens:
                try:
                    next(g)
                    nxt.append(g)
                except StopIteration:
                    pass
            gens = nxt

        with tc.tile_pool(name="fin", bufs=1) as pool:
            tot = pool.tile([P, 1], dt.float32, tag="tot")
            V.tensor_scalar(tot[:], accs["rgb"][:], W_RGB / (R * 3), None,
                            Alu.mult)
            for snm, lvl, _ in HALVES:
                V.scalar_tensor_tensor(tot[:], accs[snm][:],
                                       W_INTER / (R * (LVL[lvl]["X"] - 1)),
                                       tot[:], Alu.mult, Alu.add)
            V.scalar_tensor_tensor(tot[:], accs["p1"][:], W_DIST * 2.0 / R,
                                   tot[:], Alu.mult, Alu.add)
            V.scalar_tensor_tensor(tot[:], accs["p2"][:], W_DIST / (3.0 * R),
                                   tot[:], Alu.mult, Alu.add)
            V.scalar_tensor_tensor(tot[:], accs["hash"][:],
                                   W_HASH / (NUM_SEGMENTS * 2.0), tot[:],
                                   Alu.mult, Alu.add)
            res = pool.tile([1, 1], dt.float32, tag="res")
            G.tensor_reduce(res[:], tot[:], AX.C, Alu.add)
            nc.sync.dma_start(out_ap, res[:])


# ---------------- host side ----------------
_module_cache = {}


def _get_module():
    if "nc" not in _module_cache:
        _module_cache["nc"] = build_module()
    return _module_cache["nc"]


def shard_inputs(inputs):
    f32 = np.float32
    pd = np.ascontiguousarray(inputs["pd_rgbs"], f32)
    gt = np.ascontiguousarray(inputs["gt_rgbs"], f32)
    sd = np.ascontiguousarray(inputs["render_sdist"], f32)
    rw = np.ascontiguousarray(inputs["render_weights"], f32)
    ps0 = np.ascontiguousarray(inputs["prop_sdist_0"], f32)
    pw0 = np.ascontiguousarray(inputs["prop_weights_0"], f32)
    ps1 = np.ascontiguousarray(inputs["prop_sdist_1"], f32)
    pw1 = np.ascontiguousarray(inputs["prop_weights_1"], f32)
    hashes = {}
    for lvl in (0, 1):
        idx = np.asarray(inputs[f"enc_idx_{lvl}"]).astype(np.int32)
        emb = np.ascontiguousarray(inputs[f"enc_embds_{lvl}"], f32)
        idx_pad = np.full(M + 2 * HALO, -1, np.int32)
        idx_pad[HALO:HALO + M] = idx
        emb_pad = np.zeros((M + 2 * HALO, 2), f32)
        emb_pad[HALO:HALO + M] = emb
        hashes[lvl] = (idx_pad, emb_pad)

    consts = {}
    rep = lambda row: np.ascontiguousarray(np.tile(row, (P, 1)))
    for lvl, L in LVL.items():
        EW, QWS, X = L["EW"], L["QWS"], L["X"]
        NL = NB * EW
        NQ = NB * QWS
        ioG = np.concatenate([2 * np.arange(b * EW, (b + 1) * EW,
                                            dtype=np.uint16)
                              for b in range(NB)])
        # query dest: rank-1 + b*QWS; C' = C + 98b so fold +98b here:
        # ioQ2 = i+1 + b*QWS + 98b -> (ioQ2 - C')*qf - 1 = rank-1 + b*QWS
        ioQ2 = np.concatenate([np.arange(1, EW + 1, dtype=np.uint16)
                               + b * QWS + 98 * b for b in range(NB)])
        packed = np.concatenate([ioG, ioQ2]).astype(np.uint16)
        consts[f"c_u16_l{lvl}"] = rep(packed.view(np.int16))
        msk = np.ones(NL, f32)
        msk[::EW] = 0.0
        mcnt = np.ones(NL, f32)
        for b in range(NB):
            mcnt[b * EW] = b
        parts_ = [msk, mcnt]
        if lvl == 1:
            parts_.append(np.concatenate([np.arange(1, EW + 1, dtype=f32)
                                          + b * QWS + 98 * b
                                          for b in range(NB)]))
        consts[f"c_maskf_l{lvl}"] = rep(np.concatenate(parts_))
    m48 = np.ones(NBLK * 48, f32)
    m48[::48] = 0.0
    consts["c_mask48"] = rep(m48)
    consts["c_ones"] = rep(np.ones(HCOLS, f32))

    in_maps = []
    for c in range(N_CORES):
        r0 = c * RPC
        lo = c * MPC
        im = {
            "pd": pd[r0:r0 + RPC], "gt": gt[r0:r0 + RPC],
            "sd": sd[r0:r0 + RPC], "rw": rw[r0:r0 + RPC],
            "ps0": ps0[r0:r0 + RPC], "pw0": pw0[r0:r0 + RPC],
            "ps1": ps1[r0:r0 + RPC], "pw1": pw1[r0:r0 + RPC],
        }
        for lvl in (0, 1):
            idx_pad, emb_pad = hashes[lvl]
            im[f"hi{lvl}"] = np.ascontiguousarray(idx_pad[lo:lo + HSLICE])
            im[f"he{lvl}"] = np.ascontiguousarray(
                emb_pad[lo:lo + HSLICE].reshape(-1))
        im.update(consts)
        in_maps.append(im)
    return in_maps


def kernel(**inputs) -> np.ndarray:
    nc = _get_module()
    in_maps = shard_inputs(inputs)
    res = run_bass_kernel_spmd(nc, in_maps, core_ids=list(range(N_CORES)))
    total = np.float64(0.0)
    for r in res.results:
        total += np.float64(r["out"][0, 0])
    return np.float32(total)


# revision 9
# speedup vs baseline: 1.7396x; 1.0092x over previous
"""Trainium2 Bass kernel v2 for nn_Loss_dict_50646254354805 (NeRF-style loss).

v2 vs baseline:
- bitonic merges on uint16 quantized keys (value*15000 + 2 tag bits) -> DVE
  2x perf mode; keys determine ORDER only.
- exact f32 values (queries/em/ep) and radio reach the merged domain via
  batched u16-half local_scatters through one shared index table (idxcat):
  merged positions come from the C/Cm count scans.
- one merge per level; post-merge work split into two 2-block half-streams
  with per-stream engine maps; all generators emitted stage-interleaved so
  DVE / Pool / Act overlap.
"""
import numpy as np

import concourse.bass as bass
import concourse.mybir as mybir
import concourse.tile as tile
from concourse import bacc
from concourse.bass_utils import run_bass_kernel_spmd

dt = mybir.dt
Alu = mybir.AluOpType
AX = mybir.AxisListType
Act = mybir.ActivationFunctionType
P = 128

PULSE = (0.01, 0.005)
W_RGB, W_INTER, W_DIST, W_HASH = 1.0, 1.0, 0.01, 0.1
NUM_SEGMENTS = 65536
R, N = 4096, 48
M = R * N
N_CORES = 8
RPC = R // N_CORES
NBLK = RPC // P               # 4 ray blocks per core
MPC = M // N_CORES
HALO = 64
HROW = MPC // P
HCOLS = HROW + HALO + 1
HSLICE = HALO + MPC + HALO

VOFF = 0.97
QS = 15000.0                  # key quantization scale
PADK = 0xFFFC

LVL = {0: dict(X=257, n2=512), 1: dict(X=97, n2=256)}
for _L in LVL.values():
    _L["EW"] = ((_L["X"] + 98 + 1 + 7) // 8) * 8        # 360 / 200
    _L["QWS"] = _L["EW"] - 98                           # 262 / 102

NB = 2                        # blocks per half-stream
HALVES = [("l1a", 1, 0), ("l1b", 1, 2), ("l0a", 0, 0), ("l0b", 0, 2)]


def _blk(ap, n):
    return ap.rearrange("p (b n) -> p b n", n=n)


def _ts_int(eng, out, in0, imm1, op0, imm2=None, op1=None):
    ins_ = [eng.lower_ap(in0), mybir.ImmediateValue(dtype=dt.int32, value=int(imm1))]
    kw = dict(op0=op0)
    if imm2 is not None:
        ins_.append(mybir.ImmediateValue(dtype=dt.int32, value=int(imm2)))
        kw["op1"] = op1
    return eng.add_instruction(mybir.InstTensorScalarPtr(
        name=eng.bass.get_next_instruction_name(),
        ins=ins_, outs=[eng.lower_ap(out)], **kw))

BIGF = 3.0


def _merge_gen(eng, bufa, bufb, width, out, ew=None, trim4d=True):
    """Ascending bitonic merge over [P, NBLK*width] u16 ping-pong tiles.
    Generator: yields after each stage so two levels' merges interleave in
    the engine queue. Result tile is appended to `out`.

    If ew is given, only outputs [0, ew+2d-1] of each block are needed
    downstream, so late stages skip whole 2d-chunks beyond that window."""
    cur, nxt = bufa, bufb
    d = width // 2
    while d >= 1:
        nch = width // (2 * d)
        keep = nch
        if ew is not None and trim4d:
            keep = min(nch, -(-(ew + 2 * d - 1) // (2 * d)))
        if keep == nch:
            c3 = cur[:].rearrange("p (c td) -> p c td", td=2 * d)
            n3 = nxt[:].rearrange("p (c td) -> p c td", td=2 * d)
        else:
            c3 = cur[:].rearrange("p (b c td) -> p (b c) td",
                                  td=2 * d, c=nch)[: , 0:0]  # placeholder
        if keep == nch:
            lo_in, hi_in = c3[:, :, 0:d], c3[:, :, d:2 * d]
            eng.tensor_tensor(n3[:, :, 0:d], lo_in, hi_in, Alu.min)
            eng.tensor_tensor(n3[:, :, d:2 * d], lo_in, hi_in, Alu.max)
        else:
            c4 = cur[:].rearrange("p (b c td) -> p b c td", td=2 * d, c=nch)
            n4 = nxt[:].rearrange("p (b c td) -> p b c td", td=2 * d, c=nch)
            lo_in = c4[:, :, 0:keep, 0:d]
            hi_in = c4[:, :, 0:keep, d:2 * d]
            eng.tensor_tensor(n4[:, :, 0:keep, 0:d], lo_in, hi_in, Alu.min)
            eng.tensor_tensor(n4[:, :, 0:keep, d:2 * d], lo_in, hi_in, Alu.max)
        cur, nxt = nxt, cur
        d //= 2
        if d >= 1:
            yield
    out.append(cur)


def _emit_level_merge(nc, tc, pool, lvl, s_sh, x_ap, pwt_ap, out, eng):
    """Generator: quantize + b1/b2 merges for all 4 blocks of one level.

    lvl 0: uint16 quantized keys, merged on DVE (2x mode).
    lvl 1: f32-bitcast tagged keys (baseline-style), merged on Pool where
    f32 min/max is legal -- frees DVE during the big level-0 merge."""
    ME, ME2, EE = eng["ME"], eng["ME2"], eng["EE"]
    AE = nc.scalar
    fkeys = eng.get("fkeys", False)
    L = LVL[lvl]
    X, n2 = L["X"], L["n2"]
    pw = PULSE[lvl]
    kdt = dt.float32 if fkeys else dt.uint16

    xt = pool.tile([P, NBLK * X], dt.float32, tag="xt")
    nc.sync.dma_start(_blk(xt[:], X), x_ap.rearrange("(b p) x -> p b x", p=P))
    pwt = pool.tile([P, NBLK * (X - 1)], dt.float32, tag="pwt")
    nc.sync.dma_start(_blk(pwt[:], X - 1),
                      pwt_ap.rearrange("(b p) x -> p b x", p=P))
    out["xt"] = xt
    out["pwt"] = pwt

    b2a = pool.tile([P, NBLK * n2], kdt, tag="b2a")
    b2b = pool.tile([P, NBLK * n2], kdt, tag="b2b")
    b2a3 = _blk(b2a[:], n2)
    b1a = pool.tile([P, NBLK * 128], kdt, tag="b1a")
    b1b = pool.tile([P, NBLK * 128], kdt, tag="b1b")
    b1a3 = _blk(b1a[:], 128)
    if fkeys:
        nc.gpsimd.memset(b1a[:], BIGF)
        emsh = pool.tile([P, NBLK * 49], dt.float32, tag="emsh")
        AE.activation(emsh[:], s_sh[:], Act.Copy, bias=1.0 - pw)
        epsh = pool.tile([P, NBLK * 49], dt.float32, tag="epsh")
        AE.activation(epsh[:], s_sh[:], Act.Copy, bias=1.0 + pw)
        _ts_int(EE, b1a3[:, :, 0:49].bitcast(dt.int32),
                _blk(emsh[:], 49).bitcast(dt.int32), ~3, Alu.bitwise_and,
                1, Alu.bitwise_or)
        _ts_int(EE, b1a3[:, :, 79:128][:, :, ::-1].bitcast(dt.int32),
                _blk(epsh[:], 49).bitcast(dt.int32), ~3, Alu.bitwise_and,
                3, Alu.bitwise_or)
    else:
        nc.gpsimd.memset(b1a[:], PADK)
        emq = pool.tile([P, NBLK * 49], dt.uint16, tag="emq")
        EE.tensor_scalar(emq[:], s_sh[:], QS, (1.0 - pw - VOFF) * QS + 0.5,
                         Alu.mult, Alu.add)
        epq = pool.tile([P, NBLK * 49], dt.uint16, tag="epq")
        EE.tensor_scalar(epq[:], s_sh[:], QS, (1.0 + pw - VOFF) * QS + 0.5,
                         Alu.mult, Alu.add)
        EE.tensor_scalar(b1a3[:, :, 0:49], _blk(emq[:], 49), 4, 1,
                         Alu.mult, Alu.add)
        EE.tensor_scalar(b1a3[:, :, 79:128][:, :, ::-1], _blk(epq[:], 49), 4, 3,
                         Alu.mult, Alu.add)
    yield
    _r1 = []
    yield from _merge_gen(ME, b1a, b1b, 128, _r1, ew=98, trim4d=not fkeys)
    b1 = _r1[0]
    yield
    if fkeys:
        nc.gpsimd.memset(b2a3[:, :, X:n2 - 128], BIGF)
        xsh = pool.tile([P, NBLK * X], dt.float32, tag="xsh")
        AE.activation(xsh[:], xt[:], Act.Copy, bias=1.0)
        _ts_int(EE, b2a3[:, :, 0:X].bitcast(dt.int32),
                _blk(xsh[:], X).bitcast(dt.int32), ~3, Alu.bitwise_and)
        EE.tensor_copy(b2a3[:, :, n2 - 128:n2][:, :, ::-1], _blk(b1[:], 128))
    else:
        nc.gpsimd.memset(b2a3[:, :, X:n2 - 128], PADK)
        xq = pool.tile([P, NBLK * X], dt.uint16, tag="xq")
        EE.tensor_scalar(xq[:], xt[:], QS, (1.0 - VOFF) * QS + 0.5,
                         Alu.mult, Alu.add)
        EE.tensor_scalar(b2a3[:, :, 0:X], _blk(xq[:], X), 4, None, Alu.mult)
        EE.tensor_copy(b2a3[:, :, n2 - 128:n2][:, :, ::-1], _blk(b1[:], 128))
    yield
    _r2 = []
    yield from _merge_gen(ME2, b2a, b2b, n2, _r2, ew=L["EW"],
                          trim4d=not fkeys)
    out["m"] = _r2[0]
    yield


def _emit_half(nc, pool, lvl, b0, s_sh, radio_full, mout, consts, acc, eng):
    """Generator: post-merge pipeline for blocks [b0, b0+NB) of one level."""
    SE, XE, EE, FE = (eng[k] for k in ("SE", "XE", "EE", "FE"))
    TE = eng.get("TE", EE)
    SE2 = eng.get("SE2", SE)
    fkeys = eng.get("fkeys", False)
    fchain = eng.get("fchain", False)
    mdt = dt.float32 if (fkeys or fchain) else dt.uint16
    AE = nc.scalar
    L = LVL[lvl]
    X, n2, EW, QWS = L["X"], L["n2"], L["EW"], L["QWS"]
    NL = NB * EW
    NQ = NB * QWS
    NE = NB * 49
    VW = NQ + 2 * NE          # vcat width: [x | em | ep]
    pw = PULSE[lvl]
    maskf, mask_cnt, io49p, ioG, ioQ2 = consts

    def blkE(ap):
        return ap.rearrange("p (b n) -> p b n", b=NB)

    ss = s_sh[:][:, b0 * 49:(b0 + NB) * 49]

    # ---------- sources: exact values + radio (independent of merge) ----------
    vcat = pool.tile([P, VW], dt.float32, tag="vcat")
    nc.gpsimd.memset(_blk(vcat[:, 0:NQ], QWS)[:, :, X:QWS], 0.0)
    radcat = pool.tile([P, 2 * NE], dt.float32, tag="radcat")
    rsl_ = radio_full[:][:, b0 * 49:(b0 + NB) * 49]
    FE.tensor_copy(radcat[:, 0:NE], rsl_)
    FE.tensor_scalar(radcat[:, NE:2 * NE], radcat[:, 0:NE], -1.0, None, Alu.mult)
    yield
    # wait for merge result
    while "m" not in mout:
        yield
    m = mout["m"]
    xt, pwt_full = mout["xt"], mout["pwt"]
    mSh = _blk(m[:], n2)[:, b0:b0 + NB, 0:EW]       # [P, NB, EW] strided
    xts = _blk(xt[:], X)[:, b0:b0 + NB]             # [P, NB, X]
    pwt = _blk(pwt_full[:], X - 1)[:, b0:b0 + NB]
    AE.activation(_blk(vcat[:, 0:NQ], QWS)[:, :, 0:X], xts, Act.Copy)
    AE.activation(_blk(vcat[:, NQ:NQ + NE], 49), _blk(ss, 49), Act.Copy, bias=-pw)
    AE.activation(_blk(vcat[:, NQ + NE:VW], 49), _blk(ss, 49), Act.Copy, bias=pw)
    yield

    # ---------- tags + counts ----------
    if fkeys:
        tag32 = pool.tile([P, NL], dt.int32, tag="tag32")
        _ts_int(XE, blkE(tag32[:]), mSh.bitcast(dt.int32), 3, Alu.bitwise_and)
        ev_f = pool.tile([P, NL], dt.float32, tag="ev_f")
        em_f = pool.tile([P, NL], dt.float32, tag="em_f")
        ep_f = pool.tile([P, NL], dt.float32, tag="ep_f")
        _ts_int(TE, em_f[:], tag32[:], 1, Alu.is_equal)
        _ts_int(TE, ep_f[:], tag32[:], 3, Alu.is_equal)
        FE.tensor_tensor(ev_f[:], em_f[:], ep_f[:], Alu.add)
    elif fchain:
        tagb_t = pool.tile([P, NL], dt.uint16, tag="tagb")
        tagb = tagb_t[:]
        XE.tensor_scalar(blkE(tagb), mSh, 3, None, Alu.bitwise_and)
        em_f = pool.tile([P, NL], dt.float32, tag="em_f")
        TE.tensor_scalar(em_f[:], tagb, 1, None, Alu.is_equal)
        ep_f = pool.tile([P, NL], dt.float32, tag="ep_f")
        TE.tensor_scalar(ep_f[:], tagb, 3, None, Alu.is_equal)
        ev_f = pool.tile([P, NL], dt.float32, tag="ev_f")
        FE.tensor_tensor(ev_f[:], em_f[:], ep_f[:], Alu.add)
    else:
        tagb_t = pool.tile([P, NL], dt.uint16, tag="tagb")
        tagb = tagb_t[:]
        XE.tensor_scalar(blkE(tagb), mSh, 3, None, Alu.bitwise_and)
        MQ = eng.get("MQ", TE)
        ev_f = pool.tile([P, NL], dt.uint16, tag="ev_f")
        TE.tensor_scalar(ev_f[:], tagb, 1, None, Alu.bitwise_and)
        em_f = pool.tile([P, NL], dt.uint16, tag="em_f")
        MQ.tensor_scalar(em_f[:], tagb, 1, None, Alu.is_equal)
        ep_f = pool.tile([P, NL], dt.uint16, tag="ep_f")
        MQ.tensor_scalar(ep_f[:], tagb, 3, None, Alu.is_equal)
    yield
    C = pool.tile([P, NL], mdt, tag="C")
    SE.tensor_tensor_scan(C[:], mask_cnt, ev_f[:], 0.0, Alu.mult, Alu.add)
    Cm = pool.tile([P, NL], mdt, tag="Cm")
    SE.tensor_tensor_scan(Cm[:], mask_cnt, em_f[:], 0.0, Alu.mult, Alu.add)
    yield

    # ---------- idxcat: merged position of every source element ----------
    t2 = pool.tile([P, NL], mdt, tag="t2")
    t3 = pool.tile([P, NL], mdt, tag="t3")
    if fkeys:
        t1 = tag32                                  # dead after masks
    elif fchain:
        t1 = pool.tile([P, NL], dt.float32, tag="t1f")
    else:
        t1 = tagb_t                                 # dead after masks
    # block offsets (49b/98b) ride in from the mask_cnt scan carry; section
    # offsets NQ / NQ+NE are flat immediates. One combined scatter:
    # t1 = (Cm'+NQ)*em + (C'-Cm'+NQ+NE)*ep + (ioQ2-C')*qf - 1
    EE.tensor_tensor(t2[:], C[:], Cm[:], Alu.subtract)
    EE.tensor_scalar(t2[:], t2[:], NQ + NE, None, Alu.add)
    EE.tensor_tensor(t2[:], t2[:], ep_f[:], Alu.mult)
    t1v = t1[:].bitcast(dt.float32) if fkeys else t1[:]
    ffull = fkeys or fchain
    EE.tensor_scalar(t1v, Cm[:], NQ, None, Alu.add)
    EE.tensor_tensor(t1v, t1v, em_f[:], Alu.mult)
    EE.tensor_tensor(t1v, t1v, t2[:], Alu.add)
    qf = em_f                                       # em_f dead after t1
    TE2 = FE if ffull else TE
    TE2.tensor_scalar(qf[:], ev_f[:], 0, None, Alu.is_equal)
    EE.tensor_tensor(t3[:], ioQ2, C[:], Alu.subtract)
    EE.tensor_tensor(t3[:], t3[:], qf[:], Alu.mult)
    EE.tensor_tensor(t1v, t1v, t3[:], Alu.add)
    EE.tensor_scalar(t1v, t1v, 1, None, Alu.subtract)       # idx all
    EE.tensor_scalar(t2[:], t3[:], 1, None, Alu.subtract)       # idxq
    if ffull:
        t1s = ev_f[:].bitcast(dt.int16)[:, 0:NL]    # ev_f dead after qf
        AE.activation(t1s, t1v, Act.Copy)
        t2s = ep_f[:].bitcast(dt.int16)[:, 0:NL]    # ep_f dead after t2
        AE.activation(t2s, t2[:], Act.Copy)
    else:
        t1s = t1[:].bitcast(dt.int16)
        t2s = t2[:].bitcast(dt.int16)
    yield
    idxcat = pool.tile([P, VW], dt.uint16, tag="idxcat")
    nc.gpsimd.local_scatter(idxcat[:].bitcast(dt.int16), ioG,
                            t1s, channels=P,
                            num_elems=VW, num_idxs=NL)
    idx2 = pool.tile([P, 2 * VW], dt.uint16, tag="idx2")
    i2v = idx2[:].rearrange("p (n two) -> p n two", two=2)
    AE.activation(i2v[:, :, 0], idxcat[:], Act.Copy)
    AE.activation(i2v[:, :, 1], idxcat[:], Act.Copy, bias=1.0)
    yield

    # ---------- pair-scatter exact values + radio into merged domain ----------
    v = pool.tile([P, NL], dt.float32, tag="v")
    nc.gpsimd.local_scatter(v[:].bitcast(dt.int16),
                            vcat[:].bitcast(dt.int16),
                            idx2[:].bitcast(dt.int16), channels=P,
                            num_elems=2 * NL, num_idxs=2 * VW)
    F1 = pool.tile([P, NL], dt.float32, tag="F1")   # radio_m
    nc.gpsimd.local_scatter(F1[:].bitcast(dt.int16),
                            radcat[:].bitcast(dt.int16),
                            idx2[:, 2 * NQ:2 * VW].bitcast(dt.int16), channels=P,
                            num_elems=2 * NL, num_idxs=4 * NE)
    yield

    # ---------- density reconstruction ----------
    F2 = pool.tile([P, NL], dt.float32, tag="F2")
    SE2.tensor_tensor_scan(F2[:], maskf, F1[:], 0.0, Alu.mult, Alu.add)  # g
    if ffull:
        dv = t3                                     # t3 dead after idx phase
    else:
        dv = pool.tile([P, NL], dt.float32, tag="dv")
    dv3 = blkE(dv[:])
    v3 = blkE(v[:])
    nc.gpsimd.memset(dv3[:, :, 0:1], 0.0)
    FE.tensor_tensor(dv3[:, :, 1:EW], v3[:, :, 1:EW], v3[:, :, 0:EW - 1],
                     Alu.subtract)
    yield
    wg = v                                          # v dead after dv
    wg3 = blkE(wg[:])
    nc.gpsimd.memset(wg3[:, :, 0:1], 0.0)
    FE.tensor_tensor(wg3[:, :, 1:EW], dv3[:, :, 1:EW],
                     blkE(F2[:])[:, :, 0:EW - 1], Alu.mult)
    w_t = F1                                        # radio dead after g
    SE2.tensor_tensor_scan(w_t[:], maskf, wg[:], 0.0, Alu.mult, Alu.add)
    yield
    wc = wg                                         # wg dead
    AE.activation(wc[:], w_t[:], Act.Relu, scale=0.5)
    scr = pool.tile([P, NL], dt.float32, tag="scr")
    wc3 = blkE(wc[:])
    s3_ = blkE(scr[:])
    nc.gpsimd.memset(s3_[:, :, 0:1], 0.0)
    FE.tensor_tensor(s3_[:, :, 1:EW], wc3[:, :, 1:EW], wc3[:, :, 0:EW - 1],
                     Alu.add)
    area = w_t                                      # w dead after wc
    a3 = blkE(area[:])
    nc.gpsimd.memset(a3[:, :, 0:1], 0.0)
    FE.tensor_tensor(a3[:, :, 1:EW], s3_[:, :, 1:EW],
                     dv3[:, :, 1:EW], Alu.mult)
    cdf = F2                                        # g dead after wg
    SE2.tensor_tensor_scan(cdf[:], maskf, area[:], 0.0, Alu.mult, Alu.add)
    yield

    # ---------- compact cdf at query slots (pair-scatter) ----------
    idx2q = idx2                                    # idx2 dead after scatters
    i2qv = idx2q[:][:, 0:2 * NL].rearrange("p (n two) -> p n two", two=2)
    tqu = Cm[:].bitcast(dt.uint16)[:, 0:NL]         # Cm dead after t1
    if ffull:
        AE.activation(tqu.bitcast(dt.int16), t2[:], Act.Copy, scale=2.0)
    else:
        EE.tensor_scalar(tqu, t2[:], 2, None, Alu.mult)
    AE.activation(i2qv[:, :, 0], tqu, Act.Copy)
    AE.activation(i2qv[:, :, 1], tqu, Act.Copy, bias=1.0)
    cdfq = vcat                                     # vcat dead after v scatter
    cqn = cdfq[:][:, 0:NQ]
    nc.gpsimd.local_scatter(cqn.bitcast(dt.int16),
                            cdf[:].bitcast(dt.int16),
                            idx2q[:][:, 0:2 * NL].bitcast(dt.int16), channels=P,
                            num_elems=2 * NQ, num_idxs=2 * NL)
    del t2s
    yield

    # ---------- loss tail ----------
    NW = NB * (X - 1)
    ws = scr                                        # dead after area
    ws2 = ws[:][:, 0:NW]
    cqf = _blk(cdfq[:][:, 0:NQ], QWS)
    FE.tensor_tensor(_blk(ws2, X - 1), cqf[:, :, 1:X], cqf[:, :, 0:X - 1],
                     Alu.subtract)
    FE.tensor_tensor(_blk(ws2, X - 1), _blk(ws2, X - 1), pwt, Alu.subtract)
    den = area                                      # dead after cdf
    den2 = den[:][:, 0:NW]
    AE.activation(_blk(den2, X - 1), pwt, Act.Copy, bias=1e-5)
    nc.vector.reciprocal(den2, den2)
    rsl = dv                                        # dead after area
    AE.activation(rsl[:][:, 0:NW], ws2, Act.Relu)
    FE.tensor_tensor(ws2, ws2, rsl[:][:, 0:NW], Alu.mult)
    FE.tensor_tensor(ws2, ws2, den2, Alu.mult)
    nc.vector.tensor_reduce(acc[:], _blk(ws2, X - 1), AX.XY, Alu.add)
    yield


def _emit_setup(nc, pool, s_sh, radios, accs, mask48, aps):
    V, G = nc.vector, nc.gpsimd
    rw_sh = pool.tile([P, NBLK * 48], dt.float32, tag="rw_sh")
    nc.sync.dma_start(_blk(rw_sh[:], 48),
                      aps["rw"].rearrange("(b p) x -> p b x", p=P))
    s3 = _blk(s_sh[:], 49)
    ds = pool.tile([P, NBLK * 48], dt.float32, tag="ds")
    G.tensor_tensor(_blk(ds[:], 48), s3[:, :, 1:49], s3[:, :, 0:48], Alu.subtract)
    dse = pool.tile([P, NBLK * 48], dt.float32, tag="dse")
    nc.scalar.activation(dse[:], ds[:], Act.Copy, bias=1e-8)
    V.reciprocal(dse[:], dse[:])
    wnorm = pool.tile([P, NBLK * 48], dt.float32, tag="wnorm")
    G.tensor_tensor(wnorm[:], rw_sh[:], dse[:], Alu.mult)
    wnp = pool.tile([P, NBLK * 50], dt.float32, tag="wnp")
    G.memset(wnp[:], 0.0)
    V.tensor_copy(_blk(wnp[:], 50)[:, :, 1:49], _blk(wnorm[:], 48))
    diff = pool.tile([P, NBLK * 49], dt.float32, tag="diff")
    wnp3 = _blk(wnp[:], 50)
    G.tensor_tensor(_blk(diff[:], 49), wnp3[:, :, 1:50], wnp3[:, :, 0:49],
                    Alu.subtract)
    for lvl in (0, 1):
        G.tensor_scalar(radios[lvl][:], diff[:], 1.0 / (2 * PULSE[lvl]), None,
                        Alu.mult)
    yield

    mid = pool.tile([P, NBLK * 48], dt.float32, tag="mid")
    G.tensor_tensor(_blk(mid[:], 48), s3[:, :, 1:49], s3[:, :, 0:48], Alu.add)
    wm = pool.tile([P, NBLK * 48], dt.float32, tag="wm")
    V.scalar_tensor_tensor(wm[:], mid[:], 0.5, rw_sh[:], Alu.mult, Alu.mult)
    Cin = pool.tile([P, NBLK * 48], dt.float32, tag="Cin")
    V.tensor_tensor_scan(Cin[:], mask48, rw_sh[:], 0.0, Alu.mult, Alu.add)
    Sin = pool.tile([P, NBLK * 48], dt.float32, tag="Sin")
    V.tensor_tensor_scan(Sin[:], mask48, wm[:], 0.0, Alu.mult, Alu.add)
    yield
    A = pool.tile([P, NBLK * 47], dt.float32, tag="A47")
    m3 = _blk(mid[:], 48)
    c3 = _blk(Cin[:], 48)
    sw3 = _blk(Sin[:], 48)
    rw3 = _blk(rw_sh[:], 48)
    A3 = _blk(A[:], 47)
    V.scalar_tensor_tensor(A3, m3[:, :, 1:48], 0.5, c3[:, :, 0:47],
                           Alu.mult, Alu.mult)
    V.tensor_tensor(A3, A3, sw3[:, :, 0:47], Alu.subtract)
    V.tensor_tensor(A3, A3, rw3[:, :, 1:48], Alu.mult)
    V.tensor_reduce(accs["p1"][:], A3, AX.XY, Alu.add)
    t2 = pool.tile([P, NBLK * 48], dt.float32, tag="t2d")
    G.tensor_tensor(t2[:], rw_sh[:], rw_sh[:], Alu.mult)
    G.tensor_tensor(t2[:], t2[:], ds[:], Alu.mult)
    V.tensor_reduce(accs["p2"][:], _blk(t2[:], 48), AX.XY, Alu.add)
    yield

    pdt = pool.tile([P, NBLK * 3], dt.float32, tag="pdt")
    gtt = pool.tile([P, NBLK * 3], dt.float32, tag="gtt")
    nc.sync.dma_start(_blk(pdt[:], 3), aps["pd"].rearrange("(b p) c -> p b c", p=P))
    nc.sync.dma_start(_blk(gtt[:], 3), aps["gt"].rearrange("(b p) c -> p b c", p=P))
    d = pool.tile([P, NBLK * 3], dt.float32, tag="rgbd")
    V.tensor_tensor(d[:], pdt[:], gtt[:], Alu.subtract)
    V.tensor_tensor(d[:], d[:], d[:], Alu.mult)
    V.tensor_reduce(accs["rgb"][:], d[:], AX.X, Alu.add)
    yield


def _emit_hash(nc, pool, lvl, ones_h, acc, aps, first):
    E = nc.gpsimd
    idx = pool.tile([P, HCOLS], dt.int32, tag="hidx")
    src = aps[f"hi{lvl}"]
    nc.sync.dma_start(idx[:], bass.AP(tensor=src.tensor, offset=src.offset,
                                      ap=[[HROW, P], [1, HCOLS]]))
    emb = pool.tile([P, HCOLS * 2], dt.float32, tag="hemb")
    esrc = aps[f"he{lvl}"]
    nc.sync.dma_start(emb[:], bass.AP(tensor=esrc.tensor, offset=esrc.offset,
                                      ap=[[HROW * 2, P], [1, HCOLS * 2]]))
    sq = pool.tile([P, HCOLS * 2], dt.float32, tag="hsq")
    E.tensor_tensor(sq[:], emb[:], emb[:], Alu.mult)
    wv = pool.tile([P, HCOLS], dt.float32, tag="hw")
    sq3 = sq[:].rearrange("p (n two) -> p n two", two=2)
    E.tensor_tensor(wv[:], sq3[:, :, 0], sq3[:, :, 1], Alu.add)
    eq = pool.tile([P, HCOLS], dt.float32, tag="heq")
    nc.gpsimd.memset(eq[:, 0:1], 0.0)
    nc.vector.tensor_tensor(eq[:, 1:HCOLS], idx[:, 1:HCOLS], idx[:, 0:HCOLS - 1],
                             Alu.is_equal)
    yield
    S = pool.tile([P, HCOLS], dt.float32, tag="hS")
    nc.vector.tensor_tensor_scan(S[:], eq[:], wv[:], 0.0, Alu.mult, Alu.add)
    cc = pool.tile([P, HCOLS], dt.float32, tag="hcc")
    nc.vector.tensor_tensor_scan(cc[:], eq[:], ones_h, 0.0, Alu.mult, Alu.add)
    yield
    ratio = pool.tile([P, HCOLS], dt.float32, tag="hr")
    nc.vector.reciprocal(cc[:], cc[:])
    E.tensor_tensor(ratio[:], S[:], cc[:], Alu.mult)
    me = pool.tile([P, HCOLS], dt.float32, tag="hme")
    nc.scalar.activation(me[:, 0:HCOLS - 1], eq[:, 1:HCOLS], Act.Copy,
                         bias=1.0, scale=-1.0)
    E.tensor_tensor(ratio[:, HALO:HALO + HROW], ratio[:, HALO:HALO + HROW],
                    me[:, HALO:HALO + HROW], Alu.mult)
    if first:
        nc.vector.tensor_reduce(acc[:], ratio[:, HALO:HALO + HROW], AX.X, Alu.add)
    else:
        part = pool.tile([P, 1], dt.float32, tag="hpart")
        nc.vector.tensor_reduce(part[:], ratio[:, HALO:HALO + HROW], AX.X,
                                Alu.add)
        E.tensor_tensor(acc[:], acc[:], part[:], Alu.add)
    yield


def build_module(parts=("rgb", "dist", "hash", "l0", "l1")):
    nc = bacc.Bacc("TRN2", target_bir_lowering=False, debug=False,
                   enable_asserts=False, num_devices=N_CORES)
    aps = {}

    def din(name, shape, dtype=dt.float32):
        aps[name] = nc.dram_tensor(name, shape, dtype, kind="ExternalInput").ap()
    din("pd", [RPC, 3]); din("gt", [RPC, 3])
    din("sd", [RPC, 49]); din("rw", [RPC, 48])
    din("ps0", [RPC, 257]); din("pw0", [RPC, 256])
    din("ps1", [RPC, 97]); din("pw1", [RPC, 96])
    din("hi0", [HSLICE], dt.int32); din("he0", [HSLICE * 2])
    din("hi1", [HSLICE], dt.int32); din("he1", [HSLICE * 2])
    for lvl, L in LVL.items():
        NL = NB * L["EW"]
        din(f"c_u16_l{lvl}", [P, 2 * NL], dt.int16)
        din(f"c_maskf_l{lvl}", [P, (3 if lvl == 1 else 2) * NL])
    din("c_mask48", [P, NBLK * 48]); din("c_ones", [P, HCOLS])
    out_ap = nc.dram_tensor("out", [1, 1], dt.float32, kind="ExternalOutput").ap()

    with tile.TileContext(nc) as tc:
        _emit(nc, tc, aps, out_ap, parts)
    nc.compile()
    return nc


def _emit(nc, tc, aps, out_ap, parts=("rgb", "dist", "hash", "l0", "l1")):
    import contextlib
    V, G = nc.vector, nc.gpsimd
    with contextlib.ExitStack() as ctx:
        spool = ctx.enter_context(tc.tile_pool(name="shared", bufs=1))
        s_sh = spool.tile([P, NBLK * 49], dt.float32, tag="s_sh")
        nc.sync.dma_start(_blk(s_sh[:], 49),
                          aps["sd"].rearrange("(b p) x -> p b x", p=P))
        radios = {l: spool.tile([P, NBLK * 49], dt.float32, tag=f"radio{l}",
                                name=f"radio{l}")
                  for l in (0, 1)}

        cpool = ctx.enter_context(tc.tile_pool(name="consts", bufs=1))
        mask48 = cpool.tile([P, NBLK * 48], dt.float32, tag="mask48")
        ones_h = cpool.tile([P, HCOLS], dt.float32, tag="ones_h")
        lvl_consts = {}
        cdma = []
        cdma.append((mask48[:], aps["c_mask48"]))
        cdma.append((ones_h[:], aps["c_ones"]))
        for lvl, L in LVL.items():
            NL = NB * L["EW"]
            cu = cpool.tile([P, 2 * NL], dt.int16, tag=f"cu16_{lvl}",
                            name=f"cu16_{lvl}")
            mf = cpool.tile([P, (3 if lvl == 1 else 2) * NL], dt.float32,
                            tag=f"maskf_{lvl}", name=f"maskf_{lvl}")
            cdma.append((cu[:], aps[f"c_u16_l{lvl}"]))
            cdma.append((mf[:], aps[f"c_maskf_l{lvl}"]))
            cuv = cu[:].bitcast(dt.uint16)
            ioq2 = (mf[:][:, 2 * NL:3 * NL] if lvl == 1
                    else cuv[:, NL:2 * NL])
            lvl_consts[lvl] = (mf[:][:, 0:NL], mf[:][:, NL:2 * NL],
                               None, cu[:][:, 0:NL], ioq2)
            # (maskf, mask_cnt(f32), unused, ioG(i16), ioQ2)

        def _emit_consts():
            for dst, src_ap in cdma:
                nc.sync.dma_start(dst, src_ap)
            yield

        accs = {}
        for name in ("rgb", "p1", "p2", "hash", "l0a", "l0b", "l1a", "l1b"):
            accs[name] = cpool.tile([P, 1], dt.float32, tag=f"acc_{name}",
                                    name=f"acc_{name}")
            V.memset(accs[name][:], 0.0)

        spool = ctx.enter_context(tc.tile_pool(name="shared", bufs=1))
        s_sh = spool.tile([P, NBLK * 49], dt.float32, tag="s_sh")
        nc.sync.dma_start(_blk(s_sh[:], 49),
                          aps["sd"].rearrange("(b p) x -> p b x", p=P))
        radios = {l: spool.tile([P, NBLK * 49], dt.float32, tag=f"radio{l}",
                                name=f"radio{l}")
                  for l in (0, 1)}

        MRG = {0: dict(ME=V, ME2=V, EE=V),
               1: dict(ME=V, ME2=V, EE=V)}
        HEM = {
            "l0a": dict(SE=V, SE2=V, XE=V, EE=V, TE=V, FE=V),
            "l0b": dict(SE=V, SE2=V, XE=V, EE=V, TE=V, FE=G),
            "l1a": dict(SE=V, SE2=V, XE=V, EE=G, TE=G, FE=G, fchain=True),
            "l1b": dict(SE=V, SE2=V, XE=V, EE=G, TE=G, FE=G, fchain=True),
        }

        gens = []
        setup_pool = ctx.enter_context(tc.tile_pool(name="setup", bufs=1))
        setup_gen = _emit_setup(nc, setup_pool, s_sh, radios, accs, mask48[:],
                                aps)
        mouts = {}
        for lvl in (0, 1):
            if f"l{lvl}" not in parts:
                continue
            mouts[lvl] = {}
            mp = ctx.enter_context(tc.tile_pool(name=f"mrg{lvl}", bufs=1))
            gens.append(_emit_level_merge(nc, tc, mp, lvl, s_sh,
                                          aps[f"ps{lvl}"], aps[f"pw{lvl}"],
                                          mouts[lvl], MRG[lvl]))
        gens.append(setup_gen)
        gens.append(_emit_consts())
        for name, lvl, b0 in HALVES:
            if f"l{lvl}" not in parts:
                continue
            hp = ctx.enter_context(tc.tile_pool(name=name, bufs=1))
            gens.append(_emit_half(nc, hp, lvl, b0, s_sh, radios[lvl],
                                   mouts[lvl], lvl_consts[lvl], accs[name],
                                   HEM[name]))
        if "hash" in parts:
            for lvl in (0, 1):
                hp2 = ctx.enter_context(tc.tile_pool(name=f"hash{lvl}", bufs=1))
                gens.append(_emit_hash(nc, hp2, lvl, ones_h[:], accs["hash"],
                                       aps, first=(lvl == 0)))

        while gens:
            nxt = []
            for g in gens:
                try:
                    next(g)
                    nxt.append(g)
                except StopIteration:
                    pass
            gens = nxt

        with tc.tile_pool(name="fin", bufs=1) as pool:
            tot = pool.tile([P, 1], dt.float32, tag="tot")
            V.tensor_scalar(tot[:], accs["rgb"][:], W_RGB / (R * 3), None,
                            Alu.mult)
            for snm, lvl, _ in HALVES:
                V.scalar_tensor_tensor(tot[:], accs[snm][:],
                                       W_INTER / (R * (LVL[lvl]["X"] - 1)),
                                       tot[:], Alu.mult, Alu.add)
            V.scalar_tensor_tensor(tot[:], accs["p1"][:], W_DIST * 2.0 / R,
                                   tot[:], Alu.mult, Alu.add)
            V.scalar_tensor_tensor(tot[:], accs["p2"][:], W_DIST / (3.0 * R),
                                   tot[:], Alu.mult, Alu.add)
            V.scalar_tensor_tensor(tot[:], accs["hash"][:],
                                   W_HASH / (NUM_SEGMENTS * 2.0), tot[:],
                                   Alu.mult, Alu.add)
            res = pool.tile([1, 1], dt.float32, tag="res")
            G.tensor_reduce(res[:], tot[:], AX.C, Alu.add)
            nc.sync.dma_start(out_ap, res[:])


# ---------------- host side ----------------
_module_cache = {}


def _get_module():
    if "nc" not in _module_cache:
        _module_cache["nc"] = build_module()
    return _module_cache["nc"]


def shard_inputs(inputs):
    f32 = np.float32
    pd = np.ascontiguousarray(inputs["pd_rgbs"], f32)
    gt = np.ascontiguousarray(inputs["gt_rgbs"], f32)
    sd = np.ascontiguousarray(inputs["render_sdist"], f32)
    rw = np.ascontiguousarray(inputs["render_weights"], f32)
    ps0 = np.ascontiguousarray(inputs["prop_sdist_0"], f32)
    pw0 = np.ascontiguousarray(inputs["prop_weights_0"], f32)
    ps1 = np.ascontiguousarray(inputs["prop_sdist_1"], f32)
    pw1 = np.ascontiguousarray(inputs["prop_weights_1"], f32)
    hashes = {}
    for lvl in (0, 1):
        idx = np.asarray(inputs[f"enc_idx_{lvl}"]).astype(np.int32)
        emb = np.ascontiguousarray(inputs[f"enc_embds_{lvl}"], f32)
        idx_pad = np.full(M + 2 * HALO, -1, np.int32)
        idx_pad[HALO:HALO + M] = idx
        emb_pad = np.zeros((M + 2 * HALO, 2), f32)
        emb_pad[HALO:HALO + M] = emb
        hashes[lvl] = (idx_pad, emb_pad)

    consts = {}
    rep = lambda row: np.ascontiguousarray(np.tile(row, (P, 1)))
    for lvl, L in LVL.items():
        EW, QWS, X = L["EW"], L["QWS"], L["X"]
        NL = NB * EW
        NQ = NB * QWS
        ioG = np.concatenate([2 * np.arange(b * EW, (b + 1) * EW,
                                            dtype=np.uint16)
                              for b in range(NB)])
        # query dest: rank-1 + b*QWS; C' = C + 98b so fold +98b here:
        # ioQ2 = i+1 + b*QWS + 98b -> (ioQ2 - C')*qf - 1 = rank-1 + b*QWS
        ioQ2 = np.concatenate([np.arange(1, EW + 1, dtype=np.uint16)
                               + b * QWS + 98 * b for b in range(NB)])
        packed = np.concatenate([ioG, ioQ2]).astype(np.uint16)
        consts[f"c_u16_l{lvl}"] = rep(packed.view(np.int16))
        msk = np.ones(NL, f32)
        msk[::EW] = 0.0
        mcnt = np.ones(NL, f32)
        for b in range(NB):
            mcnt[b * EW] = b
        parts_ = [msk, mcnt]
        if lvl == 1:
            parts_.append(np.concatenate([np.arange(1, EW + 1, dtype=f32)
                                          + b * QWS + 98 * b
                                          for b in range(NB)]))
        consts[f"c_maskf_l{lvl}"] = rep(np.concatenate(parts_))
    m48 = np.ones(NBLK * 48, f32)
    m48[::48] = 0.0
    consts["c_mask48"] = rep(m48)
    consts["c_ones"] = rep(np.ones(HCOLS, f32))

    in_maps = []
    for c in range(N_CORES):
        r0 = c * RPC
        lo = c * MPC
        im = {
            "pd": pd[r0:r0 + RPC], "gt": gt[r0:r0 + RPC],
            "sd": sd[r0:r0 + RPC], "rw": rw[r0:r0 + RPC],
            "ps0": ps0[r0:r0 + RPC], "pw0": pw0[r0:r0 + RPC],
            "ps1": ps1[r0:r0 + RPC], "pw1": pw1[r0:r0 + RPC],
        }
        for lvl in (0, 1):
            idx_pad, emb_pad = hashes[lvl]
            im[f"hi{lvl}"] = np.ascontiguousarray(idx_pad[lo:lo + HSLICE])
            im[f"he{lvl}"] = np.ascontiguousarray(
                emb_pad[lo:lo + HSLICE].reshape(-1))
        im.update(consts)
        in_maps.append(im)
    return in_maps


def kernel(**inputs) -> np.ndarray:
    nc = _get_module()
    in_maps = shard_inputs(inputs)
    res = run_bass_kernel_spmd(nc, in_maps, core_ids=list(range(N_CORES)))
    total = np.float64(0.0)
    for r in res.results:
        total += np.float64(r["out"][0, 0])
    return np.float32(total)
